# revision 2
# baseline (speedup 1.0000x reference)
"""GATv2 (2-layer, GraphNorm, MLP head) on 8 Trainium2 NeuronCores.

Design (vs the v1 edge-tile/one-hot-matmul kernel):
- dst-per-partition layout: each dst node owns one SBUF partition slot; its
  incoming edges lie along the free dimension.  Softmax and the weighted sum
  become free-dim tensor_reduce ops — no one-hot matmuls, no xr edge gather.
- Destinations are sorted by (lo_degree, hi_degree) and packed into chunks of
  1024 (128 partitions x 8 cores) so the rectangular edge padding stays small.
  Consecutive chunks merge into "windows" that share one dma_gather pair,
  amortizing the ~1us SWDGE fixed cost per gather.
- att is folded into the node tables (xl'' = att*xl): since lrelu is
  positively homogeneous and  min(x, .2x) = Prelu_{alpha=5}(0.2x),  the
  per-channel score term att_c*lrelu(v_c) becomes a plain Prelu over
  channels permuted pos-first per head.  This kills one full-size DVE pass.
  The aggregation output is un-scaled by 1/att at the end.
- Tables are fp16 (DVE runs 2x on 16-bit); scores skip the segment-max
  (exp never overflows here), pad edges point at a -1e4 table row so their
  exp underflows to exactly 0.
- conv bias + GraphNorm fold into the following launch's affine.

5 launches: A (layer-0 transforms), B0 (conv0), C (norm0+relu+layer-1
transforms), B1 (conv1), E (norm1+relu+MLP head).  Host work between
launches is index prep + memory movement only.
"""

import hashlib
import numpy as np

import concourse.bass as bass  # noqa: F401
import concourse.bacc as bacc
import concourse.tile as tile
from concourse import mybir
from concourse import bass_utils
from concourse.masks import make_identity

F32 = mybir.dt.float32
F16 = mybir.dt.float16
I16 = mybir.dt.int16
AF = mybir.ActivationFunctionType
ALU = mybir.AluOpType
AX = mybir.AxisListType

N, IN, H, C, E = 50000, 128, 2, 64, 800000
HC = H * C  # 128
NEG_SLOPE = 0.2
EPS_GN = 1e-5
NCORES = 8
P = 128

NCHUNK = 49                  # chunks of 1024 dsts (128 per core x 8)
SHARD_SLOTS = NCHUNK * P     # 6272 dst slots per core
NLO = 31360                  # nodes [0, NLO) gathered from the lo table
ROWS_LO = NLO + 1            # row 0 = pad(-1e4), node n -> row n+1
ROWS_HI = 50000 - NLO + 177  # 18817: nodes NLO.. at row n+1-ROWS_LO, spares, pad
ROWS_TOT = ROWS_LO + ROWS_HI  # 50178
PADHI_IDX = ROWS_HI - 1      # hi-local index of the hi pad row
PAD_VAL = -1e4
SBUF_CAP = 96                # max G*(Dlo+Dhi) per window
G_MAX = 8

_cache = {}


# ----------------------------------------------------------------------------
# host-side planning
# ----------------------------------------------------------------------------

def _wrap_idx_multi(buf):
    """[8, n] int16 -> [8, 128, n//16]: idx i -> [i%16, i//16], tiled x8."""
    nc_, n = buf.shape
    w = buf.reshape(nc_, n // 16, 16).transpose(0, 2, 1)  # [8, 16, n/16]
    return np.tile(w, (1, 8, 1))                          # [8, 128, n/16]


def build_plan(edge_index):
    ei = np.asarray(edge_index).astype(np.int64)
    loop = np.arange(N, dtype=np.int64)
    src = np.concatenate([ei[0], loop])
    dst = np.concatenate([ei[1], loop])
    is_lo = src < NLO

    lo_deg = np.bincount(dst[is_lo], minlength=N)
    hi_deg = np.bincount(dst[~is_lo], minlength=N)

    # Chunk packing (lo_deg and hi_deg are independent Poissons, so no 1D
    # sort bins both): lo-sorted bands of 7 chunks, hi-sorted within a band.
    # All chunks of a band share Dlo, so window-merging within a band only
    # maxes the (sorted, adjacent) Dhi values.
    o1 = np.argsort(-lo_deg, kind="stable")
    BAND = 7 * 1024
    parts = []
    for b in range((N + BAND - 1) // BAND):
        band = o1[b * BAND:(b + 1) * BAND]
        parts.append(band[np.argsort(-hi_deg[band], kind="stable")])
    order = np.concatenate(parts)
    rank = np.empty(N, np.int64)
    rank[order] = np.arange(N)
    chunk = rank // 1024
    within = rank % 1024
    core_of = within // P
    part_of = within % P

    ld = np.zeros(NCHUNK * 1024, np.int64)
    hd = np.zeros(NCHUNK * 1024, np.int64)
    ld[: N] = lo_deg[order]
    hd[: N] = hi_deg[order]
    Dlo_c = np.maximum(ld.reshape(NCHUNK, 1024).max(1), 1)
    Dhi_c = np.maximum(hd.reshape(NCHUNK, 1024).max(1), 1)

    # windows: merge consecutive chunks (sorted desc, so maxes come first)
    windows = []  # (g0, G, Dlo, Dhi)
    g = 0
    while g < NCHUNK:
        Dl, Dh = int(Dlo_c[g]), int(Dhi_c[g])
        G = 1
        waste = 0
        while G < G_MAX and g + G < NCHUNK:
            nl = max(Dl, int(Dlo_c[g + G]))
            nh = max(Dh, int(Dhi_c[g + G]))
            if (G + 1) * (nl + nh) > SBUF_CAP:
                break
            nw = (G + 1) * (nl + nh) - sum(
                int(Dlo_c[g + k] + Dhi_c[g + k]) for k in range(G + 1))
            if nw > 4:
                break
            Dl, Dh = nl, nh
            waste = nw
            G += 1
        windows.append((g, G, Dl, Dh))
        g += G

    # per-core flat idx buffer layout: [w0-lo | w0-hi | w1-lo | ...]
    base_lo = np.zeros(NCHUNK, np.int64)   # indexed by chunk
    base_hi = np.zeros(NCHUNK, np.int64)
    w_of_chunk = np.zeros(NCHUNK, np.int64)
    glocal = np.zeros(NCHUNK, np.int64)
    Dlo_w = np.zeros(NCHUNK, np.int64)     # per chunk: its window's Dlo
    Dhi_w = np.zeros(NCHUNK, np.int64)
    tot = 0
    for wi, (g0, G, Dl, Dh) in enumerate(windows):
        for k in range(G):
            ch = g0 + k
            w_of_chunk[ch] = wi
            glocal[ch] = k
            Dlo_w[ch] = Dl
            Dhi_w[ch] = Dh
            base_lo[ch] = tot
            base_hi[ch] = tot + G * Dl * P
        tot += G * (Dl + Dh) * P
    TOTI = tot

    # pad template (per window/region), then scatter real edges
    tmpl = np.empty(TOTI, np.int16)
    off = 0
    for (g0, G, Dl, Dh) in windows:
        tmpl[off: off + G * Dl * P] = 0          # lo pad row
        off += G * Dl * P
        tmpl[off: off + G * Dh * P] = PADHI_IDX  # hi pad row
        off += G * Dh * P
    buf = np.tile(tmpl, (NCORES, 1))

    for side in (0, 1):  # 0 = lo, 1 = hi
        mask = is_lo if side == 0 else ~is_lo
        es = np.nonzero(mask)[0]
        d_e = dst[es]
        o2 = np.argsort(d_e, kind="stable")
        es = es[o2]
        d_e = d_e[o2]
        first = np.searchsorted(d_e, np.arange(N))
        j = np.arange(len(es)) - first[d_e]
        ch = chunk[d_e]
        Dr = (Dlo_w if side == 0 else Dhi_w)[ch]
        base = (base_lo if side == 0 else base_hi)[ch]
        t = glocal[ch] * Dr + j
        pos = base + t * P + part_of[d_e]
        val = src[es] + 1 if side == 0 else src[es] + 1 - ROWS_LO
        buf[core_of[d_e], pos] = val.astype(np.int16)

    idx16 = _wrap_idx_multi(buf)  # [8, 128, TOTI//16]

    # slot maps
    node_of_slot = np.full((NCORES, SHARD_SLOTS), -1, np.int64)
    slot = chunk * P + part_of
    node_of_slot[core_of, slot] = np.arange(N)
    row_of_slot = np.empty((NCORES, SHARD_SLOTS), np.int64)
    pad_mask = node_of_slot < 0
    row_of_slot[~pad_mask] = node_of_slot[~pad_mask] + 1
    row_of_slot[pad_mask] = 50001 + np.arange(pad_mask.sum())  # spare rows

    real = float(len(src))
    return {
        "windows": windows, "TOTI": TOTI, "idx16": idx16,
        "node_of_slot": node_of_slot, "row_of_slot": row_of_slot,
        "pad_factor": TOTI / real,
    }


# ----------------------------------------------------------------------------
# kernel builders
# ----------------------------------------------------------------------------

def _new_nc(nq=1):
    return bacc.Bacc("TRN2", target_bir_lowering=False, num_swdge_queues=nq)


def build_transform():
    """Launch A: xl'' = x @ Wl'' + bl'', xr'' = x @ Wr'' + br'' (fp16 out)."""
    nc = _new_nc()
    x = nc.dram_tensor("x", [SHARD_SLOTS, IN], F32, kind="ExternalInput")
    Wl = nc.dram_tensor("Wl", [IN, HC], F16, kind="ExternalInput")
    Wr = nc.dram_tensor("Wr", [IN, HC], F16, kind="ExternalInput")
    blr = nc.dram_tensor("blr", [P, HC], F32, kind="ExternalInput")
    brr = nc.dram_tensor("brr", [P, HC], F32, kind="ExternalInput")
    xl = nc.dram_tensor("xl", [SHARD_SLOTS, HC], F16, kind="ExternalOutput")
    xr = nc.dram_tensor("xr", [SHARD_SLOTS, HC], F16, kind="ExternalOutput")

    with tile.TileContext(nc) as tc:
        with (
            tc.tile_pool(name="const", bufs=1) as cpool,
            tc.tile_pool(name="sbuf", bufs=3) as pool,
            tc.tile_pool(name="psum", bufs=2, space="PSUM") as psum,
        ):
            ident = cpool.tile([P, P], F16)
            make_identity(nc, ident[:])
            wl_t = cpool.tile([IN, HC], F16)
            wr_t = cpool.tile([IN, HC], F16)
            bl_t = cpool.tile([P, HC], F32)
            br_t = cpool.tile([P, HC], F32)
            nc.sync.dma_start(out=wl_t[:], in_=Wl[:, :])
            nc.sync.dma_start(out=wr_t[:], in_=Wr[:, :])
            nc.sync.dma_start(out=bl_t[:], in_=blr[:, :])
            nc.sync.dma_start(out=br_t[:], in_=brr[:, :])

            for t in range(NCHUNK):
                xt = pool.tile([P, IN], F32, tag="xt")
                nc.sync.dma_start(out=xt[:], in_=x[t * P:(t + 1) * P, :])
                xt16 = pool.tile([P, IN], F16, tag="xt16")
                nc.vector.tensor_copy(out=xt16[:], in_=xt[:])
                xT_ps = psum.tile([P, P], F16, tag="xT")
                nc.tensor.transpose(xT_ps[:], xt16[:], ident[:])
                xT = pool.tile([P, P], F16, tag="xTs")
                nc.vector.tensor_copy(out=xT[:], in_=xT_ps[:])
                for (w_t, b_t, out_d, tag) in ((wl_t, bl_t, xl, "l"),
                                               (wr_t, br_t, xr, "r")):
                    ps = psum.tile([P, HC], F32, tag="mm" + tag)
                    nc.tensor.matmul(ps[:], xT[:], w_t[:], start=True, stop=True)
                    ot = pool.tile([P, HC], F16, tag="ot" + tag)
                    nc.vector.tensor_add(out=ot[:], in0=ps[:], in1=b_t[:])
                    nc.sync.dma_start(out=out_d[t * P:(t + 1) * P, :], in_=ot[:])
    nc.finalize()
    return nc


def build_conv(windows, TOTI, ranges):
    """Launch B: GATv2 conv, dst-per-partition layout.

    ranges = (p0, p1): count of positive-att channels per head (channels are
    host-permuted pos-first within each head).
    """
    p0, p1 = ranges
    nc = _new_nc(nq=4)
    tlo = nc.dram_tensor("tlo", [ROWS_LO, HC], F16, kind="ExternalInput")
    thi = nc.dram_tensor("thi", [ROWS_HI, HC], F16, kind="ExternalInput")
    xr_d = nc.dram_tensor("xr", [P, NCHUNK * HC], F16, kind="ExternalInput")
    idx_d = nc.dram_tensor("idx16", [P, TOTI // 16], I16, kind="ExternalInput")
    invatt = nc.dram_tensor("invatt", [P, HC], F32, kind="ExternalInput")
    onescol = nc.dram_tensor("onescol", [P, 1], F32, kind="ExternalInput")
    out_d = nc.dram_tensor("out", [P, NCHUNK * HC], F16, kind="ExternalOutput")
    stats = nc.dram_tensor("stats", [1, 2 * HC], F32, kind="ExternalOutput")

    # activation ranges in c-major space: head h occupies (c, h) columns;
    # pos channels are c < p_h.  (c0, clen, h, alpha)
    act_ranges = []
    for h, pp in ((0, p0), (1, p1)):
        if pp > 0:
            act_ranges.append((0, pp, h, NEG_SLOPE))
        if pp < C:
            act_ranges.append((pp, C - pp, h, 5.0))

    NW = len(windows)
    with tile.TileContext(nc) as tc:
        with (
            tc.tile_pool(name="const", bufs=1) as cpool,
            tc.tile_pool(name="gath", bufs=3) as gpool,
            tc.tile_pool(name="work", bufs=2) as pool,
            tc.tile_pool(name="oh", bufs=3) as ohpool,
            tc.tile_pool(name="pstat", bufs=1, space="PSUM") as pstat,
        ):
            inv_t = cpool.tile([P, HC], F32)
            ones_t = cpool.tile([P, 1], F32)
            acc = cpool.tile([P, 2 * HC], F32)
            nc.sync.dma_start(out=inv_t[:], in_=invatt[:, :])
            nc.sync.dma_start(out=ones_t[:], in_=onescol[:, :])
            nc.vector.memset(acc[:], 0.0)

            state = {}  # per-window live tiles

            def emit_load(i):
                g0, G, Dl, Dh = windows[i]
                nlo, nhi = G * Dl * P, G * Dh * P
                ioff = sum(w[1] * (w[2] + w[3]) * P for w in windows[:i]) // 16
                ilo = gpool.tile([P, nlo // 16], I16, tag="ilo")
                ihi = gpool.tile([P, nhi // 16], I16, tag="ihi")
                nc.sync.dma_start(out=ilo[:], in_=idx_d[:, ioff: ioff + nlo // 16])
                nc.sync.dma_start(
                    out=ihi[:], in_=idx_d[:, ioff + nlo // 16: ioff + (nlo + nhi) // 16])
                glo = gpool.tile([P, G * Dl, HC], F16, tag="glo")
                ghi = gpool.tile([P, G * Dh, HC], F16, tag="ghi")
                nc.gpsimd.dma_gather(glo[:], tlo[:, :], ilo[:], nlo, nlo, HC,
                                     single_packet=False,
                                     queue_num=(2 * i) % 4)
                nc.gpsimd.dma_gather(ghi[:], thi[:, :], ihi[:], nhi, nhi, HC,
                                     single_packet=False,
                                     queue_num=(2 * i + 1) % 4)
                xrw = gpool.tile([P, G, HC], F16, tag="xrw")
                nc.sync.dma_start(out=xrw[:], in_=xr_d[:, g0 * HC:(g0 + G) * HC])
                state[i] = {"glo": glo, "ghi": ghi, "xrw": xrw}

            def emit_add_prelu(i):
                g0, G, Dl, Dh = windows[i]
                st = state[i]
                for (reg, Dr) in (("lo", Dl), ("hi", Dh)):
                    xlg = st["g" + reg]
                    v = pool.tile([P, G * Dr, HC], F16, tag="v" + reg)
                    xr_b = st["xrw"][:].unsqueeze(2).broadcast_to([P, G, Dr, HC])
                    nc.vector.tensor_add(
                        out=v[:].rearrange("p (g d) c -> p g d c", g=G),
                        in0=xlg[:].rearrange("p (g d) c -> p g d c", g=G),
                        in1=xr_b)
                    vv = v[:].rearrange("p (g d) (c h) -> p g d c h",
                                        g=G, h=H)
                    for (c0, ln, h, alpha) in act_ranges:
                        sl = vv[:, :, :, c0:c0 + ln, h:h + 1]
                        scale = 1.0 if alpha == NEG_SLOPE else NEG_SLOPE
                        nc.scalar.activation(sl, sl, AF.Prelu, scale=scale,
                                             alpha=alpha)
                    st["v" + reg] = v

            def emit_scores(i):
                g0, G, Dl, Dh = windows[i]
                st = state[i]
                for (reg, Dr) in (("lo", Dl), ("hi", Dh)):
                    v = st["v" + reg]
                    vv = v[:].rearrange("p (g d) (c h) -> p g d c h",
                                        g=G, h=H)
                    cur = C
                    while cur > 1:  # pairwise tree; C is a power of two
                        half = cur // 2
                        nc.vector.tensor_tensor(
                            out=vv[:, :, :, 0:half, :],
                            in0=vv[:, :, :, 0:half, :],
                            in1=vv[:, :, :, half:cur, :], op=ALU.add)
                        cur = half
                    pex = pool.tile([P, G, Dr, H], F16, tag="pex" + reg)
                    nc.scalar.activation(
                        pex[:], vv[:, :, :, 0, :], AF.Exp)
                    st["pex" + reg] = pex

            def emit_main(i):
                g0, G, Dl, Dh = windows[i]
                st = state[i]
                den = pool.tile([P, G, H], F32, tag="den")
                nc.vector.tensor_reduce(
                    out=den[:],
                    in_=st["pexlo"][:].rearrange("p g d h -> p g h d"),
                    axis=AX.X, op=ALU.add)
                den2 = pool.tile([P, G, H], F32, tag="den2")
                nc.vector.tensor_reduce(
                    out=den2[:],
                    in_=st["pexhi"][:].rearrange("p g d h -> p g h d"),
                    axis=AX.X, op=ALU.add)
                nc.vector.tensor_add(out=den[:], in0=den[:], in1=den2[:])
                rec = pool.tile([P, G, H], F32, tag="rec")
                nc.vector.tensor_scalar(out=den[:], in0=den[:], scalar1=1e-16,
                                        scalar2=None, op0=ALU.add)
                nc.vector.reciprocal(out=rec[:], in_=den[:])
                for (reg, Dr) in (("lo", Dl), ("hi", Dh)):
                    xlg = st["g" + reg]
                    t_r = st["v" + reg]  # overwrite (dead after scores)
                    pex = st["pex" + reg]
                    p_b = pex[:].rearrange("p g d h -> p (g d) h") \
                        .unsqueeze(2).broadcast_to([P, G * Dr, C, H])
                    nc.vector.tensor_mul(
                        out=t_r[:].rearrange("p g (c h) -> p g c h", h=H),
                        in0=xlg[:].rearrange("p g (c h) -> p g c h", h=H),
                        in1=p_b)
                    # pairwise tree over D (odd tail folded into the front)
                    tv = t_r[:].rearrange("p (g d) c -> p g d c", g=G)
                    cur = Dr
                    while cur > 1:
                        half = cur // 2
                        rem = cur - 2 * half
                        if rem:
                            nc.vector.tensor_tensor(
                                out=tv[:, :, 0:rem, :],
                                in0=tv[:, :, 0:rem, :],
                                in1=tv[:, :, 2 * half:cur, :], op=ALU.add)
                        nc.vector.tensor_tensor(
                            out=tv[:, :, 0:half, :],
                            in0=tv[:, :, 0:half, :],
                            in1=tv[:, :, half:cur - rem, :], op=ALU.add)
                        cur = half
                osum = pool.tile([P, G, HC], F32, tag="osum")
                nc.vector.tensor_add(
                    out=osum[:],
                    in0=st["vlo"][:].rearrange("p (g d) c -> p g d c", g=G)
                    [:, :, 0, :],
                    in1=st["vhi"][:].rearrange("p (g d) c -> p g d c", g=G)
                    [:, :, 0, :])
                rec_b = rec[:].rearrange("p g h -> p g h").unsqueeze(2) \
                    .broadcast_to([P, G, C, H])
                nc.vector.tensor_mul(
                    out=osum[:].rearrange("p g (c h) -> p g c h", c=C),
                    in0=osum[:].rearrange("p g (c h) -> p g c h", c=C),
                    in1=rec_b)
                oh = ohpool.tile([P, G, HC], F16, tag="oh")
                inv_b = inv_t[:].unsqueeze(1).broadcast_to([P, G, HC])
                nc.vector.tensor_mul(out=oh[:], in0=osum[:], in1=inv_b)
                nc.sync.dma_start(out=out_d[:, g0 * HC:(g0 + G) * HC],
                                  in_=oh[:].rearrange("p g c -> p (g c)"))
                st["oh"] = oh
                for k in ("glo", "ghi", "vlo", "vhi", "pexlo", "pexhi"):
                    st.pop(k, None)

            def emit_stats(i):
                g0, G, Dl, Dh = windows[i]
                st = state.pop(i)
                oh = st["oh"]
                sq = pool.tile([P, G, HC], F32, tag="sq")
                nc.scalar.activation(sq[:], oh[:], AF.Square)
                s1 = pool.tile([P, HC], F32, tag="s1")
                nc.vector.tensor_reduce(
                    out=s1[:], in_=oh[:].rearrange("p g c -> p c g"),
                    axis=AX.X, op=ALU.add)
                nc.vector.tensor_add(out=acc[:, 0:HC], in0=acc[:, 0:HC],
                                     in1=s1[:])
                s2 = pool.tile([P, HC], F32, tag="s2")
                nc.vector.tensor_reduce(
                    out=s2[:], in_=sq[:].rearrange("p g c -> p c g"),
                    axis=AX.X, op=ALU.add)
                nc.vector.tensor_add(out=acc[:, HC:2 * HC], in0=acc[:, HC:2 * HC],
                                     in1=s2[:])

            # software-pipelined emission.  Per-iteration engine-queue order is
            # chosen so ACT's exp(i-1) precedes the 8 prelus(i) (else the DVE
            # wmults of window i-1 would stall ~5us behind them), and gathers
            # run one window ahead of their adds.
            emit_load(0)
            for i in range(NW + 2):
                if i + 1 < NW:
                    emit_load(i + 1)
                if 1 <= i <= NW:
                    emit_scores(i - 1)
                if i < NW:
                    emit_add_prelu(i)
                if 1 <= i <= NW:
                    emit_main(i - 1)
                if 2 <= i <= NW + 1:
                    emit_stats(i - 2)

            st_ps = pstat.tile([1, 2 * HC], F32, tag="st")
            nc.tensor.matmul(st_ps[:], ones_t[:], acc[:], start=True, stop=True)
            stt = pool.tile([1, 2 * HC], F32, tag="stt")
            nc.vector.tensor_copy(out=stt[:], in_=st_ps[:])
            nc.sync.dma_start(out=stats[:, :], in_=stt[:])
    nc.finalize()
    return nc


def _emit_norm_prelude(nc, cpool, pconst, stats, ones8, onesr, gamma, beta, ms,
                       biasr):
    """Common GraphNorm-affine computation with conv-bias folding.

    Returns (a_rep, b_rep): normalized = a_rep * o' + b_rep where o' is the
    bias-less conv output."""
    st8 = cpool.tile([NCORES, 2 * HC], F32)
    o8 = cpool.tile([NCORES, 1], F32)
    orow = cpool.tile([1, P], F32)
    g_t = cpool.tile([1, HC], F32)
    be_t = cpool.tile([1, HC], F32)
    ms_t = cpool.tile([1, HC], F32)
    bi_t = cpool.tile([1, HC], F32)
    nc.sync.dma_start(out=st8[:], in_=stats[:, :])
    nc.sync.dma_start(out=o8[:], in_=ones8[:, :])
    nc.sync.dma_start(out=orow[:], in_=onesr[:, :])
    nc.sync.dma_start(out=g_t[:], in_=gamma[:, :])
    nc.sync.dma_start(out=be_t[:], in_=beta[:, :])
    nc.sync.dma_start(out=ms_t[:], in_=ms[:, :])
    nc.sync.dma_start(out=bi_t[:], in_=biasr[:, :])

    sg_ps = pconst.tile([1, 2 * HC], F32, tag="sg")
    nc.tensor.matmul(sg_ps[:], o8[:], st8[:], start=True, stop=True)
    # mean_o = S1/N ; mean_y = mean_o + bias
    mean = cpool.tile([1, HC], F32)
    nc.vector.tensor_scalar(out=mean[:], in0=sg_ps[:, 0:HC],
                            scalar1=1.0 / N, scalar2=None, op0=ALU.mult)
    mean_y = cpool.tile([1, HC], F32)
    nc.vector.tensor_add(out=mean_y[:], in0=mean[:], in1=bi_t[:])
    # E[y^2] = S2/N + bias*(2*mean_o + bias)
    ey2 = cpool.tile([1, HC], F32)
    nc.vector.tensor_scalar(out=ey2[:], in0=sg_ps[:, HC:2 * HC],
                            scalar1=1.0 / N, scalar2=None, op0=ALU.mult)
    t1 = cpool.tile([1, HC], F32)
    nc.vector.tensor_scalar(out=t1[:], in0=mean[:], scalar1=2.0,
                            scalar2=None, op0=ALU.mult)
    nc.vector.tensor_add(out=t1[:], in0=t1[:], in1=bi_t[:])
    nc.vector.tensor_mul(out=t1[:], in0=t1[:], in1=bi_t[:])
    nc.vector.tensor_add(out=ey2[:], in0=ey2[:], in1=t1[:])
    # var = E[y^2] - ms*(2-ms)*mean_y^2
    two_minus = cpool.tile([1, HC], F32)
    nc.vector.tensor_scalar(out=two_minus[:], in0=ms_t[:], scalar1=-1.0,
                            scalar2=2.0, op0=ALU.mult, op1=ALU.add)
    msm = cpool.tile([1, HC], F32)
    nc.vector.tensor_mul(out=msm[:], in0=two_minus[:], in1=ms_t[:])
    m2 = cpool.tile([1, HC], F32)
    nc.vector.tensor_mul(out=m2[:], in0=mean_y[:], in1=mean_y[:])
    var = cpool.tile([1, HC], F32)
    nc.vector.tensor_mul(out=var[:], in0=m2[:], in1=msm[:])
    nc.vector.tensor_tensor(out=var[:], in0=ey2[:], in1=var[:],
                            op=ALU.subtract)
    nc.vector.tensor_scalar(out=var[:], in0=var[:], scalar1=EPS_GN,
                            scalar2=None, op0=ALU.add)
    sd = cpool.tile([1, HC], F32)
    nc.scalar.activation(sd[:], var[:], AF.Sqrt)
    rsd = cpool.tile([1, HC], F32)
    nc.vector.reciprocal(out=rsd[:], in_=sd[:])
    arow = cpool.tile([1, HC], F32)      # A = gamma * rsd
    nc.vector.tensor_mul(out=arow[:], in0=g_t[:], in1=rsd[:])
    brow = cpool.tile([1, HC], F32)      # B = beta - A*ms*mean_y
    nc.vector.tensor_mul(out=brow[:], in0=arow[:], in1=ms_t[:])
    nc.vector.tensor_mul(out=brow[:], in0=brow[:], in1=mean_y[:])
    nc.vector.tensor_tensor(out=brow[:], in0=be_t[:], in1=brow[:],
                            op=ALU.subtract)
    # fold: normalized = A*(o'+bias) + B = A*o' + (A*bias + B)
    b2row = cpool.tile([1, HC], F32)
    nc.vector.tensor_mul(out=b2row[:], in0=arow[:], in1=bi_t[:])
    nc.vector.tensor_add(out=b2row[:], in0=b2row[:], in1=brow[:])
    # broadcast to [P, HC]
    a_ps = pconst.tile([P, HC], F32, tag="arep")
    b_ps = pconst.tile([P, HC], F32, tag="brep")
    nc.tensor.matmul(a_ps[:], orow[:], arow[:], start=True, stop=True)
    nc.tensor.matmul(b_ps[:], orow[:], b2row[:], start=True, stop=True)
    a_rep = cpool.tile([P, HC], F32)
    b_rep = cpool.tile([P, HC], F32)
    nc.vector.tensor_copy(out=a_rep[:], in_=a_ps[:])
    nc.vector.tensor_copy(out=b_rep[:], in_=b_ps[:])
    return a_rep, b_rep


def build_norm_transform():
    """Launch C: h = relu(norm(out0+bias)); xl1'' = h@Wl1''+bl1''; xr1''."""
    nc = _new_nc()
    x = nc.dram_tensor("x", [P, NCHUNK * HC], F16, kind="ExternalInput")
    stats = nc.dram_tensor("stats", [NCORES, 2 * HC], F32, kind="ExternalInput")
    ones8 = nc.dram_tensor("ones8", [NCORES, 1], F32, kind="ExternalInput")
    onesr = nc.dram_tensor("onesr", [1, P], F32, kind="ExternalInput")
    gamma = nc.dram_tensor("gamma", [1, HC], F32, kind="ExternalInput")
    beta = nc.dram_tensor("beta", [1, HC], F32, kind="ExternalInput")
    ms = nc.dram_tensor("ms", [1, HC], F32, kind="ExternalInput")
    biasr = nc.dram_tensor("biasr", [1, HC], F32, kind="ExternalInput")
    Wl = nc.dram_tensor("Wl", [HC, HC], F16, kind="ExternalInput")
    Wr = nc.dram_tensor("Wr", [HC, HC], F16, kind="ExternalInput")
    blr = nc.dram_tensor("blr", [P, HC], F32, kind="ExternalInput")
    brr = nc.dram_tensor("brr", [P, HC], F32, kind="ExternalInput")
    xl = nc.dram_tensor("xl", [SHARD_SLOTS, HC], F16, kind="ExternalOutput")
    xr = nc.dram_tensor("xr", [SHARD_SLOTS, HC], F16, kind="ExternalOutput")

    with tile.TileContext(nc) as tc:
        with (
            tc.tile_pool(name="const", bufs=1) as cpool,
            tc.tile_pool(name="sbuf", bufs=3) as pool,
            tc.tile_pool(name="psum", bufs=2, space="PSUM") as psum,
            tc.tile_pool(name="pconst", bufs=1, space="PSUM") as pconst,
        ):
            ident = cpool.tile([P, P], F16)
            make_identity(nc, ident[:])
            wl_t = cpool.tile([HC, HC], F16)
            wr_t = cpool.tile([HC, HC], F16)
            bl_t = cpool.tile([P, HC], F32)
            br_t = cpool.tile([P, HC], F32)
            nc.sync.dma_start(out=wl_t[:], in_=Wl[:, :])
            nc.sync.dma_start(out=wr_t[:], in_=Wr[:, :])
            nc.sync.dma_start(out=bl_t[:], in_=blr[:, :])
            nc.sync.dma_start(out=br_t[:], in_=brr[:, :])
            a_rep, b_rep = _emit_norm_prelude(
                nc, cpool, pconst, stats, ones8, onesr, gamma, beta, ms, biasr)

            for t in range(NCHUNK):
                xt = pool.tile([P, HC], F16, tag="xt")
                nc.sync.dma_start(out=xt[:], in_=x[:, t * HC:(t + 1) * HC])
                hn = pool.tile([P, HC], F32, tag="hn")
                nc.vector.tensor_mul(out=hn[:], in0=xt[:], in1=a_rep[:])
                nc.vector.tensor_add(out=hn[:], in0=hn[:], in1=b_rep[:])
                hn16 = pool.tile([P, HC], F16, tag="hn16")
                nc.scalar.activation(hn16[:], hn[:], AF.Relu)
                xT_ps = psum.tile([P, P], F16, tag="xT")
                nc.tensor.transpose(xT_ps[:], hn16[:], ident[:])
                xT = pool.tile([P, P], F16, tag="xTs")
                nc.vector.tensor_copy(out=xT[:], in_=xT_ps[:])
                ps = psum.tile([P, 2 * HC], F32, tag="mm")
                nc.tensor.matmul(ps[:, 0:HC], xT[:], wl_t[:], start=True,
                                 stop=True)
                nc.tensor.matmul(ps[:, HC:2 * HC], xT[:], wr_t[:], start=True,
                                 stop=True)
                for (b_t, out_dd, sl, tag) in ((bl_t, xl, slice(0, HC), "l"),
                                               (br_t, xr, slice(HC, 2 * HC), "r")):
                    ot = pool.tile([P, HC], F16, tag="ot" + tag)
                    nc.vector.tensor_add(out=ot[:], in0=ps[:, sl], in1=b_t[:])
                    nc.sync.dma_start(out=out_dd[t * P:(t + 1) * P, :], in_=ot[:])
    nc.finalize()
    return nc


def build_norm_mlp():
    """Launch E: h = relu(norm(out1+bias)); y = relu(h@W1+b1)@W2+b2."""
    nc = _new_nc()
    x = nc.dram_tensor("x", [P, NCHUNK * HC], F16, kind="ExternalInput")
    stats = nc.dram_tensor("stats", [NCORES, 2 * HC], F32, kind="ExternalInput")
    ones8 = nc.dram_tensor("ones8", [NCORES, 1], F32, kind="ExternalInput")
    onesr = nc.dram_tensor("onesr", [1, P], F32, kind="ExternalInput")
    gamma = nc.dram_tensor("gamma", [1, HC], F32, kind="ExternalInput")
    beta = nc.dram_tensor("beta", [1, HC], F32, kind="ExternalInput")
    ms = nc.dram_tensor("ms", [1, HC], F32, kind="ExternalInput")
    biasr = nc.dram_tensor("biasr", [1, HC], F32, kind="ExternalInput")
    W1 = nc.dram_tensor("W1", [HC, C], F16, kind="ExternalInput")
    b1r = nc.dram_tensor("b1r", [P, C], F32, kind="ExternalInput")
    W2 = nc.dram_tensor("W2", [C, 2], F16, kind="ExternalInput")
    b2r = nc.dram_tensor("b2r", [P, 2], F32, kind="ExternalInput")
    y = nc.dram_tensor("y", [SHARD_SLOTS, 2], F32, kind="ExternalOutput")

    with tile.TileContext(nc) as tc:
        with (
            tc.tile_pool(name="const", bufs=1) as cpool,
            tc.tile_pool(name="sbuf", bufs=3) as pool,
            tc.tile_pool(name="psum", bufs=2, space="PSUM") as psum,
            tc.tile_pool(name="pone", bufs=1, space="PSUM") as pone,
            tc.tile_pool(name="pconst", bufs=1, space="PSUM") as pconst,
        ):
            ident = cpool.tile([P, P], F16)
            make_identity(nc, ident[:])
            w1_t = cpool.tile([HC, C], F16)
            b1_t = cpool.tile([P, C], F32)
            w2_t = cpool.tile([C, 2], F16)
            b2_t = cpool.tile([P, 2], F32)
            nc.sync.dma_start(out=w1_t[:], in_=W1[:, :])
            nc.sync.dma_start(out=b1_t[:], in_=b1r[:, :])
            nc.sync.dma_start(out=w2_t[:], in_=W2[:, :])
            nc.sync.dma_start(out=b2_t[:], in_=b2r[:, :])
            a_rep, b_rep = _emit_norm_prelude(
                nc, cpool, pconst, stats, ones8, onesr, gamma, beta, ms, biasr)

            for t in range(NCHUNK):
                xt = pool.tile([P, HC], F16, tag="xt")
                nc.sync.dma_start(out=xt[:], in_=x[:, t * HC:(t + 1) * HC])
                hn = pool.tile([P, HC], F32, tag="hn")
                nc.vector.tensor_mul(out=hn[:], in0=xt[:], in1=a_rep[:])
                nc.vector.tensor_add(out=hn[:], in0=hn[:], in1=b_rep[:])
                hn16 = pool.tile([P, HC], F16, tag="hn16")
                nc.scalar.activation(hn16[:], hn[:], AF.Relu)
                xT_ps = psum.tile([P, P], F16, tag="xT")
                nc.tensor.transpose(xT_ps[:], hn16[:], ident[:])
                xT = pool.tile([P, P], F16, tag="xTs")
                nc.vector.tensor_copy(out=xT[:], in_=xT_ps[:])
                z_ps = pone.tile([P, C], F32, tag="z")
                nc.tensor.matmul(z_ps[:], xT[:], w1_t[:], start=True, stop=True)
                z = pool.tile([P, C], F32, tag="zs")
                nc.vector.tensor_add(out=z[:], in0=z_ps[:], in1=b1_t[:])
                z16 = pool.tile([P, C], F16, tag="z16")
                nc.scalar.activation(z16[:], z[:], AF.Relu)
                zT_ps = pone.tile([C, P], F16, tag="zT")
                nc.tensor.transpose(zT_ps[:], z16[:], ident[:])
                zT = pool.tile([C, P], F16, tag="zTs")
                nc.vector.tensor_copy(out=zT[:], in_=zT_ps[:])
                y_ps = pone.tile([P, 2], F32, tag="y")
                nc.tensor.matmul(y_ps[:], zT[:], w2_t[:], start=True, stop=True)
                yt = pool.tile([P, 2], F32, tag="yt")
                nc.vector.tensor_add(out=yt[:], in0=y_ps[:], in1=b2_t[:])
                nc.sync.dma_start(out=y[t * P:(t + 1) * P, :], in_=yt[:])
    nc.finalize()
    return nc


# ----------------------------------------------------------------------------
# host orchestration
# ----------------------------------------------------------------------------

TRACE = False
LAST_EXEC_NS = []


def _run(nc, in_maps, trace=None):
    trace = TRACE if trace is None else trace
    last_err = None
    for attempt in range(3):
        try:
            res = bass_utils.run_bass_kernel_spmd(
                nc, in_maps, core_ids=list(range(NCORES)), trace=trace)
            LAST_EXEC_NS.append(res.exec_time_ns)
            return res
        except Exception as e:
            last_err = e
            import time as _t
            _t.sleep(2.0 * (attempt + 1))
    raise last_err


def _rep(v):
    v = np.asarray(v, np.float32).reshape(1, -1)
    return np.tile(v, (P, 1))


def _head_perm(att):
    """Channel order: c-major head-interleaved (col = c*H + h), pos-att-first
    within each head.  Keeps the innermost stride of per-(edge,head)-scalar
    broadcasts at 1 so DVE 2x applies.  Returns (perm, (p0, p1))."""
    att = np.asarray(att, np.float32).reshape(H, C)
    heads = []
    counts = []
    for h in range(H):
        pos = np.nonzero(att[h] > 0)[0]
        neg = np.nonzero(att[h] <= 0)[0]
        heads.append(np.concatenate([pos, neg]) + h * C)
        counts.append(len(pos))
    perm = np.empty(HC, np.int64)
    for c in range(C):
        for h in range(H):
            perm[c * H + h] = heads[h][c]
    return perm, tuple(counts)


def _assemble_table(xl_shards, row_of_slot):
    tbl = np.empty((ROWS_TOT, HC), np.float16)
    tbl[0] = PAD_VAL
    tbl[ROWS_TOT - 1] = PAD_VAL
    allrows = row_of_slot.reshape(-1)
    tbl[allrows] = np.concatenate(xl_shards, axis=0)
    return tbl


def kernel(**inputs):
    LAST_EXEC_NS.clear()
    x = np.asarray(inputs["x"], np.float32)
    edge_index = np.asarray(inputs["edge_index"])
    key = hashlib.sha1(np.ascontiguousarray(edge_index).tobytes()).hexdigest()

    att0 = np.asarray(inputs["att0"], np.float32).reshape(-1)
    att1 = np.asarray(inputs["att1"], np.float32).reshape(-1)
    pi0, r0 = _head_perm(att0)
    pi1, r1 = _head_perm(att1)

    if _cache.get("edge_key") != key:
        plan = build_plan(edge_index)
        _cache.clear()
        _cache["edge_key"] = key
        _cache["plan"] = plan
        _cache["ncA"] = build_transform()
        _cache["ncC"] = build_norm_transform()
        _cache["ncE"] = build_norm_mlp()
    plan = _cache["plan"]
    if _cache.get("r0") != r0:
        _cache["ncB0"] = build_conv(plan["windows"], plan["TOTI"], r0)
        _cache["r0"] = r0
    if _cache.get("r1") != r1:
        if r1 == r0:
            _cache["ncB1"] = _cache["ncB0"]
        else:
            _cache["ncB1"] = build_conv(plan["windows"], plan["TOTI"], r1)
        _cache["r1"] = r1
    ncA, ncB0, ncC, ncB1, ncE = (_cache["ncA"], _cache["ncB0"], _cache["ncC"],
                                 _cache["ncB1"], _cache["ncE"])

    node_of_slot = plan["node_of_slot"]
    row_of_slot = plan["row_of_slot"]

    # ---- host weight prep (channel perms + att folding) ----
    a0p = att0[pi0]
    a1p = att1[pi1]
    inv0 = _rep(1.0 / a0p)
    inv1 = _rep(1.0 / a1p)

    Wl0 = (np.asarray(inputs["Wl0"], np.float32)[:, pi0] * a0p).astype(np.float16)
    Wr0 = (np.asarray(inputs["Wr0"], np.float32)[:, pi0] * a0p).astype(np.float16)
    bl0 = np.asarray(inputs["bl0"], np.float32)[pi0] * a0p
    br0 = np.asarray(inputs["br0"], np.float32)[pi0] * a0p
    # layer-1 weights: rows in pi0 space (h lives there), cols pi1+att1-scaled
    Wl1 = (np.asarray(inputs["Wl1"], np.float32)[pi0][:, pi1] * a1p).astype(np.float16)
    Wr1 = (np.asarray(inputs["Wr1"], np.float32)[pi0][:, pi1] * a1p).astype(np.float16)
    bl1 = np.asarray(inputs["bl1"], np.float32)[pi1] * a1p
    br1 = np.asarray(inputs["br1"], np.float32)[pi1] * a1p
    W1 = np.asarray(inputs["W1"], np.float32)[pi1].astype(np.float16)
    b1 = np.asarray(inputs["b1"], np.float32)
    W2 = np.asarray(inputs["W2"], np.float32).astype(np.float16)
    b2 = np.asarray(inputs["b2"], np.float32)

    g0 = np.asarray(inputs["g0"], np.float32)[pi0].reshape(1, HC)
    be0 = np.asarray(inputs["be0"], np.float32)[pi0].reshape(1, HC)
    ms0 = np.asarray(inputs["ms0"], np.float32)[pi0].reshape(1, HC)
    bias0 = np.asarray(inputs["bias0"], np.float32)[pi0].reshape(1, HC)
    g1 = np.asarray(inputs["g1"], np.float32)[pi1].reshape(1, HC)
    be1 = np.asarray(inputs["be1"], np.float32)[pi1].reshape(1, HC)
    ms1 = np.asarray(inputs["ms1"], np.float32)[pi1].reshape(1, HC)
    bias1 = np.asarray(inputs["bias1"], np.float32)[pi1].reshape(1, HC)

    ones8 = np.ones((NCORES, 1), np.float32)
    onesr = np.ones((1, P), np.float32)
    onescol = np.ones((P, 1), np.float32)

    # ---- launch A: layer-0 transforms ----
    x_slots = [x[np.clip(node_of_slot[ci], 0, N - 1)] for ci in range(NCORES)]
    in_maps = [{"x": x_slots[ci], "Wl": Wl0, "Wr": Wr0,
                "blr": _rep(bl0), "brr": _rep(br0)} for ci in range(NCORES)]
    resA = _run(ncA, in_maps)
    xl_sh = [resA.results[ci]["xl"] for ci in range(NCORES)]
    xr_sh = [resA.results[ci]["xr"] for ci in range(NCORES)]

    def conv(ncB, xl_shards, xr_shards, inv):
        tbl = _assemble_table(xl_shards, row_of_slot)
        tlo = np.ascontiguousarray(tbl[:ROWS_LO])
        thi = np.ascontiguousarray(tbl[ROWS_LO:])
        in_maps = []
        for ci in range(NCORES):
            xr_pm = np.ascontiguousarray(
                xr_shards[ci].reshape(NCHUNK, P, HC).transpose(1, 0, 2)
                .reshape(P, NCHUNK * HC))
            in_maps.append({
                "tlo": tlo, "thi": thi, "xr": xr_pm,
                "idx16": plan["idx16"][ci], "invatt": inv,
                "onescol": onescol,
            })
        res = _run(ncB, in_maps)
        outs = [res.results[ci]["out"] for ci in range(NCORES)]
        stats = np.concatenate([res.results[ci]["stats"] for ci in range(NCORES)],
                               axis=0)
        return outs, stats

    out0, stats0 = conv(ncB0, xl_sh, xr_sh, inv0)

    # ---- launch C: norm0 + relu + layer-1 transforms ----
    in_maps = [{"x": out0[ci], "stats": stats0, "ones8": ones8, "onesr": onesr,
                "gamma": g0, "beta": be0, "ms": ms0, "biasr": bias0,
                "Wl": Wl1, "Wr": Wr1, "blr": _rep(bl1), "brr": _rep(br1)}
               for ci in range(NCORES)]
    resC = _run(ncC, in_maps)
    xl1_sh = [resC.results[ci]["xl"] for ci in range(NCORES)]
    xr1_sh = [resC.results[ci]["xr"] for ci in range(NCORES)]

    out1, stats1 = conv(ncB1, xl1_sh, xr1_sh, inv1)

    # ---- launch E: norm1 + relu + MLP ----
    in_maps = [{"x": out1[ci], "stats": stats1, "ones8": ones8, "onesr": onesr,
                "gamma": g1, "beta": be1, "ms": ms1, "biasr": bias1,
                "W1": W1, "b1r": _rep(b1), "W2": W2, "b2r": _rep(b2)}
               for ci in range(NCORES)]
    resE = _run(ncE, in_maps)

    y = np.empty((N, 2), np.float32)
    for ci in range(NCORES):
        valid = node_of_slot[ci] >= 0
        y[node_of_slot[ci][valid]] = resE.results[ci]["y"][valid]
    return y


# revision 3
# speedup vs baseline: 1.2440x; 1.2440x over previous
"""GATv2 (2-layer, GraphNorm, MLP head) on 8 Trainium2 NeuronCores — v2.

Design (vs the v1 edge-tile/one-hot-matmul kernel):
- dst-per-partition layout: each dst node owns one SBUF partition slot; its
  incoming edges lie along the free dimension.  Softmax and the weighted sum
  become free-dim tensor_reduce ops — no one-hot matmuls, no xr edge gather.
- Destinations are sorted by (lo_degree, hi_degree) and packed into chunks of
  1024 (128 partitions x 8 cores) so the rectangular edge padding stays small.
  Consecutive chunks merge into "windows" that share one dma_gather pair,
  amortizing the ~1us SWDGE fixed cost per gather.
- att is folded into the node tables (xl'' = att*xl): since lrelu is
  positively homogeneous and  min(x, .2x) = Prelu_{alpha=5}(0.2x),  the
  per-channel score term att_c*lrelu(v_c) becomes a plain Prelu over
  channels permuted pos-first per head.  This kills one full-size DVE pass.
  The aggregation output is un-scaled by 1/att at the end.
- Tables are fp16 (DVE runs 2x on 16-bit); scores skip the segment-max
  (exp never overflows here), pad edges point at a -1e4 table row so their
  exp underflows to exactly 0.
- conv bias + GraphNorm fold into the following launch's affine.

5 launches: A (layer-0 transforms), B0 (conv0), C (norm0+relu+layer-1
transforms), B1 (conv1), E (norm1+relu+MLP head).  Host work between
launches is index prep + memory movement only.
"""

import hashlib
import numpy as np

import concourse.bass as bass  # noqa: F401
import concourse.bacc as bacc
import concourse.tile as tile
from concourse import mybir
from concourse import bass_utils
from concourse.masks import make_identity

F32 = mybir.dt.float32
F16 = mybir.dt.float16
I16 = mybir.dt.int16
AF = mybir.ActivationFunctionType
ALU = mybir.AluOpType
AX = mybir.AxisListType

N, IN, H, C, E = 50000, 128, 2, 64, 800000
HC = H * C  # 128
NEG_SLOPE = 0.2
EPS_GN = 1e-5
NCORES = 8
P = 128

NCHUNK = 49                  # chunks of 1024 dsts (128 per core x 8)
SHARD_SLOTS = NCHUNK * P     # 6272 dst slots per core
NLO = 31360                  # nodes [0, NLO) gathered from the lo table
ROWS_LO = NLO + 1            # row 0 = pad(-1e4), node n -> row n+1
ROWS_HI = 50000 - NLO + 177  # 18817: nodes NLO.. at row n+1-ROWS_LO, spares, pad
ROWS_TOT = ROWS_LO + ROWS_HI  # 50178
PADHI_IDX = ROWS_HI - 1      # hi-local index of the hi pad row
PAD_VAL = -1e4
SBUF_CAP = 96                # max G*(Dlo+Dhi) per window
G_MAX = 8

_cache = {}


# ----------------------------------------------------------------------------
# host-side planning
# ----------------------------------------------------------------------------

def _wrap_idx_multi(buf):
    """[8, n] int16 -> [8, 128, n//16]: idx i -> [i%16, i//16], tiled x8."""
    nc_, n = buf.shape
    w = buf.reshape(nc_, n // 16, 16).transpose(0, 2, 1)  # [8, 16, n/16]
    return np.tile(w, (1, 8, 1))                          # [8, 128, n/16]


def build_plan(edge_index):
    ei = np.asarray(edge_index).astype(np.int64)
    loop = np.arange(N, dtype=np.int64)
    src = np.concatenate([ei[0], loop])
    dst = np.concatenate([ei[1], loop])
    is_lo = src < NLO

    lo_deg = np.bincount(dst[is_lo], minlength=N)
    hi_deg = np.bincount(dst[~is_lo], minlength=N)

    # Chunk packing (lo_deg and hi_deg are independent Poissons, so no 1D
    # sort bins both): lo-sorted bands of 7 chunks, hi-sorted within a band.
    # All chunks of a band share Dlo, so window-merging within a band only
    # maxes the (sorted, adjacent) Dhi values.
    o1 = np.argsort(-lo_deg, kind="stable")
    BAND = 7 * 1024
    parts = []
    for b in range((N + BAND - 1) // BAND):
        band = o1[b * BAND:(b + 1) * BAND]
        parts.append(band[np.argsort(-hi_deg[band], kind="stable")])
    order = np.concatenate(parts)
    rank = np.empty(N, np.int64)
    rank[order] = np.arange(N)
    chunk = rank // 1024
    within = rank % 1024
    core_of = within // P
    part_of = within % P

    ld = np.zeros(NCHUNK * 1024, np.int64)
    hd = np.zeros(NCHUNK * 1024, np.int64)
    ld[: N] = lo_deg[order]
    hd[: N] = hi_deg[order]
    Dlo_c = np.maximum(ld.reshape(NCHUNK, 1024).max(1), 1)
    Dhi_c = np.maximum(hd.reshape(NCHUNK, 1024).max(1), 1)

    # windows: merge consecutive chunks (sorted desc, so maxes come first)
    windows = []  # (g0, G, Dlo, Dhi)
    g = 0
    while g < NCHUNK:
        Dl, Dh = int(Dlo_c[g]), int(Dhi_c[g])
        G = 1
        waste = 0
        while G < G_MAX and g + G < NCHUNK:
            nl = max(Dl, int(Dlo_c[g + G]))
            nh = max(Dh, int(Dhi_c[g + G]))
            if (G + 1) * (nl + nh) > SBUF_CAP:
                break
            nw = (G + 1) * (nl + nh) - sum(
                int(Dlo_c[g + k] + Dhi_c[g + k]) for k in range(G + 1))
            if nw > 4:
                break
            Dl, Dh = nl, nh
            waste = nw
            G += 1
        windows.append((g, G, Dl, Dh))
        g += G

    # per-core flat idx buffer layout: [w0-lo | w0-hi | w1-lo | ...]
    base_lo = np.zeros(NCHUNK, np.int64)   # indexed by chunk
    base_hi = np.zeros(NCHUNK, np.int64)
    w_of_chunk = np.zeros(NCHUNK, np.int64)
    glocal = np.zeros(NCHUNK, np.int64)
    Dlo_w = np.zeros(NCHUNK, np.int64)     # per chunk: its window's Dlo
    Dhi_w = np.zeros(NCHUNK, np.int64)
    tot = 0
    for wi, (g0, G, Dl, Dh) in enumerate(windows):
        for k in range(G):
            ch = g0 + k
            w_of_chunk[ch] = wi
            glocal[ch] = k
            Dlo_w[ch] = Dl
            Dhi_w[ch] = Dh
            base_lo[ch] = tot
            base_hi[ch] = tot + G * Dl * P
        tot += G * (Dl + Dh) * P
    TOTI = tot

    # pad template (per window/region), then scatter real edges
    tmpl = np.empty(TOTI, np.int16)
    off = 0
    for (g0, G, Dl, Dh) in windows:
        tmpl[off: off + G * Dl * P] = 0          # lo pad row
        off += G * Dl * P
        tmpl[off: off + G * Dh * P] = PADHI_IDX  # hi pad row
        off += G * Dh * P
    buf = np.tile(tmpl, (NCORES, 1))

    for side in (0, 1):  # 0 = lo, 1 = hi
        mask = is_lo if side == 0 else ~is_lo
        es = np.nonzero(mask)[0]
        d_e = dst[es]
        o2 = np.argsort(d_e, kind="stable")
        es = es[o2]
        d_e = d_e[o2]
        first = np.searchsorted(d_e, np.arange(N))
        j = np.arange(len(es)) - first[d_e]
        ch = chunk[d_e]
        Dr = (Dlo_w if side == 0 else Dhi_w)[ch]
        base = (base_lo if side == 0 else base_hi)[ch]
        t = glocal[ch] * Dr + j
        pos = base + t * P + part_of[d_e]
        val = src[es] + 1 if side == 0 else src[es] + 1 - ROWS_LO
        buf[core_of[d_e], pos] = val.astype(np.int16)

    idx16 = _wrap_idx_multi(buf)  # [8, 128, TOTI//16]

    # slot maps
    node_of_slot = np.full((NCORES, SHARD_SLOTS), -1, np.int64)
    slot = chunk * P + part_of
    node_of_slot[core_of, slot] = np.arange(N)
    row_of_slot = np.empty((NCORES, SHARD_SLOTS), np.int64)
    pad_mask = node_of_slot < 0
    row_of_slot[~pad_mask] = node_of_slot[~pad_mask] + 1
    row_of_slot[pad_mask] = 50001 + np.arange(pad_mask.sum())  # spare rows

    real = float(len(src))
    return {
        "windows": windows, "TOTI": TOTI, "idx16": idx16,
        "node_of_slot": node_of_slot, "row_of_slot": row_of_slot,
        "pad_factor": TOTI / real,
    }


# ----------------------------------------------------------------------------
# kernel builders
# ----------------------------------------------------------------------------

def _new_nc(nq=1):
    return bacc.Bacc("TRN2", target_bir_lowering=False, num_swdge_queues=nq)


def build_transform():
    """Launch A: xl'' = x @ Wl'' + bl'', xr'' = x @ Wr'' + br'' (fp16 out)."""
    nc = _new_nc()
    x = nc.dram_tensor("x", [SHARD_SLOTS, IN], F32, kind="ExternalInput")
    Wl = nc.dram_tensor("Wl", [IN, HC], F16, kind="ExternalInput")
    Wr = nc.dram_tensor("Wr", [IN, HC], F16, kind="ExternalInput")
    blr = nc.dram_tensor("blr", [P, HC], F32, kind="ExternalInput")
    brr = nc.dram_tensor("brr", [P, HC], F32, kind="ExternalInput")
    xl = nc.dram_tensor("xl", [SHARD_SLOTS, HC], F16, kind="ExternalOutput")
    xr = nc.dram_tensor("xr", [SHARD_SLOTS, HC], F16, kind="ExternalOutput")

    with tile.TileContext(nc) as tc:
        with (
            tc.tile_pool(name="const", bufs=1) as cpool,
            tc.tile_pool(name="sbuf", bufs=3) as pool,
            tc.tile_pool(name="psum", bufs=2, space="PSUM") as psum,
        ):
            ident = cpool.tile([P, P], F16)
            make_identity(nc, ident[:])
            wl_t = cpool.tile([IN, HC], F16)
            wr_t = cpool.tile([IN, HC], F16)
            bl_t = cpool.tile([P, HC], F32)
            br_t = cpool.tile([P, HC], F32)
            nc.sync.dma_start(out=wl_t[:], in_=Wl[:, :])
            nc.sync.dma_start(out=wr_t[:], in_=Wr[:, :])
            nc.sync.dma_start(out=bl_t[:], in_=blr[:, :])
            nc.sync.dma_start(out=br_t[:], in_=brr[:, :])

            for t in range(NCHUNK):
                xt = pool.tile([P, IN], F32, tag="xt")
                nc.sync.dma_start(out=xt[:], in_=x[t * P:(t + 1) * P, :])
                xt16 = pool.tile([P, IN], F16, tag="xt16")
                nc.vector.tensor_copy(out=xt16[:], in_=xt[:])
                xT_ps = psum.tile([P, P], F16, tag="xT")
                nc.tensor.transpose(xT_ps[:], xt16[:], ident[:])
                xT = pool.tile([P, P], F16, tag="xTs")
                nc.vector.tensor_copy(out=xT[:], in_=xT_ps[:])
                for (w_t, b_t, out_d, tag) in ((wl_t, bl_t, xl, "l"),
                                               (wr_t, br_t, xr, "r")):
                    ps = psum.tile([P, HC], F32, tag="mm" + tag)
                    nc.tensor.matmul(ps[:], xT[:], w_t[:], start=True, stop=True)
                    ot = pool.tile([P, HC], F16, tag="ot" + tag)
                    nc.vector.tensor_add(out=ot[:], in0=ps[:], in1=b_t[:])
                    nc.sync.dma_start(out=out_d[t * P:(t + 1) * P, :], in_=ot[:])
    nc.finalize()
    return nc


def build_conv(windows, TOTI, ranges):
    """Launch B: GATv2 conv, dst-per-partition layout.

    ranges = (p0, p1): count of positive-att channels per head (channels are
    host-permuted pos-first within each head).
    """
    p0, p1 = ranges
    nc = _new_nc(nq=4)
    tlo = nc.dram_tensor("tlo", [ROWS_LO, HC], F16, kind="ExternalInput")
    thi = nc.dram_tensor("thi", [ROWS_HI, HC], F16, kind="ExternalInput")
    xr_d = nc.dram_tensor("xr", [P, NCHUNK * HC], F16, kind="ExternalInput")
    idx_d = nc.dram_tensor("idx16", [P, TOTI // 16], I16, kind="ExternalInput")
    invatt = nc.dram_tensor("invatt", [P, HC], F32, kind="ExternalInput")
    onescol = nc.dram_tensor("onescol", [P, 1], F32, kind="ExternalInput")
    out_d = nc.dram_tensor("out", [P, NCHUNK * HC], F16, kind="ExternalOutput")
    stats = nc.dram_tensor("stats", [1, 2 * HC], F32, kind="ExternalOutput")

    # activation ranges in c-major space: head h occupies (c, h) columns;
    # pos channels are c < p_h.  (c0, clen, h, alpha)
    act_ranges = []
    for h, pp in ((0, p0), (1, p1)):
        if pp > 0:
            act_ranges.append((0, pp, h, NEG_SLOPE))
        if pp < C:
            act_ranges.append((pp, C - pp, h, 5.0))

    NW = len(windows)
    with tile.TileContext(nc) as tc:
        with (
            tc.tile_pool(name="const", bufs=1) as cpool,
            tc.tile_pool(name="gath", bufs=3) as gpool,
            tc.tile_pool(name="work", bufs=2) as pool,
            tc.tile_pool(name="oh", bufs=3) as ohpool,
            tc.tile_pool(name="pstat", bufs=1, space="PSUM") as pstat,
        ):
            inv_t = cpool.tile([P, HC], F32)
            ones_t = cpool.tile([P, 1], F32)
            acc = cpool.tile([P, 2 * HC], F32)
            nc.sync.dma_start(out=inv_t[:], in_=invatt[:, :])
            nc.sync.dma_start(out=ones_t[:], in_=onescol[:, :])
            nc.vector.memset(acc[:], 0.0)

            state = {}  # per-window live tiles

            def emit_load(i):
                g0, G, Dl, Dh = windows[i]
                nlo, nhi = G * Dl * P, G * Dh * P
                ioff = sum(w[1] * (w[2] + w[3]) * P for w in windows[:i]) // 16
                ilo = gpool.tile([P, nlo // 16], I16, tag="ilo")
                ihi = gpool.tile([P, nhi // 16], I16, tag="ihi")
                nc.sync.dma_start(out=ilo[:], in_=idx_d[:, ioff: ioff + nlo // 16])
                nc.sync.dma_start(
                    out=ihi[:], in_=idx_d[:, ioff + nlo // 16: ioff + (nlo + nhi) // 16])
                glo = gpool.tile([P, G * Dl, HC], F16, tag="glo")
                ghi = gpool.tile([P, G * Dh, HC], F16, tag="ghi")
                nc.gpsimd.dma_gather(glo[:], tlo[:, :], ilo[:], nlo, nlo, HC,
                                     single_packet=False,
                                     queue_num=(2 * i) % 4)
                nc.gpsimd.dma_gather(ghi[:], thi[:, :], ihi[:], nhi, nhi, HC,
                                     single_packet=False,
                                     queue_num=(2 * i + 1) % 4)
                xrw = gpool.tile([P, G, HC], F16, tag="xrw")
                nc.sync.dma_start(out=xrw[:], in_=xr_d[:, g0 * HC:(g0 + G) * HC])
                state[i] = {"glo": glo, "ghi": ghi, "xrw": xrw}

            def emit_add_prelu(i):
                g0, G, Dl, Dh = windows[i]
                st = state[i]
                for (reg, Dr) in (("lo", Dl), ("hi", Dh)):
                    xlg = st["g" + reg]
                    v = pool.tile([P, G * Dr, HC], F16, tag="v" + reg)
                    xr_b = st["xrw"][:].unsqueeze(2).broadcast_to([P, G, Dr, HC])
                    nc.vector.tensor_add(
                        out=v[:].rearrange("p (g d) c -> p g d c", g=G),
                        in0=xlg[:].rearrange("p (g d) c -> p g d c", g=G),
                        in1=xr_b)
                    vv = v[:].rearrange("p (g d) (c h) -> p g d c h",
                                        g=G, h=H)
                    for (c0, ln, h, alpha) in act_ranges:
                        sl = vv[:, :, :, c0:c0 + ln, h:h + 1]
                        scale = 1.0 if alpha == NEG_SLOPE else NEG_SLOPE
                        nc.scalar.activation(sl, sl, AF.Prelu, scale=scale,
                                             alpha=alpha)
                    st["v" + reg] = v

            def emit_scores(i):
                g0, G, Dl, Dh = windows[i]
                st = state[i]
                for (reg, Dr) in (("lo", Dl), ("hi", Dh)):
                    v = st["v" + reg]
                    vv = v[:].rearrange("p (g d) (c h) -> p g d c h",
                                        g=G, h=H)
                    cur = C
                    while cur > 1:  # pairwise tree; C is a power of two
                        half = cur // 2
                        nc.vector.tensor_tensor(
                            out=vv[:, :, :, 0:half, :],
                            in0=vv[:, :, :, 0:half, :],
                            in1=vv[:, :, :, half:cur, :], op=ALU.add)
                        cur = half
                    pex = pool.tile([P, G, Dr, H], F16, tag="pex" + reg)
                    nc.scalar.activation(
                        pex[:], vv[:, :, :, 0, :], AF.Exp)
                    st["pex" + reg] = pex

            def emit_main(i):
                g0, G, Dl, Dh = windows[i]
                st = state[i]
                den = pool.tile([P, G, H], F32, tag="den")
                nc.vector.tensor_reduce(
                    out=den[:],
                    in_=st["pexlo"][:].rearrange("p g d h -> p g h d"),
                    axis=AX.X, op=ALU.add)
                den2 = pool.tile([P, G, H], F32, tag="den2")
                nc.vector.tensor_reduce(
                    out=den2[:],
                    in_=st["pexhi"][:].rearrange("p g d h -> p g h d"),
                    axis=AX.X, op=ALU.add)
                nc.vector.tensor_add(out=den[:], in0=den[:], in1=den2[:])
                rec = pool.tile([P, G, H], F32, tag="rec")
                nc.vector.tensor_scalar(out=den[:], in0=den[:], scalar1=1e-16,
                                        scalar2=None, op0=ALU.add)
                nc.vector.reciprocal(out=rec[:], in_=den[:])
                for (reg, Dr) in (("lo", Dl), ("hi", Dh)):
                    xlg = st["g" + reg]
                    t_r = st["v" + reg]  # overwrite (dead after scores)
                    pex = st["pex" + reg]
                    p_b = pex[:].rearrange("p g d h -> p (g d) h") \
                        .unsqueeze(2).broadcast_to([P, G * Dr, C, H])
                    nc.vector.tensor_mul(
                        out=t_r[:].rearrange("p g (c h) -> p g c h", h=H),
                        in0=xlg[:].rearrange("p g (c h) -> p g c h", h=H),
                        in1=p_b)
                    # pairwise tree over D (odd tail folded into the front)
                    tv = t_r[:].rearrange("p (g d) c -> p g d c", g=G)
                    cur = Dr
                    while cur > 1:
                        half = cur // 2
                        rem = cur - 2 * half
                        if rem:
                            nc.vector.tensor_tensor(
                                out=tv[:, :, 0:rem, :],
                                in0=tv[:, :, 0:rem, :],
                                in1=tv[:, :, 2 * half:cur, :], op=ALU.add)
                        nc.vector.tensor_tensor(
                            out=tv[:, :, 0:half, :],
                            in0=tv[:, :, 0:half, :],
                            in1=tv[:, :, half:cur - rem, :], op=ALU.add)
                        cur = half
                osum = pool.tile([P, G, HC], F32, tag="osum")
                nc.vector.tensor_add(
                    out=osum[:],
                    in0=st["vlo"][:].rearrange("p (g d) c -> p g d c", g=G)
                    [:, :, 0, :],
                    in1=st["vhi"][:].rearrange("p (g d) c -> p g d c", g=G)
                    [:, :, 0, :])
                rec_b = rec[:].rearrange("p g h -> p g h").unsqueeze(2) \
                    .broadcast_to([P, G, C, H])
                nc.vector.tensor_mul(
                    out=osum[:].rearrange("p g (c h) -> p g c h", c=C),
                    in0=osum[:].rearrange("p g (c h) -> p g c h", c=C),
                    in1=rec_b)
                oh = ohpool.tile([P, G, HC], F16, tag="oh")
                inv_b = inv_t[:].unsqueeze(1).broadcast_to([P, G, HC])
                nc.vector.tensor_mul(out=oh[:], in0=osum[:], in1=inv_b)
                nc.sync.dma_start(out=out_d[:, g0 * HC:(g0 + G) * HC],
                                  in_=oh[:].rearrange("p g c -> p (g c)"))
                st["oh"] = oh
                for k in ("glo", "ghi", "vlo", "vhi", "pexlo", "pexhi"):
                    st.pop(k, None)

            def emit_stats(i):
                g0, G, Dl, Dh = windows[i]
                st = state.pop(i)
                oh = st["oh"]
                sq = pool.tile([P, G, HC], F32, tag="sq")
                nc.scalar.activation(sq[:], oh[:], AF.Square)
                s1 = pool.tile([P, HC], F32, tag="s1")
                nc.vector.tensor_reduce(
                    out=s1[:], in_=oh[:].rearrange("p g c -> p c g"),
                    axis=AX.X, op=ALU.add)
                nc.vector.tensor_add(out=acc[:, 0:HC], in0=acc[:, 0:HC],
                                     in1=s1[:])
                s2 = pool.tile([P, HC], F32, tag="s2")
                nc.vector.tensor_reduce(
                    out=s2[:], in_=sq[:].rearrange("p g c -> p c g"),
                    axis=AX.X, op=ALU.add)
                nc.vector.tensor_add(out=acc[:, HC:2 * HC], in0=acc[:, HC:2 * HC],
                                     in1=s2[:])

            # software-pipelined emission.  Per-iteration engine-queue order is
            # chosen so ACT's exp(i-1) precedes the 8 prelus(i) (else the DVE
            # wmults of window i-1 would stall ~5us behind them), and gathers
            # run one window ahead of their adds.
            emit_load(0)
            for i in range(NW + 2):
                if i + 1 < NW:
                    emit_load(i + 1)
                if 1 <= i <= NW:
                    emit_scores(i - 1)
                if i < NW:
                    emit_add_prelu(i)
                if 1 <= i <= NW:
                    emit_main(i - 1)
                if 2 <= i <= NW + 1:
                    emit_stats(i - 2)

            st_ps = pstat.tile([1, 2 * HC], F32, tag="st")
            nc.tensor.matmul(st_ps[:], ones_t[:], acc[:], start=True, stop=True)
            stt = pool.tile([1, 2 * HC], F32, tag="stt")
            nc.vector.tensor_copy(out=stt[:], in_=st_ps[:])
            nc.sync.dma_start(out=stats[:, :], in_=stt[:])
    nc.finalize()
    return nc


def _emit_norm_prelude(nc, cpool, pconst, stats, ones8, onesr, gamma, beta, ms,
                       biasr):
    """Common GraphNorm-affine computation with conv-bias folding.

    Returns (a_rep, b_rep): normalized = a_rep * o' + b_rep where o' is the
    bias-less conv output."""
    st8 = cpool.tile([NCORES, 2 * HC], F32)
    o8 = cpool.tile([NCORES, 1], F32)
    orow = cpool.tile([1, P], F32)
    g_t = cpool.tile([1, HC], F32)
    be_t = cpool.tile([1, HC], F32)
    ms_t = cpool.tile([1, HC], F32)
    bi_t = cpool.tile([1, HC], F32)
    nc.sync.dma_start(out=st8[:], in_=stats[:, :])
    nc.sync.dma_start(out=o8[:], in_=ones8[:, :])
    nc.sync.dma_start(out=orow[:], in_=onesr[:, :])
    nc.sync.dma_start(out=g_t[:], in_=gamma[:, :])
    nc.sync.dma_start(out=be_t[:], in_=beta[:, :])
    nc.sync.dma_start(out=ms_t[:], in_=ms[:, :])
    nc.sync.dma_start(out=bi_t[:], in_=biasr[:, :])

    sg_ps = pconst.tile([1, 2 * HC], F32, tag="sg")
    nc.tensor.matmul(sg_ps[:], o8[:], st8[:], start=True, stop=True)
    # mean_o = S1/N ; mean_y = mean_o + bias
    mean = cpool.tile([1, HC], F32)
    nc.vector.tensor_scalar(out=mean[:], in0=sg_ps[:, 0:HC],
                            scalar1=1.0 / N, scalar2=None, op0=ALU.mult)
    mean_y = cpool.tile([1, HC], F32)
    nc.vector.tensor_add(out=mean_y[:], in0=mean[:], in1=bi_t[:])
    # E[y^2] = S2/N + bias*(2*mean_o + bias)
    ey2 = cpool.tile([1, HC], F32)
    nc.vector.tensor_scalar(out=ey2[:], in0=sg_ps[:, HC:2 * HC],
                            scalar1=1.0 / N, scalar2=None, op0=ALU.mult)
    t1 = cpool.tile([1, HC], F32)
    nc.vector.tensor_scalar(out=t1[:], in0=mean[:], scalar1=2.0,
                            scalar2=None, op0=ALU.mult)
    nc.vector.tensor_add(out=t1[:], in0=t1[:], in1=bi_t[:])
    nc.vector.tensor_mul(out=t1[:], in0=t1[:], in1=bi_t[:])
    nc.vector.tensor_add(out=ey2[:], in0=ey2[:], in1=t1[:])
    # var = E[y^2] - ms*(2-ms)*mean_y^2
    two_minus = cpool.tile([1, HC], F32)
    nc.vector.tensor_scalar(out=two_minus[:], in0=ms_t[:], scalar1=-1.0,
                            scalar2=2.0, op0=ALU.mult, op1=ALU.add)
    msm = cpool.tile([1, HC], F32)
    nc.vector.tensor_mul(out=msm[:], in0=two_minus[:], in1=ms_t[:])
    m2 = cpool.tile([1, HC], F32)
    nc.vector.tensor_mul(out=m2[:], in0=mean_y[:], in1=mean_y[:])
    var = cpool.tile([1, HC], F32)
    nc.vector.tensor_mul(out=var[:], in0=m2[:], in1=msm[:])
    nc.vector.tensor_tensor(out=var[:], in0=ey2[:], in1=var[:],
                            op=ALU.subtract)
    nc.vector.tensor_scalar(out=var[:], in0=var[:], scalar1=EPS_GN,
                            scalar2=None, op0=ALU.add)
    sd = cpool.tile([1, HC], F32)
    nc.scalar.activation(sd[:], var[:], AF.Sqrt)
    rsd = cpool.tile([1, HC], F32)
    nc.vector.reciprocal(out=rsd[:], in_=sd[:])
    arow = cpool.tile([1, HC], F32)      # A = gamma * rsd
    nc.vector.tensor_mul(out=arow[:], in0=g_t[:], in1=rsd[:])
    brow = cpool.tile([1, HC], F32)      # B = beta - A*ms*mean_y
    nc.vector.tensor_mul(out=brow[:], in0=arow[:], in1=ms_t[:])
    nc.vector.tensor_mul(out=brow[:], in0=brow[:], in1=mean_y[:])
    nc.vector.tensor_tensor(out=brow[:], in0=be_t[:], in1=brow[:],
                            op=ALU.subtract)
    # fold: normalized = A*(o'+bias) + B = A*o' + (A*bias + B)
    b2row = cpool.tile([1, HC], F32)
    nc.vector.tensor_mul(out=b2row[:], in0=arow[:], in1=bi_t[:])
    nc.vector.tensor_add(out=b2row[:], in0=b2row[:], in1=brow[:])
    # broadcast to [P, HC]
    a_ps = pconst.tile([P, HC], F32, tag="arep")
    b_ps = pconst.tile([P, HC], F32, tag="brep")
    nc.tensor.matmul(a_ps[:], orow[:], arow[:], start=True, stop=True)
    nc.tensor.matmul(b_ps[:], orow[:], b2row[:], start=True, stop=True)
    a_rep = cpool.tile([P, HC], F32)
    b_rep = cpool.tile([P, HC], F32)
    nc.vector.tensor_copy(out=a_rep[:], in_=a_ps[:])
    nc.vector.tensor_copy(out=b_rep[:], in_=b_ps[:])
    return a_rep, b_rep


def build_norm_transform():
    """Launch C: h = relu(norm(out0+bias)); xl1'' = h@Wl1''+bl1''; xr1''."""
    nc = _new_nc()
    x = nc.dram_tensor("x", [P, NCHUNK * HC], F16, kind="ExternalInput")
    stats = nc.dram_tensor("stats", [NCORES, 2 * HC], F32, kind="ExternalInput")
    ones8 = nc.dram_tensor("ones8", [NCORES, 1], F32, kind="ExternalInput")
    onesr = nc.dram_tensor("onesr", [1, P], F32, kind="ExternalInput")
    gamma = nc.dram_tensor("gamma", [1, HC], F32, kind="ExternalInput")
    beta = nc.dram_tensor("beta", [1, HC], F32, kind="ExternalInput")
    ms = nc.dram_tensor("ms", [1, HC], F32, kind="ExternalInput")
    biasr = nc.dram_tensor("biasr", [1, HC], F32, kind="ExternalInput")
    Wl = nc.dram_tensor("Wl", [HC, HC], F16, kind="ExternalInput")
    Wr = nc.dram_tensor("Wr", [HC, HC], F16, kind="ExternalInput")
    blr = nc.dram_tensor("blr", [P, HC], F32, kind="ExternalInput")
    brr = nc.dram_tensor("brr", [P, HC], F32, kind="ExternalInput")
    xl = nc.dram_tensor("xl", [SHARD_SLOTS, HC], F16, kind="ExternalOutput")
    xr = nc.dram_tensor("xr", [SHARD_SLOTS, HC], F16, kind="ExternalOutput")

    with tile.TileContext(nc) as tc:
        with (
            tc.tile_pool(name="const", bufs=1) as cpool,
            tc.tile_pool(name="sbuf", bufs=3) as pool,
            tc.tile_pool(name="psum", bufs=2, space="PSUM") as psum,
            tc.tile_pool(name="pconst", bufs=1, space="PSUM") as pconst,
        ):
            ident = cpool.tile([P, P], F16)
            make_identity(nc, ident[:])
            wl_t = cpool.tile([HC, HC], F16)
            wr_t = cpool.tile([HC, HC], F16)
            bl_t = cpool.tile([P, HC], F32)
            br_t = cpool.tile([P, HC], F32)
            nc.sync.dma_start(out=wl_t[:], in_=Wl[:, :])
            nc.sync.dma_start(out=wr_t[:], in_=Wr[:, :])
            nc.sync.dma_start(out=bl_t[:], in_=blr[:, :])
            nc.sync.dma_start(out=br_t[:], in_=brr[:, :])
            a_rep, b_rep = _emit_norm_prelude(
                nc, cpool, pconst, stats, ones8, onesr, gamma, beta, ms, biasr)

            for t in range(NCHUNK):
                xt = pool.tile([P, HC], F16, tag="xt")
                nc.sync.dma_start(out=xt[:], in_=x[:, t * HC:(t + 1) * HC])
                hn = pool.tile([P, HC], F32, tag="hn")
                nc.vector.tensor_mul(out=hn[:], in0=xt[:], in1=a_rep[:])
                nc.vector.tensor_add(out=hn[:], in0=hn[:], in1=b_rep[:])
                hn16 = pool.tile([P, HC], F16, tag="hn16")
                nc.scalar.activation(hn16[:], hn[:], AF.Relu)
                xT_ps = psum.tile([P, P], F16, tag="xT")
                nc.tensor.transpose(xT_ps[:], hn16[:], ident[:])
                xT = pool.tile([P, P], F16, tag="xTs")
                nc.vector.tensor_copy(out=xT[:], in_=xT_ps[:])
                ps = psum.tile([P, 2 * HC], F32, tag="mm")
                nc.tensor.matmul(ps[:, 0:HC], xT[:], wl_t[:], start=True,
                                 stop=True)
                nc.tensor.matmul(ps[:, HC:2 * HC], xT[:], wr_t[:], start=True,
                                 stop=True)
                for (b_t, out_dd, sl, tag) in ((bl_t, xl, slice(0, HC), "l"),
                                               (br_t, xr, slice(HC, 2 * HC), "r")):
                    ot = pool.tile([P, HC], F16, tag="ot" + tag)
                    nc.vector.tensor_add(out=ot[:], in0=ps[:, sl], in1=b_t[:])
                    nc.sync.dma_start(out=out_dd[t * P:(t + 1) * P, :], in_=ot[:])
    nc.finalize()
    return nc


def build_norm_mlp():
    """Launch E: h = relu(norm(out1+bias)); y = relu(h@W1+b1)@W2+b2."""
    nc = _new_nc()
    x = nc.dram_tensor("x", [P, NCHUNK * HC], F16, kind="ExternalInput")
    stats = nc.dram_tensor("stats", [NCORES, 2 * HC], F32, kind="ExternalInput")
    ones8 = nc.dram_tensor("ones8", [NCORES, 1], F32, kind="ExternalInput")
    onesr = nc.dram_tensor("onesr", [1, P], F32, kind="ExternalInput")
    gamma = nc.dram_tensor("gamma", [1, HC], F32, kind="ExternalInput")
    beta = nc.dram_tensor("beta", [1, HC], F32, kind="ExternalInput")
    ms = nc.dram_tensor("ms", [1, HC], F32, kind="ExternalInput")
    biasr = nc.dram_tensor("biasr", [1, HC], F32, kind="ExternalInput")
    W1 = nc.dram_tensor("W1", [HC, C], F16, kind="ExternalInput")
    b1r = nc.dram_tensor("b1r", [P, C], F32, kind="ExternalInput")
    W2 = nc.dram_tensor("W2", [C, 2], F16, kind="ExternalInput")
    b2r = nc.dram_tensor("b2r", [P, 2], F32, kind="ExternalInput")
    y = nc.dram_tensor("y", [SHARD_SLOTS, 2], F32, kind="ExternalOutput")

    with tile.TileContext(nc) as tc:
        with (
            tc.tile_pool(name="const", bufs=1) as cpool,
            tc.tile_pool(name="sbuf", bufs=3) as pool,
            tc.tile_pool(name="psum", bufs=2, space="PSUM") as psum,
            tc.tile_pool(name="pone", bufs=1, space="PSUM") as pone,
            tc.tile_pool(name="pconst", bufs=1, space="PSUM") as pconst,
        ):
            ident = cpool.tile([P, P], F16)
            make_identity(nc, ident[:])
            w1_t = cpool.tile([HC, C], F16)
            b1_t = cpool.tile([P, C], F32)
            w2_t = cpool.tile([C, 2], F16)
            b2_t = cpool.tile([P, 2], F32)
            nc.sync.dma_start(out=w1_t[:], in_=W1[:, :])
            nc.sync.dma_start(out=b1_t[:], in_=b1r[:, :])
            nc.sync.dma_start(out=w2_t[:], in_=W2[:, :])
            nc.sync.dma_start(out=b2_t[:], in_=b2r[:, :])
            a_rep, b_rep = _emit_norm_prelude(
                nc, cpool, pconst, stats, ones8, onesr, gamma, beta, ms, biasr)

            for t in range(NCHUNK):
                xt = pool.tile([P, HC], F16, tag="xt")
                nc.sync.dma_start(out=xt[:], in_=x[:, t * HC:(t + 1) * HC])
                hn = pool.tile([P, HC], F32, tag="hn")
                nc.vector.tensor_mul(out=hn[:], in0=xt[:], in1=a_rep[:])
                nc.vector.tensor_add(out=hn[:], in0=hn[:], in1=b_rep[:])
                hn16 = pool.tile([P, HC], F16, tag="hn16")
                nc.scalar.activation(hn16[:], hn[:], AF.Relu)
                xT_ps = psum.tile([P, P], F16, tag="xT")
                nc.tensor.transpose(xT_ps[:], hn16[:], ident[:])
                xT = pool.tile([P, P], F16, tag="xTs")
                nc.vector.tensor_copy(out=xT[:], in_=xT_ps[:])
                z_ps = pone.tile([P, C], F32, tag="z")
                nc.tensor.matmul(z_ps[:], xT[:], w1_t[:], start=True, stop=True)
                z = pool.tile([P, C], F32, tag="zs")
                nc.vector.tensor_add(out=z[:], in0=z_ps[:], in1=b1_t[:])
                z16 = pool.tile([P, C], F16, tag="z16")
                nc.scalar.activation(z16[:], z[:], AF.Relu)
                zT_ps = pone.tile([C, P], F16, tag="zT")
                nc.tensor.transpose(zT_ps[:], z16[:], ident[:])
                zT = pool.tile([C, P], F16, tag="zTs")
                nc.vector.tensor_copy(out=zT[:], in_=zT_ps[:])
                y_ps = pone.tile([P, 2], F32, tag="y")
                nc.tensor.matmul(y_ps[:], zT[:], w2_t[:], start=True, stop=True)
                yt = pool.tile([P, 2], F32, tag="yt")
                nc.vector.tensor_add(out=yt[:], in0=y_ps[:], in1=b2_t[:])
                nc.sync.dma_start(out=y[t * P:(t + 1) * P, :], in_=yt[:])
    nc.finalize()
    return nc


# ----------------------------------------------------------------------------
# host orchestration
# ----------------------------------------------------------------------------

TRACE = False
LAST_EXEC_NS = []


def _run(nc, in_maps, trace=None):
    trace = TRACE if trace is None else trace
    last_err = None
    for attempt in range(3):
        try:
            res = bass_utils.run_bass_kernel_spmd(
                nc, in_maps, core_ids=list(range(NCORES)), trace=trace)
            LAST_EXEC_NS.append(res.exec_time_ns)
            return res
        except Exception as e:
            last_err = e
            import time as _t
            _t.sleep(2.0 * (attempt + 1))
    raise last_err


def _rep(v):
    v = np.asarray(v, np.float32).reshape(1, -1)
    return np.tile(v, (P, 1))


def _head_perm(att):
    """Channel order: c-major head-interleaved (col = c*H + h), pos-att-first
    within each head.  Keeps the innermost stride of per-(edge,head)-scalar
    broadcasts at 1 so DVE 2x applies.  Returns (perm, (p0, p1))."""
    att = np.asarray(att, np.float32).reshape(H, C)
    heads = []
    counts = []
    for h in range(H):
        pos = np.nonzero(att[h] > 0)[0]
        neg = np.nonzero(att[h] <= 0)[0]
        heads.append(np.concatenate([pos, neg]) + h * C)
        counts.append(len(pos))
    perm = np.empty(HC, np.int64)
    for c in range(C):
        for h in range(H):
            perm[c * H + h] = heads[h][c]
    return perm, tuple(counts)


def _assemble_table(xl_shards, row_of_slot):
    tbl = np.empty((ROWS_TOT, HC), np.float16)
    tbl[0] = PAD_VAL
    tbl[ROWS_TOT - 1] = PAD_VAL
    allrows = row_of_slot.reshape(-1)
    tbl[allrows] = np.concatenate(xl_shards, axis=0)
    return tbl


def kernel(**inputs):
    LAST_EXEC_NS.clear()
    x = np.asarray(inputs["x"], np.float32)
    edge_index = np.asarray(inputs["edge_index"])
    key = hashlib.sha1(np.ascontiguousarray(edge_index).tobytes()).hexdigest()

    att0 = np.asarray(inputs["att0"], np.float32).reshape(-1)
    att1 = np.asarray(inputs["att1"], np.float32).reshape(-1)
    pi0, r0 = _head_perm(att0)
    pi1, r1 = _head_perm(att1)

    if _cache.get("edge_key") != key:
        plan = build_plan(edge_index)
        _cache.clear()
        _cache["edge_key"] = key
        _cache["plan"] = plan
        _cache["ncA"] = build_transform()
        _cache["ncC"] = build_norm_transform()
        _cache["ncE"] = build_norm_mlp()
    plan = _cache["plan"]
    if _cache.get("r0") != r0:
        _cache["ncB0"] = build_conv(plan["windows"], plan["TOTI"], r0)
        _cache["r0"] = r0
    if _cache.get("r1") != r1:
        if r1 == r0:
            _cache["ncB1"] = _cache["ncB0"]
        else:
            _cache["ncB1"] = build_conv(plan["windows"], plan["TOTI"], r1)
        _cache["r1"] = r1
    ncA, ncB0, ncC, ncB1, ncE = (_cache["ncA"], _cache["ncB0"], _cache["ncC"],
                                 _cache["ncB1"], _cache["ncE"])

    node_of_slot = plan["node_of_slot"]
    row_of_slot = plan["row_of_slot"]

    # ---- host weight prep (channel perms + att folding) ----
    a0p = att0[pi0]
    a1p = att1[pi1]
    inv0 = _rep(1.0 / a0p)
    inv1 = _rep(1.0 / a1p)

    Wl0 = (np.asarray(inputs["Wl0"], np.float32)[:, pi0] * a0p).astype(np.float16)
    Wr0 = (np.asarray(inputs["Wr0"], np.float32)[:, pi0] * a0p).astype(np.float16)
    bl0 = np.asarray(inputs["bl0"], np.float32)[pi0] * a0p
    br0 = np.asarray(inputs["br0"], np.float32)[pi0] * a0p
    # layer-1 weights: rows in pi0 space (h lives there), cols pi1+att1-scaled
    Wl1 = (np.asarray(inputs["Wl1"], np.float32)[pi0][:, pi1] * a1p).astype(np.float16)
    Wr1 = (np.asarray(inputs["Wr1"], np.float32)[pi0][:, pi1] * a1p).astype(np.float16)
    bl1 = np.asarray(inputs["bl1"], np.float32)[pi1] * a1p
    br1 = np.asarray(inputs["br1"], np.float32)[pi1] * a1p
    W1 = np.asarray(inputs["W1"], np.float32)[pi1].astype(np.float16)
    b1 = np.asarray(inputs["b1"], np.float32)
    W2 = np.asarray(inputs["W2"], np.float32).astype(np.float16)
    b2 = np.asarray(inputs["b2"], np.float32)

    g0 = np.asarray(inputs["g0"], np.float32)[pi0].reshape(1, HC)
    be0 = np.asarray(inputs["be0"], np.float32)[pi0].reshape(1, HC)
    ms0 = np.asarray(inputs["ms0"], np.float32)[pi0].reshape(1, HC)
    bias0 = np.asarray(inputs["bias0"], np.float32)[pi0].reshape(1, HC)
    g1 = np.asarray(inputs["g1"], np.float32)[pi1].reshape(1, HC)
    be1 = np.asarray(inputs["be1"], np.float32)[pi1].reshape(1, HC)
    ms1 = np.asarray(inputs["ms1"], np.float32)[pi1].reshape(1, HC)
    bias1 = np.asarray(inputs["bias1"], np.float32)[pi1].reshape(1, HC)

    ones8 = np.ones((NCORES, 1), np.float32)
    onesr = np.ones((1, P), np.float32)
    onescol = np.ones((P, 1), np.float32)

    # ---- launch A: layer-0 transforms ----
    x_slots = [x[np.clip(node_of_slot[ci], 0, N - 1)] for ci in range(NCORES)]
    in_maps = [{"x": x_slots[ci], "Wl": Wl0, "Wr": Wr0,
                "blr": _rep(bl0), "brr": _rep(br0)} for ci in range(NCORES)]
    resA = _run(ncA, in_maps)
    xl_sh = [resA.results[ci]["xl"] for ci in range(NCORES)]
    xr_sh = [resA.results[ci]["xr"] for ci in range(NCORES)]

    def conv(ncB, xl_shards, xr_shards, inv):
        tbl = _assemble_table(xl_shards, row_of_slot)
        tlo = np.ascontiguousarray(tbl[:ROWS_LO])
        thi = np.ascontiguousarray(tbl[ROWS_LO:])
        in_maps = []
        for ci in range(NCORES):
            xr_pm = np.ascontiguousarray(
                xr_shards[ci].reshape(NCHUNK, P, HC).transpose(1, 0, 2)
                .reshape(P, NCHUNK * HC))
            in_maps.append({
                "tlo": tlo, "thi": thi, "xr": xr_pm,
                "idx16": plan["idx16"][ci], "invatt": inv,
                "onescol": onescol,
            })
        res = _run(ncB, in_maps)
        outs = [res.results[ci]["out"] for ci in range(NCORES)]
        stats = np.concatenate([res.results[ci]["stats"] for ci in range(NCORES)],
                               axis=0)
        return outs, stats

    out0, stats0 = conv(ncB0, xl_sh, xr_sh, inv0)

    # ---- launch C: norm0 + relu + layer-1 transforms ----
    in_maps = [{"x": out0[ci], "stats": stats0, "ones8": ones8, "onesr": onesr,
                "gamma": g0, "beta": be0, "ms": ms0, "biasr": bias0,
                "Wl": Wl1, "Wr": Wr1, "blr": _rep(bl1), "brr": _rep(br1)}
               for ci in range(NCORES)]
    resC = _run(ncC, in_maps)
    xl1_sh = [resC.results[ci]["xl"] for ci in range(NCORES)]
    xr1_sh = [resC.results[ci]["xr"] for ci in range(NCORES)]

    out1, stats1 = conv(ncB1, xl1_sh, xr1_sh, inv1)

    # ---- launch E: norm1 + relu + MLP ----
    in_maps = [{"x": out1[ci], "stats": stats1, "ones8": ones8, "onesr": onesr,
                "gamma": g1, "beta": be1, "ms": ms1, "biasr": bias1,
                "W1": W1, "b1r": _rep(b1), "W2": W2, "b2r": _rep(b2)}
               for ci in range(NCORES)]
    resE = _run(ncE, in_maps)

    y = np.empty((N, 2), np.float32)
    for ci in range(NCORES):
        valid = node_of_slot[ci] >= 0
        y[node_of_slot[ci][valid]] = resE.results[ci]["y"][valid]
    return y


# revision 4
# speedup vs baseline: 1.2935x; 1.0399x over previous
"""GATv2 (2-layer, GraphNorm, MLP head) on 8 Trainium2 NeuronCores — v2.

Design (vs the v1 edge-tile/one-hot-matmul kernel):
- dst-per-partition layout: each dst node owns one SBUF partition slot; its
  incoming edges lie along the free dimension.  Softmax and the weighted sum
  become free-dim tensor_reduce ops — no one-hot matmuls, no xr edge gather.
- Destinations are sorted by (lo_degree, hi_degree) and packed into chunks of
  1024 (128 partitions x 8 cores) so the rectangular edge padding stays small.
  Consecutive chunks merge into "windows" that share one dma_gather pair,
  amortizing the ~1us SWDGE fixed cost per gather.
- att is folded into the node tables (xl'' = att*xl): since lrelu is
  positively homogeneous and  min(x, .2x) = Prelu_{alpha=5}(0.2x),  the
  per-channel score term att_c*lrelu(v_c) becomes a plain Prelu over
  channels permuted pos-first per head.  This kills one full-size DVE pass.
  The aggregation output is un-scaled by 1/att at the end.
- Tables are fp16 (DVE runs 2x on 16-bit); scores skip the segment-max
  (exp never overflows here), pad edges point at a -1e4 table row so their
  exp underflows to exactly 0.
- conv bias + GraphNorm fold into the following launch's affine.

5 launches: A (layer-0 transforms), B0 (conv0), C (norm0+relu+layer-1
transforms), B1 (conv1), E (norm1+relu+MLP head).  Host work between
launches is index prep + memory movement only.
"""

import hashlib
import numpy as np

import concourse.bass as bass  # noqa: F401
import concourse.bacc as bacc
import concourse.tile as tile
from concourse import mybir
from concourse import bass_utils
from concourse.masks import make_identity

F32 = mybir.dt.float32
F16 = mybir.dt.float16
I16 = mybir.dt.int16
AF = mybir.ActivationFunctionType
ALU = mybir.AluOpType
AX = mybir.AxisListType

N, IN, H, C, E = 50000, 128, 2, 64, 800000
HC = H * C  # 128
NEG_SLOPE = 0.2
EPS_GN = 1e-5
NCORES = 8
P = 128

NCHUNK = 49                  # chunks of 1024 dsts (128 per core x 8)
SHARD_SLOTS = NCHUNK * P     # 6272 dst slots per core
NLO = 31360                  # nodes [0, NLO) gathered from the lo table
ROWS_LO = NLO + 1            # row 0 = pad(-1e4), node n -> row n+1
ROWS_HI = 50000 - NLO + 177  # 18817: nodes NLO.. at row n+1-ROWS_LO, spares, pad
ROWS_TOT = ROWS_LO + ROWS_HI  # 50178
PADHI_IDX = ROWS_HI - 1      # hi-local index of the hi pad row
PAD_VAL = -1e4
SBUF_CAP = 96                # max G*(Dlo+Dhi) per window
G_MAX = 8

_cache = {}


# ----------------------------------------------------------------------------
# host-side planning
# ----------------------------------------------------------------------------

def _wrap_idx_multi(buf):
    """[8, n] int16 -> [8, 128, n//16]: idx i -> [i%16, i//16], tiled x8."""
    nc_, n = buf.shape
    w = buf.reshape(nc_, n // 16, 16).transpose(0, 2, 1)  # [8, 16, n/16]
    return np.tile(w, (1, 8, 1))                          # [8, 128, n/16]


def build_plan(edge_index):
    ei = np.asarray(edge_index).astype(np.int64)
    loop = np.arange(N, dtype=np.int64)
    src = np.concatenate([ei[0], loop])
    dst = np.concatenate([ei[1], loop])
    is_lo = src < NLO

    lo_deg = np.bincount(dst[is_lo], minlength=N)
    hi_deg = np.bincount(dst[~is_lo], minlength=N)

    # Chunk packing (lo_deg and hi_deg are independent Poissons, so no 1D
    # sort bins both): lo-sorted bands of 7 chunks, hi-sorted within a band.
    # All chunks of a band share Dlo, so window-merging within a band only
    # maxes the (sorted, adjacent) Dhi values.
    o1 = np.argsort(-lo_deg, kind="stable")
    BAND = 7 * 1024
    parts = []
    for b in range((N + BAND - 1) // BAND):
        band = o1[b * BAND:(b + 1) * BAND]
        parts.append(band[np.argsort(-hi_deg[band], kind="stable")])
    order = np.concatenate(parts)
    rank = np.empty(N, np.int64)
    rank[order] = np.arange(N)
    chunk = rank // 1024
    within = rank % 1024
    core_of = within // P
    part_of = within % P

    ld = np.zeros(NCHUNK * 1024, np.int64)
    hd = np.zeros(NCHUNK * 1024, np.int64)
    ld[: N] = lo_deg[order]
    hd[: N] = hi_deg[order]
    Dlo_c = np.maximum(ld.reshape(NCHUNK, 1024).max(1), 1)
    Dhi_c = np.maximum(hd.reshape(NCHUNK, 1024).max(1), 1)

    # windows: merge consecutive chunks (sorted desc, so maxes come first)
    windows = []  # (g0, G, Dlo, Dhi)
    g = 0
    while g < NCHUNK:
        Dl, Dh = int(Dlo_c[g]), int(Dhi_c[g])
        G = 1
        waste = 0
        while G < G_MAX and g + G < NCHUNK:
            nl = max(Dl, int(Dlo_c[g + G]))
            nh = max(Dh, int(Dhi_c[g + G]))
            if (G + 1) * (nl + nh) > SBUF_CAP:
                break
            nw = (G + 1) * (nl + nh) - sum(
                int(Dlo_c[g + k] + Dhi_c[g + k]) for k in range(G + 1))
            if nw > 4:
                break
            Dl, Dh = nl, nh
            waste = nw
            G += 1
        windows.append((g, G, Dl, Dh))
        g += G

    # per-core flat idx buffer layout: [w0-lo | w0-hi | w1-lo | ...]
    base_lo = np.zeros(NCHUNK, np.int64)   # indexed by chunk
    base_hi = np.zeros(NCHUNK, np.int64)
    w_of_chunk = np.zeros(NCHUNK, np.int64)
    glocal = np.zeros(NCHUNK, np.int64)
    Dlo_w = np.zeros(NCHUNK, np.int64)     # per chunk: its window's Dlo
    Dhi_w = np.zeros(NCHUNK, np.int64)
    tot = 0
    for wi, (g0, G, Dl, Dh) in enumerate(windows):
        for k in range(G):
            ch = g0 + k
            w_of_chunk[ch] = wi
            glocal[ch] = k
            Dlo_w[ch] = Dl
            Dhi_w[ch] = Dh
            base_lo[ch] = tot
            base_hi[ch] = tot + G * Dl * P
        tot += G * (Dl + Dh) * P
    TOTI = tot

    # pad template (per window/region), then scatter real edges
    tmpl = np.empty(TOTI, np.int16)
    off = 0
    for (g0, G, Dl, Dh) in windows:
        tmpl[off: off + G * Dl * P] = 0          # lo pad row
        off += G * Dl * P
        tmpl[off: off + G * Dh * P] = PADHI_IDX  # hi pad row
        off += G * Dh * P
    buf = np.tile(tmpl, (NCORES, 1))

    for side in (0, 1):  # 0 = lo, 1 = hi
        mask = is_lo if side == 0 else ~is_lo
        es = np.nonzero(mask)[0]
        d_e = dst[es]
        o2 = np.argsort(d_e, kind="stable")
        es = es[o2]
        d_e = d_e[o2]
        first = np.searchsorted(d_e, np.arange(N))
        j = np.arange(len(es)) - first[d_e]
        ch = chunk[d_e]
        Dr = (Dlo_w if side == 0 else Dhi_w)[ch]
        base = (base_lo if side == 0 else base_hi)[ch]
        t = glocal[ch] * Dr + j
        pos = base + t * P + part_of[d_e]
        val = src[es] + 1 if side == 0 else src[es] + 1 - ROWS_LO
        buf[core_of[d_e], pos] = val.astype(np.int16)

    idx16 = _wrap_idx_multi(buf)  # [8, 128, TOTI//16]

    # slot maps
    node_of_slot = np.full((NCORES, SHARD_SLOTS), -1, np.int64)
    slot = chunk * P + part_of
    node_of_slot[core_of, slot] = np.arange(N)
    row_of_slot = np.empty((NCORES, SHARD_SLOTS), np.int64)
    pad_mask = node_of_slot < 0
    row_of_slot[~pad_mask] = node_of_slot[~pad_mask] + 1
    row_of_slot[pad_mask] = 50001 + np.arange(pad_mask.sum())  # spare rows

    real = float(len(src))
    return {
        "windows": windows, "TOTI": TOTI, "idx16": idx16,
        "node_of_slot": node_of_slot, "row_of_slot": row_of_slot,
        "pad_factor": TOTI / real,
    }


# ----------------------------------------------------------------------------
# kernel builders
# ----------------------------------------------------------------------------

def _new_nc(nq=1):
    return bacc.Bacc("TRN2", target_bir_lowering=False, num_swdge_queues=nq)


def build_transform():
    """Launch A: xl'' = x @ Wl'' + bl'', xr'' = x @ Wr'' + br'' (fp16 out)."""
    nc = _new_nc()
    x = nc.dram_tensor("x", [SHARD_SLOTS, IN], F32, kind="ExternalInput")
    Wl = nc.dram_tensor("Wl", [IN, HC], F16, kind="ExternalInput")
    Wr = nc.dram_tensor("Wr", [IN, HC], F16, kind="ExternalInput")
    blr = nc.dram_tensor("blr", [P, HC], F32, kind="ExternalInput")
    brr = nc.dram_tensor("brr", [P, HC], F32, kind="ExternalInput")
    xl = nc.dram_tensor("xl", [SHARD_SLOTS, HC], F16, kind="ExternalOutput")
    xr = nc.dram_tensor("xr", [SHARD_SLOTS, HC], F16, kind="ExternalOutput")

    with tile.TileContext(nc) as tc:
        with (
            tc.tile_pool(name="const", bufs=1) as cpool,
            tc.tile_pool(name="sbuf", bufs=3) as pool,
            tc.tile_pool(name="psum", bufs=2, space="PSUM") as psum,
        ):
            ident = cpool.tile([P, P], F16)
            make_identity(nc, ident[:])
            wl_t = cpool.tile([IN, HC], F16)
            wr_t = cpool.tile([IN, HC], F16)
            bl_t = cpool.tile([P, HC], F32)
            br_t = cpool.tile([P, HC], F32)
            nc.sync.dma_start(out=wl_t[:], in_=Wl[:, :])
            nc.sync.dma_start(out=wr_t[:], in_=Wr[:, :])
            nc.sync.dma_start(out=bl_t[:], in_=blr[:, :])
            nc.sync.dma_start(out=br_t[:], in_=brr[:, :])

            for t in range(NCHUNK):
                xt = pool.tile([P, IN], F32, tag="xt")
                nc.sync.dma_start(out=xt[:], in_=x[t * P:(t + 1) * P, :])
                xt16 = pool.tile([P, IN], F16, tag="xt16")
                nc.vector.tensor_copy(out=xt16[:], in_=xt[:])
                xT_ps = psum.tile([P, P], F16, tag="xT")
                nc.tensor.transpose(xT_ps[:], xt16[:], ident[:])
                xT = pool.tile([P, P], F16, tag="xTs")
                nc.vector.tensor_copy(out=xT[:], in_=xT_ps[:])
                for (w_t, b_t, out_d, tag) in ((wl_t, bl_t, xl, "l"),
                                               (wr_t, br_t, xr, "r")):
                    ps = psum.tile([P, HC], F32, tag="mm" + tag)
                    nc.tensor.matmul(ps[:], xT[:], w_t[:], start=True, stop=True)
                    ot = pool.tile([P, HC], F16, tag="ot" + tag)
                    nc.vector.tensor_add(out=ot[:], in0=ps[:], in1=b_t[:])
                    nc.sync.dma_start(out=out_d[t * P:(t + 1) * P, :], in_=ot[:])
    nc.finalize()
    return nc


def build_conv(windows, TOTI, ranges):
    """Launch B: GATv2 conv, dst-per-partition layout.

    ranges = (p0, p1): count of positive-att channels per head (channels are
    host-permuted pos-first within each head).
    """
    p0, p1 = ranges
    nc = _new_nc(nq=4)
    tlo = nc.dram_tensor("tlo", [ROWS_LO, HC], F16, kind="ExternalInput")
    thi = nc.dram_tensor("thi", [ROWS_HI, HC], F16, kind="ExternalInput")
    xr_d = nc.dram_tensor("xr", [P, NCHUNK * HC], F16, kind="ExternalInput")
    idx_d = nc.dram_tensor("idx16", [P, TOTI // 16], I16, kind="ExternalInput")
    invatt = nc.dram_tensor("invatt", [P, HC], F32, kind="ExternalInput")
    onescol = nc.dram_tensor("onescol", [P, 1], F32, kind="ExternalInput")
    out_d = nc.dram_tensor("out", [P, NCHUNK * HC], F16, kind="ExternalOutput")
    stats = nc.dram_tensor("stats", [1, 2 * HC], F32, kind="ExternalOutput")

    # activation ranges in c-major space: head h occupies (c, h) columns;
    # pos channels are c < p_h.  (c0, clen, h, alpha)
    act_ranges = []
    for h, pp in ((0, p0), (1, p1)):
        if pp > 0:
            act_ranges.append((0, pp, h, NEG_SLOPE))
        if pp < C:
            act_ranges.append((pp, C - pp, h, 5.0))

    NW = len(windows)
    with tile.TileContext(nc) as tc:
        with (
            tc.tile_pool(name="const", bufs=1) as cpool,
            tc.tile_pool(name="gath", bufs=3) as gpool,
            tc.tile_pool(name="work", bufs=2) as pool,
            tc.tile_pool(name="oh", bufs=3) as ohpool,
            tc.tile_pool(name="pstat", bufs=1, space="PSUM") as pstat,
        ):
            inv_t = cpool.tile([P, HC], F32)
            ones_t = cpool.tile([P, 1], F32)
            acc = cpool.tile([P, 2 * HC], F32)
            eps_t = cpool.tile([P, 1], F32)
            nc.sync.dma_start(out=inv_t[:], in_=invatt[:, :])
            nc.sync.dma_start(out=ones_t[:], in_=onescol[:, :])
            nc.vector.memset(acc[:], 0.0)
            nc.vector.memset(eps_t[:], 1e-16)

            state = {}  # per-window live tiles

            def emit_load(i):
                g0, G, Dl, Dh = windows[i]
                nlo, nhi = G * Dl * P, G * Dh * P
                ioff = sum(w[1] * (w[2] + w[3]) * P for w in windows[:i]) // 16
                ilo = gpool.tile([P, nlo // 16], I16, tag="ilo")
                ihi = gpool.tile([P, nhi // 16], I16, tag="ihi")
                nc.sync.dma_start(out=ilo[:], in_=idx_d[:, ioff: ioff + nlo // 16])
                nc.sync.dma_start(
                    out=ihi[:], in_=idx_d[:, ioff + nlo // 16: ioff + (nlo + nhi) // 16])
                glo = gpool.tile([P, G * Dl, HC], F16, tag="glo")
                ghi = gpool.tile([P, G * Dh, HC], F16, tag="ghi")
                nc.gpsimd.dma_gather(glo[:], tlo[:, :], ilo[:], nlo, nlo, HC,
                                     single_packet=False,
                                     queue_num=(2 * i) % 4)
                nc.gpsimd.dma_gather(ghi[:], thi[:, :], ihi[:], nhi, nhi, HC,
                                     single_packet=False,
                                     queue_num=(2 * i + 1) % 4)
                xrw = gpool.tile([P, G, HC], F16, tag="xrw")
                nc.sync.dma_start(out=xrw[:], in_=xr_d[:, g0 * HC:(g0 + G) * HC])
                state[i] = {"glo": glo, "ghi": ghi, "xrw": xrw}

            def emit_add_prelu(i):
                g0, G, Dl, Dh = windows[i]
                st = state[i]
                for (reg, Dr) in (("lo", Dl), ("hi", Dh)):
                    xlg = st["g" + reg]
                    v = pool.tile([P, G * Dr, HC], F16, tag="v" + reg)
                    xr_b = st["xrw"][:].unsqueeze(2).broadcast_to([P, G, Dr, HC])
                    nc.vector.tensor_add(
                        out=v[:].rearrange("p (g d) c -> p g d c", g=G),
                        in0=xlg[:].rearrange("p (g d) c -> p g d c", g=G),
                        in1=xr_b)
                    vv = v[:].rearrange("p (g d) (c h) -> p g d c h",
                                        g=G, h=H)
                    for (c0, ln, h, alpha) in act_ranges:
                        sl = vv[:, :, :, c0:c0 + ln, h:h + 1]
                        scale = 1.0 if alpha == NEG_SLOPE else NEG_SLOPE
                        nc.scalar.activation(sl, sl, AF.Prelu, scale=scale,
                                             alpha=alpha)
                    st["v" + reg] = v

            def emit_scores(i):
                g0, G, Dl, Dh = windows[i]
                st = state[i]
                for (reg, Dr) in (("lo", Dl), ("hi", Dh)):
                    v = st["v" + reg]
                    # c-major makes c-halves contiguous: 3D slab adds
                    vv = v[:]  # [P, G*Dr, HC]
                    cur = C
                    while cur > 1:
                        half = cur // 2
                        nc.vector.tensor_tensor(
                            out=vv[:, :, 0:half * H],
                            in0=vv[:, :, 0:half * H],
                            in1=vv[:, :, half * H:cur * H], op=ALU.add)
                        cur = half
                    pex = pool.tile([P, G, Dr, H], F16, tag="pex" + reg)
                    nc.scalar.activation(
                        pex[:], vv[:, :, 0:H].rearrange(
                            "p (g d) h -> p g d h", g=G), AF.Exp)
                    st["pex" + reg] = pex

            def emit_main(i):
                g0, G, Dl, Dh = windows[i]
                st = state[i]
                den = pool.tile([P, G, H], F32, tag="den")
                nc.vector.tensor_reduce(
                    out=den[:],
                    in_=st["pexlo"][:].rearrange("p g d h -> p g h d"),
                    axis=AX.X, op=ALU.add)
                den2 = pool.tile([P, G, H], F32, tag="den2")
                nc.vector.tensor_reduce(
                    out=den2[:],
                    in_=st["pexhi"][:].rearrange("p g d h -> p g h d"),
                    axis=AX.X, op=ALU.add)
                nc.vector.tensor_add(out=den[:], in0=den[:], in1=den2[:])
                rec = pool.tile([P, G, H], F32, tag="rec")
                nc.scalar.activation(den[:], den[:], AF.Identity,
                                     bias=eps_t[:])
                nc.vector.reciprocal(out=rec[:], in_=den[:])
                for (reg, Dr) in (("lo", Dl), ("hi", Dh)):
                    xlg = st["g" + reg]
                    t_r = st["v" + reg]  # overwrite (dead after scores)
                    pex = st["pex" + reg]
                    p_b = pex[:].rearrange("p g d h -> p (g d) h") \
                        .unsqueeze(2).broadcast_to([P, G * Dr, C, H])
                    nc.vector.tensor_mul(
                        out=t_r[:].rearrange("p g (c h) -> p g c h", h=H),
                        in0=xlg[:].rearrange("p g (c h) -> p g c h", h=H),
                        in1=p_b)
                    # pairwise tree over D (odd tail folded into the front)
                    tv = t_r[:].rearrange("p (g d) c -> p g (d c)", g=G)
                    cur = Dr
                    while cur > 1:
                        half = cur // 2
                        rem = cur - 2 * half
                        if rem:
                            nc.vector.tensor_tensor(
                                out=tv[:, :, 0:rem * HC],
                                in0=tv[:, :, 0:rem * HC],
                                in1=tv[:, :, 2 * half * HC:cur * HC],
                                op=ALU.add)
                        nc.vector.tensor_tensor(
                            out=tv[:, :, 0:half * HC],
                            in0=tv[:, :, 0:half * HC],
                            in1=tv[:, :, half * HC:(cur - rem) * HC],
                            op=ALU.add)
                        cur = half
                osum = pool.tile([P, G, HC], F32, tag="osum")
                nc.vector.tensor_add(
                    out=osum[:],
                    in0=st["vlo"][:].rearrange("p (g d) c -> p g d c", g=G)
                    [:, :, 0, :],
                    in1=st["vhi"][:].rearrange("p (g d) c -> p g d c", g=G)
                    [:, :, 0, :])
                rec_b = rec[:].rearrange("p g h -> p g h").unsqueeze(2) \
                    .broadcast_to([P, G, C, H])
                nc.vector.tensor_mul(
                    out=osum[:].rearrange("p g (c h) -> p g c h", c=C),
                    in0=osum[:].rearrange("p g (c h) -> p g c h", c=C),
                    in1=rec_b)
                oh = ohpool.tile([P, G, HC], F16, tag="oh")
                inv_b = inv_t[:].unsqueeze(1).broadcast_to([P, G, HC])
                nc.vector.tensor_mul(out=oh[:], in0=osum[:], in1=inv_b)
                nc.sync.dma_start(out=out_d[:, g0 * HC:(g0 + G) * HC],
                                  in_=oh[:].rearrange("p g c -> p (g c)"))
                st["oh"] = oh
                for k in ("glo", "ghi", "vlo", "vhi", "pexlo", "pexhi"):
                    st.pop(k, None)

            def emit_stats(i):
                g0, G, Dl, Dh = windows[i]
                st = state.pop(i)
                oh = st["oh"]
                sq = pool.tile([P, G, HC], F32, tag="sq")
                nc.scalar.activation(sq[:], oh[:], AF.Square)
                s1 = pool.tile([P, HC], F32, tag="s1")
                nc.vector.tensor_reduce(
                    out=s1[:], in_=oh[:].rearrange("p g c -> p c g"),
                    axis=AX.X, op=ALU.add)
                nc.vector.tensor_add(out=acc[:, 0:HC], in0=acc[:, 0:HC],
                                     in1=s1[:])
                s2 = pool.tile([P, HC], F32, tag="s2")
                nc.vector.tensor_reduce(
                    out=s2[:], in_=sq[:].rearrange("p g c -> p c g"),
                    axis=AX.X, op=ALU.add)
                nc.vector.tensor_add(out=acc[:, HC:2 * HC], in0=acc[:, HC:2 * HC],
                                     in1=s2[:])

            # software-pipelined emission.  Per-iteration engine-queue order is
            # chosen so ACT's exp(i-1) precedes the 8 prelus(i) (else the DVE
            # wmults of window i-1 would stall ~5us behind them), and gathers
            # run one window ahead of their adds.
            emit_load(0)
            for i in range(NW + 2):
                if i + 1 < NW:
                    emit_load(i + 1)
                if 1 <= i <= NW:
                    emit_scores(i - 1)
                if i < NW:
                    emit_add_prelu(i)
                if 1 <= i <= NW:
                    emit_main(i - 1)
                if 2 <= i <= NW + 1:
                    emit_stats(i - 2)

            st_ps = pstat.tile([1, 2 * HC], F32, tag="st")
            nc.tensor.matmul(st_ps[:], ones_t[:], acc[:], start=True, stop=True)
            stt = pool.tile([1, 2 * HC], F32, tag="stt")
            nc.vector.tensor_copy(out=stt[:], in_=st_ps[:])
            nc.sync.dma_start(out=stats[:, :], in_=stt[:])
    nc.finalize()
    return nc


def _emit_norm_prelude(nc, cpool, pconst, stats, ones8, onesr, gamma, beta, ms,
                       biasr):
    """Common GraphNorm-affine computation with conv-bias folding.

    Returns (a_rep, b_rep): normalized = a_rep * o' + b_rep where o' is the
    bias-less conv output."""
    st8 = cpool.tile([NCORES, 2 * HC], F32)
    o8 = cpool.tile([NCORES, 1], F32)
    orow = cpool.tile([1, P], F32)
    g_t = cpool.tile([1, HC], F32)
    be_t = cpool.tile([1, HC], F32)
    ms_t = cpool.tile([1, HC], F32)
    bi_t = cpool.tile([1, HC], F32)
    nc.sync.dma_start(out=st8[:], in_=stats[:, :])
    nc.sync.dma_start(out=o8[:], in_=ones8[:, :])
    nc.sync.dma_start(out=orow[:], in_=onesr[:, :])
    nc.sync.dma_start(out=g_t[:], in_=gamma[:, :])
    nc.sync.dma_start(out=be_t[:], in_=beta[:, :])
    nc.sync.dma_start(out=ms_t[:], in_=ms[:, :])
    nc.sync.dma_start(out=bi_t[:], in_=biasr[:, :])

    sg_ps = pconst.tile([1, 2 * HC], F32, tag="sg")
    nc.tensor.matmul(sg_ps[:], o8[:], st8[:], start=True, stop=True)
    # mean_o = S1/N ; mean_y = mean_o + bias
    mean = cpool.tile([1, HC], F32)
    nc.scalar.mul(mean[:], sg_ps[:, 0:HC], 1.0 / N)
    mean_y = cpool.tile([1, HC], F32)
    nc.vector.tensor_add(out=mean_y[:], in0=mean[:], in1=bi_t[:])
    # E[y^2] = S2/N + bias*(2*mean_o + bias)
    ey2 = cpool.tile([1, HC], F32)
    nc.scalar.mul(ey2[:], sg_ps[:, HC:2 * HC], 1.0 / N)
    t1 = cpool.tile([1, HC], F32)
    nc.scalar.mul(t1[:], mean[:], 2.0)
    nc.vector.tensor_add(out=t1[:], in0=t1[:], in1=bi_t[:])
    nc.vector.tensor_mul(out=t1[:], in0=t1[:], in1=bi_t[:])
    nc.vector.tensor_add(out=ey2[:], in0=ey2[:], in1=t1[:])
    # var = E[y^2] - ms*(2-ms)*mean_y^2
    two_b = cpool.tile([1, 1], F32)
    nc.vector.memset(two_b[:], 2.0)
    eps_b = cpool.tile([1, 1], F32)
    nc.vector.memset(eps_b[:], EPS_GN)
    two_minus = cpool.tile([1, HC], F32)
    nc.scalar.activation(two_minus[:], ms_t[:], AF.Identity, bias=two_b[:],
                         scale=-1.0)
    msm = cpool.tile([1, HC], F32)
    nc.vector.tensor_mul(out=msm[:], in0=two_minus[:], in1=ms_t[:])
    m2 = cpool.tile([1, HC], F32)
    nc.vector.tensor_mul(out=m2[:], in0=mean_y[:], in1=mean_y[:])
    var = cpool.tile([1, HC], F32)
    nc.vector.tensor_mul(out=var[:], in0=m2[:], in1=msm[:])
    nc.vector.tensor_tensor(out=var[:], in0=ey2[:], in1=var[:],
                            op=ALU.subtract)
    nc.scalar.activation(var[:], var[:], AF.Identity, bias=eps_b[:])
    sd = cpool.tile([1, HC], F32)
    nc.scalar.activation(sd[:], var[:], AF.Sqrt)
    rsd = cpool.tile([1, HC], F32)
    nc.vector.reciprocal(out=rsd[:], in_=sd[:])
    arow = cpool.tile([1, HC], F32)      # A = gamma * rsd
    nc.vector.tensor_mul(out=arow[:], in0=g_t[:], in1=rsd[:])
    brow = cpool.tile([1, HC], F32)      # B = beta - A*ms*mean_y
    nc.vector.tensor_mul(out=brow[:], in0=arow[:], in1=ms_t[:])
    nc.vector.tensor_mul(out=brow[:], in0=brow[:], in1=mean_y[:])
    nc.vector.tensor_tensor(out=brow[:], in0=be_t[:], in1=brow[:],
                            op=ALU.subtract)
    # fold: normalized = A*(o'+bias) + B = A*o' + (A*bias + B)
    b2row = cpool.tile([1, HC], F32)
    nc.vector.tensor_mul(out=b2row[:], in0=arow[:], in1=bi_t[:])
    nc.vector.tensor_add(out=b2row[:], in0=b2row[:], in1=brow[:])
    # broadcast to [P, HC]
    a_ps = pconst.tile([P, HC], F32, tag="arep")
    b_ps = pconst.tile([P, HC], F32, tag="brep")
    nc.tensor.matmul(a_ps[:], orow[:], arow[:], start=True, stop=True)
    nc.tensor.matmul(b_ps[:], orow[:], b2row[:], start=True, stop=True)
    a_rep = cpool.tile([P, HC], F32)
    b_rep = cpool.tile([P, HC], F32)
    nc.vector.tensor_copy(out=a_rep[:], in_=a_ps[:])
    nc.vector.tensor_copy(out=b_rep[:], in_=b_ps[:])
    return a_rep, b_rep


def build_norm_transform():
    """Launch C: h = relu(norm(out0+bias)); xl1'' = h@Wl1''+bl1''; xr1''."""
    nc = _new_nc()
    x = nc.dram_tensor("x", [P, NCHUNK * HC], F16, kind="ExternalInput")
    stats = nc.dram_tensor("stats", [NCORES, 2 * HC], F32, kind="ExternalInput")
    ones8 = nc.dram_tensor("ones8", [NCORES, 1], F32, kind="ExternalInput")
    onesr = nc.dram_tensor("onesr", [1, P], F32, kind="ExternalInput")
    gamma = nc.dram_tensor("gamma", [1, HC], F32, kind="ExternalInput")
    beta = nc.dram_tensor("beta", [1, HC], F32, kind="ExternalInput")
    ms = nc.dram_tensor("ms", [1, HC], F32, kind="ExternalInput")
    biasr = nc.dram_tensor("biasr", [1, HC], F32, kind="ExternalInput")
    Wl = nc.dram_tensor("Wl", [HC, HC], F16, kind="ExternalInput")
    Wr = nc.dram_tensor("Wr", [HC, HC], F16, kind="ExternalInput")
    blr = nc.dram_tensor("blr", [P, HC], F32, kind="ExternalInput")
    brr = nc.dram_tensor("brr", [P, HC], F32, kind="ExternalInput")
    xl = nc.dram_tensor("xl", [SHARD_SLOTS, HC], F16, kind="ExternalOutput")
    xr = nc.dram_tensor("xr", [SHARD_SLOTS, HC], F16, kind="ExternalOutput")

    with tile.TileContext(nc) as tc:
        with (
            tc.tile_pool(name="const", bufs=1) as cpool,
            tc.tile_pool(name="sbuf", bufs=3) as pool,
            tc.tile_pool(name="psum", bufs=2, space="PSUM") as psum,
            tc.tile_pool(name="pconst", bufs=1, space="PSUM") as pconst,
        ):
            ident = cpool.tile([P, P], F16)
            make_identity(nc, ident[:])
            wl_t = cpool.tile([HC, HC], F16)
            wr_t = cpool.tile([HC, HC], F16)
            bl_t = cpool.tile([P, HC], F32)
            br_t = cpool.tile([P, HC], F32)
            nc.sync.dma_start(out=wl_t[:], in_=Wl[:, :])
            nc.sync.dma_start(out=wr_t[:], in_=Wr[:, :])
            nc.sync.dma_start(out=bl_t[:], in_=blr[:, :])
            nc.sync.dma_start(out=br_t[:], in_=brr[:, :])
            a_rep, b_rep = _emit_norm_prelude(
                nc, cpool, pconst, stats, ones8, onesr, gamma, beta, ms, biasr)

            for t in range(NCHUNK):
                xt = pool.tile([P, HC], F16, tag="xt")
                nc.sync.dma_start(out=xt[:], in_=x[:, t * HC:(t + 1) * HC])
                hn = pool.tile([P, HC], F32, tag="hn")
                nc.vector.tensor_mul(out=hn[:], in0=xt[:], in1=a_rep[:])
                nc.vector.tensor_add(out=hn[:], in0=hn[:], in1=b_rep[:])
                hn16 = pool.tile([P, HC], F16, tag="hn16")
                nc.scalar.activation(hn16[:], hn[:], AF.Relu)
                xT_ps = psum.tile([P, P], F16, tag="xT")
                nc.tensor.transpose(xT_ps[:], hn16[:], ident[:])
                xT = pool.tile([P, P], F16, tag="xTs")
                nc.vector.tensor_copy(out=xT[:], in_=xT_ps[:])
                ps = psum.tile([P, 2 * HC], F32, tag="mm")
                nc.tensor.matmul(ps[:, 0:HC], xT[:], wl_t[:], start=True,
                                 stop=True)
                nc.tensor.matmul(ps[:, HC:2 * HC], xT[:], wr_t[:], start=True,
                                 stop=True)
                for (b_t, out_dd, sl, tag) in ((bl_t, xl, slice(0, HC), "l"),
                                               (br_t, xr, slice(HC, 2 * HC), "r")):
                    ot = pool.tile([P, HC], F16, tag="ot" + tag)
                    nc.vector.tensor_add(out=ot[:], in0=ps[:, sl], in1=b_t[:])
                    nc.sync.dma_start(out=out_dd[t * P:(t + 1) * P, :], in_=ot[:])
    nc.finalize()
    return nc


def build_norm_mlp():
    """Launch E: h = relu(norm(out1+bias)); y = relu(h@W1+b1)@W2+b2."""
    nc = _new_nc()
    x = nc.dram_tensor("x", [P, NCHUNK * HC], F16, kind="ExternalInput")
    stats = nc.dram_tensor("stats", [NCORES, 2 * HC], F32, kind="ExternalInput")
    ones8 = nc.dram_tensor("ones8", [NCORES, 1], F32, kind="ExternalInput")
    onesr = nc.dram_tensor("onesr", [1, P], F32, kind="ExternalInput")
    gamma = nc.dram_tensor("gamma", [1, HC], F32, kind="ExternalInput")
    beta = nc.dram_tensor("beta", [1, HC], F32, kind="ExternalInput")
    ms = nc.dram_tensor("ms", [1, HC], F32, kind="ExternalInput")
    biasr = nc.dram_tensor("biasr", [1, HC], F32, kind="ExternalInput")
    W1 = nc.dram_tensor("W1", [HC, C], F16, kind="ExternalInput")
    b1r = nc.dram_tensor("b1r", [P, C], F32, kind="ExternalInput")
    W2 = nc.dram_tensor("W2", [C, 2], F16, kind="ExternalInput")
    b2r = nc.dram_tensor("b2r", [P, 2], F32, kind="ExternalInput")
    y = nc.dram_tensor("y", [SHARD_SLOTS, 2], F32, kind="ExternalOutput")

    with tile.TileContext(nc) as tc:
        with (
            tc.tile_pool(name="const", bufs=1) as cpool,
            tc.tile_pool(name="sbuf", bufs=3) as pool,
            tc.tile_pool(name="psum", bufs=2, space="PSUM") as psum,
            tc.tile_pool(name="pone", bufs=1, space="PSUM") as pone,
            tc.tile_pool(name="pconst", bufs=1, space="PSUM") as pconst,
        ):
            ident = cpool.tile([P, P], F16)
            make_identity(nc, ident[:])
            w1_t = cpool.tile([HC, C], F16)
            b1_t = cpool.tile([P, C], F32)
            w2_t = cpool.tile([C, 2], F16)
            b2_t = cpool.tile([P, 2], F32)
            nc.sync.dma_start(out=w1_t[:], in_=W1[:, :])
            nc.sync.dma_start(out=b1_t[:], in_=b1r[:, :])
            nc.sync.dma_start(out=w2_t[:], in_=W2[:, :])
            nc.sync.dma_start(out=b2_t[:], in_=b2r[:, :])
            a_rep, b_rep = _emit_norm_prelude(
                nc, cpool, pconst, stats, ones8, onesr, gamma, beta, ms, biasr)

            for t in range(NCHUNK):
                xt = pool.tile([P, HC], F16, tag="xt")
                nc.sync.dma_start(out=xt[:], in_=x[:, t * HC:(t + 1) * HC])
                hn = pool.tile([P, HC], F32, tag="hn")
                nc.vector.tensor_mul(out=hn[:], in0=xt[:], in1=a_rep[:])
                nc.vector.tensor_add(out=hn[:], in0=hn[:], in1=b_rep[:])
                hn16 = pool.tile([P, HC], F16, tag="hn16")
                nc.scalar.activation(hn16[:], hn[:], AF.Relu)
                xT_ps = psum.tile([P, P], F16, tag="xT")
                nc.tensor.transpose(xT_ps[:], hn16[:], ident[:])
                xT = pool.tile([P, P], F16, tag="xTs")
                nc.vector.tensor_copy(out=xT[:], in_=xT_ps[:])
                z_ps = pone.tile([P, C], F32, tag="z")
                nc.tensor.matmul(z_ps[:], xT[:], w1_t[:], start=True, stop=True)
                z = pool.tile([P, C], F32, tag="zs")
                nc.vector.tensor_add(out=z[:], in0=z_ps[:], in1=b1_t[:])
                z16 = pool.tile([P, C], F16, tag="z16")
                nc.scalar.activation(z16[:], z[:], AF.Relu)
                zT_ps = pone.tile([C, P], F16, tag="zT")
                nc.tensor.transpose(zT_ps[:], z16[:], ident[:])
                zT = pool.tile([C, P], F16, tag="zTs")
                nc.vector.tensor_copy(out=zT[:], in_=zT_ps[:])
                y_ps = pone.tile([P, 2], F32, tag="y")
                nc.tensor.matmul(y_ps[:], zT[:], w2_t[:], start=True, stop=True)
                yt = pool.tile([P, 2], F32, tag="yt")
                nc.vector.tensor_add(out=yt[:], in0=y_ps[:], in1=b2_t[:])
                nc.sync.dma_start(out=y[t * P:(t + 1) * P, :], in_=yt[:])
    nc.finalize()
    return nc


# ----------------------------------------------------------------------------
# host orchestration
# ----------------------------------------------------------------------------

TRACE = False
LAST_EXEC_NS = []


def _run(nc, in_maps, trace=None):
    trace = TRACE if trace is None else trace
    last_err = None
    for attempt in range(3):
        try:
            res = bass_utils.run_bass_kernel_spmd(
                nc, in_maps, core_ids=list(range(NCORES)), trace=trace)
            LAST_EXEC_NS.append(res.exec_time_ns)
            return res
        except Exception as e:
            last_err = e
            import time as _t
            _t.sleep(2.0 * (attempt + 1))
    raise last_err


def _rep(v):
    v = np.asarray(v, np.float32).reshape(1, -1)
    return np.tile(v, (P, 1))


def _head_perm(att):
    """Channel order: c-major head-interleaved (col = c*H + h), pos-att-first
    within each head.  Keeps the innermost stride of per-(edge,head)-scalar
    broadcasts at 1 so DVE 2x applies.  Returns (perm, (p0, p1))."""
    att = np.asarray(att, np.float32).reshape(H, C)
    heads = []
    counts = []
    for h in range(H):
        pos = np.nonzero(att[h] > 0)[0]
        neg = np.nonzero(att[h] <= 0)[0]
        heads.append(np.concatenate([pos, neg]) + h * C)
        counts.append(len(pos))
    perm = np.empty(HC, np.int64)
    for c in range(C):
        for h in range(H):
            perm[c * H + h] = heads[h][c]
    return perm, tuple(counts)


def _assemble_table(xl_shards, row_of_slot):
    tbl = np.empty((ROWS_TOT, HC), np.float16)
    tbl[0] = PAD_VAL
    tbl[ROWS_TOT - 1] = PAD_VAL
    allrows = row_of_slot.reshape(-1)
    tbl[allrows] = np.concatenate(xl_shards, axis=0)
    return tbl


def kernel(**inputs):
    LAST_EXEC_NS.clear()
    x = np.asarray(inputs["x"], np.float32)
    edge_index = np.asarray(inputs["edge_index"])
    key = hashlib.sha1(np.ascontiguousarray(edge_index).tobytes()).hexdigest()

    att0 = np.asarray(inputs["att0"], np.float32).reshape(-1)
    att1 = np.asarray(inputs["att1"], np.float32).reshape(-1)
    pi0, r0 = _head_perm(att0)
    pi1, r1 = _head_perm(att1)

    if _cache.get("edge_key") != key:
        plan = build_plan(edge_index)
        _cache.clear()
        _cache["edge_key"] = key
        _cache["plan"] = plan
        _cache["ncA"] = build_transform()
        _cache["ncC"] = build_norm_transform()
        _cache["ncE"] = build_norm_mlp()
    plan = _cache["plan"]
    if _cache.get("r0") != r0:
        _cache["ncB0"] = build_conv(plan["windows"], plan["TOTI"], r0)
        _cache["r0"] = r0
    if _cache.get("r1") != r1:
        if r1 == r0:
            _cache["ncB1"] = _cache["ncB0"]
        else:
            _cache["ncB1"] = build_conv(plan["windows"], plan["TOTI"], r1)
        _cache["r1"] = r1
    ncA, ncB0, ncC, ncB1, ncE = (_cache["ncA"], _cache["ncB0"], _cache["ncC"],
                                 _cache["ncB1"], _cache["ncE"])

    node_of_slot = plan["node_of_slot"]
    row_of_slot = plan["row_of_slot"]

    # ---- host weight prep (channel perms + att folding) ----
    a0p = att0[pi0]
    a1p = att1[pi1]
    inv0 = _rep(1.0 / a0p)
    inv1 = _rep(1.0 / a1p)

    Wl0 = (np.asarray(inputs["Wl0"], np.float32)[:, pi0] * a0p).astype(np.float16)
    Wr0 = (np.asarray(inputs["Wr0"], np.float32)[:, pi0] * a0p).astype(np.float16)
    bl0 = np.asarray(inputs["bl0"], np.float32)[pi0] * a0p
    br0 = np.asarray(inputs["br0"], np.float32)[pi0] * a0p
    # layer-1 weights: rows in pi0 space (h lives there), cols pi1+att1-scaled
    Wl1 = (np.asarray(inputs["Wl1"], np.float32)[pi0][:, pi1] * a1p).astype(np.float16)
    Wr1 = (np.asarray(inputs["Wr1"], np.float32)[pi0][:, pi1] * a1p).astype(np.float16)
    bl1 = np.asarray(inputs["bl1"], np.float32)[pi1] * a1p
    br1 = np.asarray(inputs["br1"], np.float32)[pi1] * a1p
    W1 = np.asarray(inputs["W1"], np.float32)[pi1].astype(np.float16)
    b1 = np.asarray(inputs["b1"], np.float32)
    W2 = np.asarray(inputs["W2"], np.float32).astype(np.float16)
    b2 = np.asarray(inputs["b2"], np.float32)

    g0 = np.asarray(inputs["g0"], np.float32)[pi0].reshape(1, HC)
    be0 = np.asarray(inputs["be0"], np.float32)[pi0].reshape(1, HC)
    ms0 = np.asarray(inputs["ms0"], np.float32)[pi0].reshape(1, HC)
    bias0 = np.asarray(inputs["bias0"], np.float32)[pi0].reshape(1, HC)
    g1 = np.asarray(inputs["g1"], np.float32)[pi1].reshape(1, HC)
    be1 = np.asarray(inputs["be1"], np.float32)[pi1].reshape(1, HC)
    ms1 = np.asarray(inputs["ms1"], np.float32)[pi1].reshape(1, HC)
    bias1 = np.asarray(inputs["bias1"], np.float32)[pi1].reshape(1, HC)

    ones8 = np.ones((NCORES, 1), np.float32)
    onesr = np.ones((1, P), np.float32)
    onescol = np.ones((P, 1), np.float32)

    # ---- launch A: layer-0 transforms ----
    x_slots = [x[np.clip(node_of_slot[ci], 0, N - 1)] for ci in range(NCORES)]
    in_maps = [{"x": x_slots[ci], "Wl": Wl0, "Wr": Wr0,
                "blr": _rep(bl0), "brr": _rep(br0)} for ci in range(NCORES)]
    resA = _run(ncA, in_maps)
    xl_sh = [resA.results[ci]["xl"] for ci in range(NCORES)]
    xr_sh = [resA.results[ci]["xr"] for ci in range(NCORES)]

    def conv(ncB, xl_shards, xr_shards, inv):
        tbl = _assemble_table(xl_shards, row_of_slot)
        tlo = np.ascontiguousarray(tbl[:ROWS_LO])
        thi = np.ascontiguousarray(tbl[ROWS_LO:])
        in_maps = []
        for ci in range(NCORES):
            xr_pm = np.ascontiguousarray(
                xr_shards[ci].reshape(NCHUNK, P, HC).transpose(1, 0, 2)
                .reshape(P, NCHUNK * HC))
            in_maps.append({
                "tlo": tlo, "thi": thi, "xr": xr_pm,
                "idx16": plan["idx16"][ci], "invatt": inv,
                "onescol": onescol,
            })
        res = _run(ncB, in_maps)
        outs = [res.results[ci]["out"] for ci in range(NCORES)]
        stats = np.concatenate([res.results[ci]["stats"] for ci in range(NCORES)],
                               axis=0)
        return outs, stats

    out0, stats0 = conv(ncB0, xl_sh, xr_sh, inv0)

    # ---- launch C: norm0 + relu + layer-1 transforms ----
    in_maps = [{"x": out0[ci], "stats": stats0, "ones8": ones8, "onesr": onesr,
                "gamma": g0, "beta": be0, "ms": ms0, "biasr": bias0,
                "Wl": Wl1, "Wr": Wr1, "blr": _rep(bl1), "brr": _rep(br1)}
               for ci in range(NCORES)]
    resC = _run(ncC, in_maps)
    xl1_sh = [resC.results[ci]["xl"] for ci in range(NCORES)]
    xr1_sh = [resC.results[ci]["xr"] for ci in range(NCORES)]

    out1, stats1 = conv(ncB1, xl1_sh, xr1_sh, inv1)

    # ---- launch E: norm1 + relu + MLP ----
    in_maps = [{"x": out1[ci], "stats": stats1, "ones8": ones8, "onesr": onesr,
                "gamma": g1, "beta": be1, "ms": ms1, "biasr": bias1,
                "W1": W1, "b1r": _rep(b1), "W2": W2, "b2r": _rep(b2)}
               for ci in range(NCORES)]
    resE = _run(ncE, in_maps)

    y = np.empty((N, 2), np.float32)
    for ci in range(NCORES):
        valid = node_of_slot[ci] >= 0
        y[node_of_slot[ci][valid]] = resE.results[ci]["y"][valid]
    return y


# revision 5
# speedup vs baseline: 1.5430x; 1.1928x over previous
"""GATv2 (2-layer, GraphNorm, MLP head) on 8 Trainium2 NeuronCores — v2.

Design (vs the v1 edge-tile/one-hot-matmul kernel):
- dst-per-partition layout: each dst node owns one SBUF partition slot; its
  incoming edges lie along the free dimension.  Softmax and the weighted sum
  become free-dim tensor_reduce ops — no one-hot matmuls, no xr edge gather.
- Destinations are sorted by (lo_degree, hi_degree) and packed into chunks of
  1024 (128 partitions x 8 cores) so the rectangular edge padding stays small.
  Consecutive chunks merge into "windows" that share one dma_gather pair,
  amortizing the ~1us SWDGE fixed cost per gather.
- att is folded into the node tables (xl'' = att*xl): since lrelu is
  positively homogeneous and  min(x, .2x) = Prelu_{alpha=5}(0.2x),  the
  per-channel score term att_c*lrelu(v_c) becomes a plain Prelu over
  channels permuted pos-first per head.  This kills one full-size DVE pass.
  The aggregation output is un-scaled by 1/att at the end.
- Tables are fp16 (DVE runs 2x on 16-bit); scores skip the segment-max
  (exp never overflows here), pad edges point at a -1e4 table row so their
  exp underflows to exactly 0.
- conv bias + GraphNorm fold into the following launch's affine.

5 launches: A (layer-0 transforms), B0 (conv0), C (norm0+relu+layer-1
transforms), B1 (conv1), E (norm1+relu+MLP head).  Host work between
launches is index prep + memory movement only.
"""

import hashlib
import numpy as np

import concourse.bass as bass  # noqa: F401
import concourse.bacc as bacc
import concourse.tile as tile
from concourse import mybir
from concourse import bass_utils
from concourse.masks import make_identity

F32 = mybir.dt.float32
F16 = mybir.dt.float16
I16 = mybir.dt.int16
AF = mybir.ActivationFunctionType
ALU = mybir.AluOpType
AX = mybir.AxisListType

N, IN, H, C, E = 50000, 128, 2, 64, 800000
HC = H * C  # 128
NEG_SLOPE = 0.2
EPS_GN = 1e-5
NCORES = 8
P = 128

NCHUNK = 49                  # chunks of 1024 dsts (128 per core x 8)
SHARD_SLOTS = NCHUNK * P     # 6272 dst slots per core
NLO = 31360                  # nodes [0, NLO) gathered from the lo table
ROWS_LO = NLO + 1            # row 0 = pad(-1e4), node n -> row n+1
ROWS_HI = 50000 - NLO + 177  # 18817: nodes NLO.. at row n+1-ROWS_LO, spares, pad
ROWS_TOT = ROWS_LO + ROWS_HI  # 50178
PADHI_IDX = ROWS_HI - 1      # hi-local index of the hi pad row
PAD_VAL = -1e4
SBUF_CAP = 96                # max G*(Dlo+Dhi) per window
G_MAX = 8

_cache = {}


# ----------------------------------------------------------------------------
# host-side planning
# ----------------------------------------------------------------------------

def _wrap_idx_multi(buf):
    """[8, n] int16 -> [8, 128, n//16]: idx i -> [i%16, i//16], tiled x8."""
    nc_, n = buf.shape
    w = buf.reshape(nc_, n // 16, 16).transpose(0, 2, 1)  # [8, 16, n/16]
    return np.tile(w, (1, 8, 1))                          # [8, 128, n/16]


def build_plan(edge_index):
    ei = np.asarray(edge_index).astype(np.int64)
    loop = np.arange(N, dtype=np.int64)
    src = np.concatenate([ei[0], loop])
    dst = np.concatenate([ei[1], loop])
    is_lo = src < NLO

    lo_deg = np.bincount(dst[is_lo], minlength=N)
    hi_deg = np.bincount(dst[~is_lo], minlength=N)

    # Chunk packing (lo_deg and hi_deg are independent Poissons, so no 1D
    # sort bins both): lo-sorted bands of 7 chunks, hi-sorted within a band.
    # All chunks of a band share Dlo, so window-merging within a band only
    # maxes the (sorted, adjacent) Dhi values.
    order = np.lexsort((-lo_deg, -(hi_deg // 3)))
    rank = np.empty(N, np.int64)
    rank[order] = np.arange(N)
    chunk = rank // 1024
    within = rank % 1024
    core_of = within // P
    part_of = within % P

    ld = np.zeros(NCHUNK * 1024, np.int64)
    hd = np.zeros(NCHUNK * 1024, np.int64)
    ld[: N] = lo_deg[order]
    hd[: N] = hi_deg[order]
    Dlo_c = np.maximum(ld.reshape(NCHUNK, 1024).max(1), 1)
    Dhi_c = np.maximum(hd.reshape(NCHUNK, 1024).max(1), 1)

    # windows: merge consecutive chunks (sorted desc, so maxes come first)
    windows = []  # (g0, G, Dlo, Dhi)
    g = 0
    while g < NCHUNK:
        Dl, Dh = int(Dlo_c[g]), int(Dhi_c[g])
        G = 1
        waste = 0
        while G < G_MAX and g + G < NCHUNK:
            nl = max(Dl, int(Dlo_c[g + G]))
            nh = max(Dh, int(Dhi_c[g + G]))
            if (G + 1) * (nl + nh) > SBUF_CAP:
                break
            nw = (G + 1) * (nl + nh) - sum(
                int(Dlo_c[g + k] + Dhi_c[g + k]) for k in range(G + 1))
            if nw > 4:
                break
            Dl, Dh = nl, nh
            waste = nw
            G += 1
        windows.append((g, G, Dl, Dh))
        g += G

    # per-core flat idx buffer layout: [w0-lo | w0-hi | w1-lo | ...]
    base_lo = np.zeros(NCHUNK, np.int64)   # indexed by chunk
    base_hi = np.zeros(NCHUNK, np.int64)
    w_of_chunk = np.zeros(NCHUNK, np.int64)
    glocal = np.zeros(NCHUNK, np.int64)
    Dlo_w = np.zeros(NCHUNK, np.int64)     # per chunk: its window's Dlo
    Dhi_w = np.zeros(NCHUNK, np.int64)
    tot = 0
    for wi, (g0, G, Dl, Dh) in enumerate(windows):
        for k in range(G):
            ch = g0 + k
            w_of_chunk[ch] = wi
            glocal[ch] = k
            Dlo_w[ch] = Dl
            Dhi_w[ch] = Dh
            base_lo[ch] = tot
            base_hi[ch] = tot + G * Dl * P
        tot += G * (Dl + Dh) * P
    TOTI = tot

    # pad template (per window/region), then scatter real edges
    tmpl = np.empty(TOTI, np.int16)
    off = 0
    for (g0, G, Dl, Dh) in windows:
        tmpl[off: off + G * Dl * P] = 0          # lo pad row
        off += G * Dl * P
        tmpl[off: off + G * Dh * P] = PADHI_IDX  # hi pad row
        off += G * Dh * P
    buf = np.tile(tmpl, (NCORES, 1))

    for side in (0, 1):  # 0 = lo, 1 = hi
        mask = is_lo if side == 0 else ~is_lo
        es = np.nonzero(mask)[0]
        d_e = dst[es]
        o2 = np.argsort(d_e, kind="stable")
        es = es[o2]
        d_e = d_e[o2]
        first = np.searchsorted(d_e, np.arange(N))
        j = np.arange(len(es)) - first[d_e]
        ch = chunk[d_e]
        Dr = (Dlo_w if side == 0 else Dhi_w)[ch]
        base = (base_lo if side == 0 else base_hi)[ch]
        t = glocal[ch] * Dr + j
        pos = base + t * P + part_of[d_e]
        val = src[es] + 1 if side == 0 else src[es] + 1 - ROWS_LO
        buf[core_of[d_e], pos] = val.astype(np.int16)

    idx16 = _wrap_idx_multi(buf)  # [8, 128, TOTI//16]

    # slot maps
    node_of_slot = np.full((NCORES, SHARD_SLOTS), -1, np.int64)
    slot = chunk * P + part_of
    node_of_slot[core_of, slot] = np.arange(N)
    row_of_slot = np.empty((NCORES, SHARD_SLOTS), np.int64)
    pad_mask = node_of_slot < 0
    row_of_slot[~pad_mask] = node_of_slot[~pad_mask] + 1
    row_of_slot[pad_mask] = 50001 + np.arange(pad_mask.sum())  # spare rows

    real = float(len(src))
    return {
        "windows": windows, "TOTI": TOTI, "idx16": idx16,
        "node_of_slot": node_of_slot, "row_of_slot": row_of_slot,
        "pad_factor": TOTI / real,
    }


# ----------------------------------------------------------------------------
# kernel builders
# ----------------------------------------------------------------------------

def _new_nc(nq=1):
    return bacc.Bacc("TRN2", target_bir_lowering=False, num_swdge_queues=nq)


def build_transform():
    """Launch A: xl'' = x @ Wl'' + bl'', xr'' = x @ Wr'' + br'' (fp16 out)."""
    nc = _new_nc()
    x = nc.dram_tensor("x", [SHARD_SLOTS, IN], F32, kind="ExternalInput")
    Wl = nc.dram_tensor("Wl", [IN, HC], F16, kind="ExternalInput")
    Wr = nc.dram_tensor("Wr", [IN, HC], F16, kind="ExternalInput")
    blr = nc.dram_tensor("blr", [P, HC], F32, kind="ExternalInput")
    brr = nc.dram_tensor("brr", [P, HC], F32, kind="ExternalInput")
    xl = nc.dram_tensor("xl", [SHARD_SLOTS, HC], F16, kind="ExternalOutput")
    xr = nc.dram_tensor("xr", [SHARD_SLOTS, HC], F16, kind="ExternalOutput")

    with tile.TileContext(nc) as tc:
        with (
            tc.tile_pool(name="const", bufs=1) as cpool,
            tc.tile_pool(name="sbuf", bufs=3) as pool,
            tc.tile_pool(name="psum", bufs=2, space="PSUM") as psum,
        ):
            ident = cpool.tile([P, P], F16)
            make_identity(nc, ident[:])
            wl_t = cpool.tile([IN, HC], F16)
            wr_t = cpool.tile([IN, HC], F16)
            bl_t = cpool.tile([P, HC], F32)
            br_t = cpool.tile([P, HC], F32)
            nc.sync.dma_start(out=wl_t[:], in_=Wl[:, :])
            nc.sync.dma_start(out=wr_t[:], in_=Wr[:, :])
            nc.sync.dma_start(out=bl_t[:], in_=blr[:, :])
            nc.sync.dma_start(out=br_t[:], in_=brr[:, :])

            for t in range(NCHUNK):
                xt = pool.tile([P, IN], F32, tag="xt")
                nc.sync.dma_start(out=xt[:], in_=x[t * P:(t + 1) * P, :])
                xt16 = pool.tile([P, IN], F16, tag="xt16")
                nc.vector.tensor_copy(out=xt16[:], in_=xt[:])
                xT_ps = psum.tile([P, P], F16, tag="xT")
                nc.tensor.transpose(xT_ps[:], xt16[:], ident[:])
                xT = pool.tile([P, P], F16, tag="xTs")
                nc.vector.tensor_copy(out=xT[:], in_=xT_ps[:])
                for (w_t, b_t, out_d, tag) in ((wl_t, bl_t, xl, "l"),
                                               (wr_t, br_t, xr, "r")):
                    ps = psum.tile([P, HC], F32, tag="mm" + tag)
                    nc.tensor.matmul(ps[:], xT[:], w_t[:], start=True, stop=True)
                    ot = pool.tile([P, HC], F16, tag="ot" + tag)
                    nc.vector.tensor_add(out=ot[:], in0=ps[:], in1=b_t[:])
                    nc.sync.dma_start(out=out_d[t * P:(t + 1) * P, :], in_=ot[:])
    nc.finalize()
    return nc


def build_conv(windows, TOTI, ranges):
    """Launch B: GATv2 conv, dst-per-partition layout.

    ranges = (p0, p1): count of positive-att channels per head (channels are
    host-permuted pos-first within each head).
    """
    p0, p1 = ranges
    nc = _new_nc(nq=4)
    tlo = nc.dram_tensor("tlo", [ROWS_LO, HC], F16, kind="ExternalInput")
    thi = nc.dram_tensor("thi", [ROWS_HI, HC], F16, kind="ExternalInput")
    xr_d = nc.dram_tensor("xr", [P, NCHUNK * HC], F16, kind="ExternalInput")
    idx_d = nc.dram_tensor("idx16", [P, TOTI // 16], I16, kind="ExternalInput")
    invatt = nc.dram_tensor("invatt", [P, HC], F32, kind="ExternalInput")
    onescol = nc.dram_tensor("onescol", [P, 1], F32, kind="ExternalInput")
    out_d = nc.dram_tensor("out", [P, NCHUNK * HC], F16, kind="ExternalOutput")
    stats = nc.dram_tensor("stats", [1, 2 * HC], F32, kind="ExternalOutput")

    # activation ranges in c-major space: head h occupies (c, h) columns;
    # pos channels are c < p_h.  (c0, clen, h, alpha)
    act_ranges = []
    for h, pp in ((0, p0), (1, p1)):
        if pp > 0:
            act_ranges.append((0, pp, h, NEG_SLOPE))
        if pp < C:
            act_ranges.append((pp, C - pp, h, 5.0))

    NW = len(windows)
    with tile.TileContext(nc) as tc:
        with (
            tc.tile_pool(name="const", bufs=1) as cpool,
            tc.tile_pool(name="gath", bufs=3) as gpool,
            tc.tile_pool(name="work", bufs=2) as pool,
            tc.tile_pool(name="oh", bufs=3) as ohpool,
            tc.tile_pool(name="pstat", bufs=1, space="PSUM") as pstat,
        ):
            inv_t = cpool.tile([P, HC], F32)
            ones_t = cpool.tile([P, 1], F32)
            acc = cpool.tile([P, 2 * HC], F32)
            eps_t = cpool.tile([P, 1], F32)
            nc.sync.dma_start(out=inv_t[:], in_=invatt[:, :])
            nc.sync.dma_start(out=ones_t[:], in_=onescol[:, :])
            nc.vector.memset(acc[:], 0.0)
            nc.vector.memset(eps_t[:], 1e-16)

            state = {}  # per-window live tiles

            def emit_load(i):
                g0, G, Dl, Dh = windows[i]
                nlo, nhi = G * Dl * P, G * Dh * P
                ioff = sum(w[1] * (w[2] + w[3]) * P for w in windows[:i]) // 16
                ilo = gpool.tile([P, nlo // 16], I16, tag="ilo")
                ihi = gpool.tile([P, nhi // 16], I16, tag="ihi")
                nc.sync.dma_start(out=ilo[:], in_=idx_d[:, ioff: ioff + nlo // 16])
                nc.sync.dma_start(
                    out=ihi[:], in_=idx_d[:, ioff + nlo // 16: ioff + (nlo + nhi) // 16])
                glo = gpool.tile([P, G * Dl, HC], F16, tag="glo")
                ghi = gpool.tile([P, G * Dh, HC], F16, tag="ghi")
                nc.gpsimd.dma_gather(glo[:], tlo[:, :], ilo[:], nlo, nlo, HC,
                                     single_packet=False,
                                     queue_num=(2 * i) % 4)
                nc.gpsimd.dma_gather(ghi[:], thi[:, :], ihi[:], nhi, nhi, HC,
                                     single_packet=False,
                                     queue_num=(2 * i + 1) % 4)
                xrw = gpool.tile([P, G, HC], F16, tag="xrw")
                nc.sync.dma_start(out=xrw[:], in_=xr_d[:, g0 * HC:(g0 + G) * HC])
                state[i] = {"glo": glo, "ghi": ghi, "xrw": xrw}

            def emit_add_prelu(i):
                g0, G, Dl, Dh = windows[i]
                st = state[i]
                for (reg, Dr) in (("lo", Dl), ("hi", Dh)):
                    xlg = st["g" + reg]
                    v = pool.tile([P, G * Dr, HC], F16, tag="v" + reg)
                    xr_b = st["xrw"][:].unsqueeze(2).broadcast_to([P, G, Dr, HC])
                    nc.vector.tensor_add(
                        out=v[:].rearrange("p (g d) c -> p g d c", g=G),
                        in0=xlg[:].rearrange("p (g d) c -> p g d c", g=G),
                        in1=xr_b)
                    vv = v[:].rearrange("p (g d) (c h) -> p g d c h",
                                        g=G, h=H)
                    for (c0, ln, h, alpha) in act_ranges:
                        sl = vv[:, :, :, c0:c0 + ln, h:h + 1]
                        scale = 1.0 if alpha == NEG_SLOPE else NEG_SLOPE
                        nc.scalar.activation(sl, sl, AF.Prelu, scale=scale,
                                             alpha=alpha)
                    st["v" + reg] = v

            def emit_scores(i):
                g0, G, Dl, Dh = windows[i]
                st = state[i]
                for (reg, Dr) in (("lo", Dl), ("hi", Dh)):
                    v = st["v" + reg]
                    # c-major makes c-halves contiguous: 3D slab adds
                    vv = v[:]  # [P, G*Dr, HC]
                    cur = C
                    while cur > 1:
                        half = cur // 2
                        nc.vector.tensor_tensor(
                            out=vv[:, :, 0:half * H],
                            in0=vv[:, :, 0:half * H],
                            in1=vv[:, :, half * H:cur * H], op=ALU.add)
                        cur = half
                    pex = pool.tile([P, G, Dr, H], F16, tag="pex" + reg)
                    nc.scalar.activation(
                        pex[:], vv[:, :, 0:H].rearrange(
                            "p (g d) h -> p g d h", g=G), AF.Exp)
                    st["pex" + reg] = pex

            def emit_main(i):
                g0, G, Dl, Dh = windows[i]
                st = state[i]
                den = pool.tile([P, G, H], F32, tag="den")
                nc.vector.tensor_reduce(
                    out=den[:],
                    in_=st["pexlo"][:].rearrange("p g d h -> p g h d"),
                    axis=AX.X, op=ALU.add)
                den2 = pool.tile([P, G, H], F32, tag="den2")
                nc.vector.tensor_reduce(
                    out=den2[:],
                    in_=st["pexhi"][:].rearrange("p g d h -> p g h d"),
                    axis=AX.X, op=ALU.add)
                nc.vector.tensor_add(out=den[:], in0=den[:], in1=den2[:])
                rec = pool.tile([P, G, H], F32, tag="rec")
                nc.scalar.activation(den[:], den[:], AF.Identity,
                                     bias=eps_t[:])
                nc.vector.reciprocal(out=rec[:], in_=den[:])
                for (reg, Dr) in (("lo", Dl), ("hi", Dh)):
                    xlg = st["g" + reg]
                    t_r = st["v" + reg]  # overwrite (dead after scores)
                    pex = st["pex" + reg]
                    p_b = pex[:].rearrange("p g d h -> p (g d) h") \
                        .unsqueeze(2).broadcast_to([P, G * Dr, C, H])
                    nc.vector.tensor_mul(
                        out=t_r[:].rearrange("p g (c h) -> p g c h", h=H),
                        in0=xlg[:].rearrange("p g (c h) -> p g c h", h=H),
                        in1=p_b)
                    # pairwise tree over D (odd tail folded into the front)
                    tv = t_r[:].rearrange("p (g d) c -> p g (d c)", g=G)
                    cur = Dr
                    while cur > 1:
                        half = cur // 2
                        rem = cur - 2 * half
                        if rem:
                            nc.vector.tensor_tensor(
                                out=tv[:, :, 0:rem * HC],
                                in0=tv[:, :, 0:rem * HC],
                                in1=tv[:, :, 2 * half * HC:cur * HC],
                                op=ALU.add)
                        nc.vector.tensor_tensor(
                            out=tv[:, :, 0:half * HC],
                            in0=tv[:, :, 0:half * HC],
                            in1=tv[:, :, half * HC:(cur - rem) * HC],
                            op=ALU.add)
                        cur = half
                osum = pool.tile([P, G, HC], F32, tag="osum")
                nc.vector.tensor_add(
                    out=osum[:],
                    in0=st["vlo"][:].rearrange("p (g d) c -> p g d c", g=G)
                    [:, :, 0, :],
                    in1=st["vhi"][:].rearrange("p (g d) c -> p g d c", g=G)
                    [:, :, 0, :])
                rec_b = rec[:].rearrange("p g h -> p g h").unsqueeze(2) \
                    .broadcast_to([P, G, C, H])
                nc.vector.tensor_mul(
                    out=osum[:].rearrange("p g (c h) -> p g c h", c=C),
                    in0=osum[:].rearrange("p g (c h) -> p g c h", c=C),
                    in1=rec_b)
                oh = ohpool.tile([P, G, HC], F16, tag="oh")
                inv_b = inv_t[:].unsqueeze(1).broadcast_to([P, G, HC])
                nc.vector.tensor_mul(out=oh[:], in0=osum[:], in1=inv_b)
                nc.sync.dma_start(out=out_d[:, g0 * HC:(g0 + G) * HC],
                                  in_=oh[:].rearrange("p g c -> p (g c)"))
                st["oh"] = oh
                for k in ("glo", "ghi", "vlo", "vhi", "pexlo", "pexhi"):
                    st.pop(k, None)

            def emit_stats(i):
                g0, G, Dl, Dh = windows[i]
                st = state.pop(i)
                oh = st["oh"]
                sq = pool.tile([P, G, HC], F32, tag="sq")
                nc.scalar.activation(sq[:], oh[:], AF.Square)
                s1 = pool.tile([P, HC], F32, tag="s1")
                nc.vector.tensor_reduce(
                    out=s1[:], in_=oh[:].rearrange("p g c -> p c g"),
                    axis=AX.X, op=ALU.add)
                nc.vector.tensor_add(out=acc[:, 0:HC], in0=acc[:, 0:HC],
                                     in1=s1[:])
                s2 = pool.tile([P, HC], F32, tag="s2")
                nc.vector.tensor_reduce(
                    out=s2[:], in_=sq[:].rearrange("p g c -> p c g"),
                    axis=AX.X, op=ALU.add)
                nc.vector.tensor_add(out=acc[:, HC:2 * HC], in0=acc[:, HC:2 * HC],
                                     in1=s2[:])

            # software-pipelined emission.  Per-iteration engine-queue order is
            # chosen so ACT's exp(i-1) precedes the 8 prelus(i) (else the DVE
            # wmults of window i-1 would stall ~5us behind them), and gathers
            # run one window ahead of their adds.
            emit_load(0)
            for i in range(NW + 2):
                if i + 1 < NW:
                    emit_load(i + 1)
                if 1 <= i <= NW:
                    emit_scores(i - 1)
                if i < NW:
                    emit_add_prelu(i)
                if 1 <= i <= NW:
                    emit_main(i - 1)
                if 2 <= i <= NW + 1:
                    emit_stats(i - 2)

            st_ps = pstat.tile([1, 2 * HC], F32, tag="st")
            nc.tensor.matmul(st_ps[:], ones_t[:], acc[:], start=True, stop=True)
            stt = pool.tile([1, 2 * HC], F32, tag="stt")
            nc.vector.tensor_copy(out=stt[:], in_=st_ps[:])
            nc.sync.dma_start(out=stats[:, :], in_=stt[:])
    nc.finalize()
    return nc


def _emit_norm_prelude(nc, cpool, pconst, stats, ones8, onesr, gamma, beta, ms,
                       biasr):
    """Common GraphNorm-affine computation with conv-bias folding.

    Returns (a_rep, b_rep): normalized = a_rep * o' + b_rep where o' is the
    bias-less conv output."""
    st8 = cpool.tile([NCORES, 2 * HC], F32)
    o8 = cpool.tile([NCORES, 1], F32)
    orow = cpool.tile([1, P], F32)
    g_t = cpool.tile([1, HC], F32)
    be_t = cpool.tile([1, HC], F32)
    ms_t = cpool.tile([1, HC], F32)
    bi_t = cpool.tile([1, HC], F32)
    nc.sync.dma_start(out=st8[:], in_=stats[:, :])
    nc.sync.dma_start(out=o8[:], in_=ones8[:, :])
    nc.sync.dma_start(out=orow[:], in_=onesr[:, :])
    nc.sync.dma_start(out=g_t[:], in_=gamma[:, :])
    nc.sync.dma_start(out=be_t[:], in_=beta[:, :])
    nc.sync.dma_start(out=ms_t[:], in_=ms[:, :])
    nc.sync.dma_start(out=bi_t[:], in_=biasr[:, :])

    sg_ps = pconst.tile([1, 2 * HC], F32, tag="sg")
    nc.tensor.matmul(sg_ps[:], o8[:], st8[:], start=True, stop=True)
    # mean_o = S1/N ; mean_y = mean_o + bias
    mean = cpool.tile([1, HC], F32)
    nc.scalar.mul(mean[:], sg_ps[:, 0:HC], 1.0 / N)
    mean_y = cpool.tile([1, HC], F32)
    nc.vector.tensor_add(out=mean_y[:], in0=mean[:], in1=bi_t[:])
    # E[y^2] = S2/N + bias*(2*mean_o + bias)
    ey2 = cpool.tile([1, HC], F32)
    nc.scalar.mul(ey2[:], sg_ps[:, HC:2 * HC], 1.0 / N)
    t1 = cpool.tile([1, HC], F32)
    nc.scalar.mul(t1[:], mean[:], 2.0)
    nc.vector.tensor_add(out=t1[:], in0=t1[:], in1=bi_t[:])
    nc.vector.tensor_mul(out=t1[:], in0=t1[:], in1=bi_t[:])
    nc.vector.tensor_add(out=ey2[:], in0=ey2[:], in1=t1[:])
    # var = E[y^2] - ms*(2-ms)*mean_y^2
    two_b = cpool.tile([1, 1], F32)
    nc.vector.memset(two_b[:], 2.0)
    eps_b = cpool.tile([1, 1], F32)
    nc.vector.memset(eps_b[:], EPS_GN)
    two_minus = cpool.tile([1, HC], F32)
    nc.scalar.activation(two_minus[:], ms_t[:], AF.Identity, bias=two_b[:],
                         scale=-1.0)
    msm = cpool.tile([1, HC], F32)
    nc.vector.tensor_mul(out=msm[:], in0=two_minus[:], in1=ms_t[:])
    m2 = cpool.tile([1, HC], F32)
    nc.vector.tensor_mul(out=m2[:], in0=mean_y[:], in1=mean_y[:])
    var = cpool.tile([1, HC], F32)
    nc.vector.tensor_mul(out=var[:], in0=m2[:], in1=msm[:])
    nc.vector.tensor_tensor(out=var[:], in0=ey2[:], in1=var[:],
                            op=ALU.subtract)
    nc.scalar.activation(var[:], var[:], AF.Identity, bias=eps_b[:])
    sd = cpool.tile([1, HC], F32)
    nc.scalar.activation(sd[:], var[:], AF.Sqrt)
    rsd = cpool.tile([1, HC], F32)
    nc.vector.reciprocal(out=rsd[:], in_=sd[:])
    arow = cpool.tile([1, HC], F32)      # A = gamma * rsd
    nc.vector.tensor_mul(out=arow[:], in0=g_t[:], in1=rsd[:])
    brow = cpool.tile([1, HC], F32)      # B = beta - A*ms*mean_y
    nc.vector.tensor_mul(out=brow[:], in0=arow[:], in1=ms_t[:])
    nc.vector.tensor_mul(out=brow[:], in0=brow[:], in1=mean_y[:])
    nc.vector.tensor_tensor(out=brow[:], in0=be_t[:], in1=brow[:],
                            op=ALU.subtract)
    # fold: normalized = A*(o'+bias) + B = A*o' + (A*bias + B)
    b2row = cpool.tile([1, HC], F32)
    nc.vector.tensor_mul(out=b2row[:], in0=arow[:], in1=bi_t[:])
    nc.vector.tensor_add(out=b2row[:], in0=b2row[:], in1=brow[:])
    # broadcast to [P, HC]
    a_ps = pconst.tile([P, HC], F32, tag="arep")
    b_ps = pconst.tile([P, HC], F32, tag="brep")
    nc.tensor.matmul(a_ps[:], orow[:], arow[:], start=True, stop=True)
    nc.tensor.matmul(b_ps[:], orow[:], b2row[:], start=True, stop=True)
    a_rep = cpool.tile([P, HC], F32)
    b_rep = cpool.tile([P, HC], F32)
    nc.vector.tensor_copy(out=a_rep[:], in_=a_ps[:])
    nc.vector.tensor_copy(out=b_rep[:], in_=b_ps[:])
    return a_rep, b_rep


def build_norm_transform():
    """Launch C: h = relu(norm(out0+bias)); xl1'' = h@Wl1''+bl1''; xr1''."""
    nc = _new_nc()
    x = nc.dram_tensor("x", [P, NCHUNK * HC], F16, kind="ExternalInput")
    stats = nc.dram_tensor("stats", [NCORES, 2 * HC], F32, kind="ExternalInput")
    ones8 = nc.dram_tensor("ones8", [NCORES, 1], F32, kind="ExternalInput")
    onesr = nc.dram_tensor("onesr", [1, P], F32, kind="ExternalInput")
    gamma = nc.dram_tensor("gamma", [1, HC], F32, kind="ExternalInput")
    beta = nc.dram_tensor("beta", [1, HC], F32, kind="ExternalInput")
    ms = nc.dram_tensor("ms", [1, HC], F32, kind="ExternalInput")
    biasr = nc.dram_tensor("biasr", [1, HC], F32, kind="ExternalInput")
    Wl = nc.dram_tensor("Wl", [HC, HC], F16, kind="ExternalInput")
    Wr = nc.dram_tensor("Wr", [HC, HC], F16, kind="ExternalInput")
    blr = nc.dram_tensor("blr", [P, HC], F32, kind="ExternalInput")
    brr = nc.dram_tensor("brr", [P, HC], F32, kind="ExternalInput")
    xl = nc.dram_tensor("xl", [SHARD_SLOTS, HC], F16, kind="ExternalOutput")
    xr = nc.dram_tensor("xr", [SHARD_SLOTS, HC], F16, kind="ExternalOutput")

    with tile.TileContext(nc) as tc:
        with (
            tc.tile_pool(name="const", bufs=1) as cpool,
            tc.tile_pool(name="sbuf", bufs=3) as pool,
            tc.tile_pool(name="psum", bufs=2, space="PSUM") as psum,
            tc.tile_pool(name="pconst", bufs=1, space="PSUM") as pconst,
        ):
            ident = cpool.tile([P, P], F16)
            make_identity(nc, ident[:])
            wl_t = cpool.tile([HC, HC], F16)
            wr_t = cpool.tile([HC, HC], F16)
            bl_t = cpool.tile([P, HC], F32)
            br_t = cpool.tile([P, HC], F32)
            nc.sync.dma_start(out=wl_t[:], in_=Wl[:, :])
            nc.sync.dma_start(out=wr_t[:], in_=Wr[:, :])
            nc.sync.dma_start(out=bl_t[:], in_=blr[:, :])
            nc.sync.dma_start(out=br_t[:], in_=brr[:, :])
            a_rep, b_rep = _emit_norm_prelude(
                nc, cpool, pconst, stats, ones8, onesr, gamma, beta, ms, biasr)

            for t in range(NCHUNK):
                xt = pool.tile([P, HC], F16, tag="xt")
                nc.sync.dma_start(out=xt[:], in_=x[:, t * HC:(t + 1) * HC])
                hn = pool.tile([P, HC], F32, tag="hn")
                nc.vector.tensor_mul(out=hn[:], in0=xt[:], in1=a_rep[:])
                nc.vector.tensor_add(out=hn[:], in0=hn[:], in1=b_rep[:])
                hn16 = pool.tile([P, HC], F16, tag="hn16")
                nc.scalar.activation(hn16[:], hn[:], AF.Relu)
                xT_ps = psum.tile([P, P], F16, tag="xT")
                nc.tensor.transpose(xT_ps[:], hn16[:], ident[:])
                xT = pool.tile([P, P], F16, tag="xTs")
                nc.vector.tensor_copy(out=xT[:], in_=xT_ps[:])
                ps = psum.tile([P, 2 * HC], F32, tag="mm")
                nc.tensor.matmul(ps[:, 0:HC], xT[:], wl_t[:], start=True,
                                 stop=True)
                nc.tensor.matmul(ps[:, HC:2 * HC], xT[:], wr_t[:], start=True,
                                 stop=True)
                for (b_t, out_dd, sl, tag) in ((bl_t, xl, slice(0, HC), "l"),
                                               (br_t, xr, slice(HC, 2 * HC), "r")):
                    ot = pool.tile([P, HC], F16, tag="ot" + tag)
                    nc.vector.tensor_add(out=ot[:], in0=ps[:, sl], in1=b_t[:])
                    nc.sync.dma_start(out=out_dd[t * P:(t + 1) * P, :], in_=ot[:])
    nc.finalize()
    return nc


def build_norm_mlp():
    """Launch E: h = relu(norm(out1+bias)); y = relu(h@W1+b1)@W2+b2."""
    nc = _new_nc()
    x = nc.dram_tensor("x", [P, NCHUNK * HC], F16, kind="ExternalInput")
    stats = nc.dram_tensor("stats", [NCORES, 2 * HC], F32, kind="ExternalInput")
    ones8 = nc.dram_tensor("ones8", [NCORES, 1], F32, kind="ExternalInput")
    onesr = nc.dram_tensor("onesr", [1, P], F32, kind="ExternalInput")
    gamma = nc.dram_tensor("gamma", [1, HC], F32, kind="ExternalInput")
    beta = nc.dram_tensor("beta", [1, HC], F32, kind="ExternalInput")
    ms = nc.dram_tensor("ms", [1, HC], F32, kind="ExternalInput")
    biasr = nc.dram_tensor("biasr", [1, HC], F32, kind="ExternalInput")
    W1 = nc.dram_tensor("W1", [HC, C], F16, kind="ExternalInput")
    b1r = nc.dram_tensor("b1r", [P, C], F32, kind="ExternalInput")
    W2 = nc.dram_tensor("W2", [C, 2], F16, kind="ExternalInput")
    b2r = nc.dram_tensor("b2r", [P, 2], F32, kind="ExternalInput")
    y = nc.dram_tensor("y", [SHARD_SLOTS, 2], F32, kind="ExternalOutput")

    with tile.TileContext(nc) as tc:
        with (
            tc.tile_pool(name="const", bufs=1) as cpool,
            tc.tile_pool(name="sbuf", bufs=3) as pool,
            tc.tile_pool(name="psum", bufs=2, space="PSUM") as psum,
            tc.tile_pool(name="pone", bufs=1, space="PSUM") as pone,
            tc.tile_pool(name="pconst", bufs=1, space="PSUM") as pconst,
        ):
            ident = cpool.tile([P, P], F16)
            make_identity(nc, ident[:])
            w1_t = cpool.tile([HC, C], F16)
            b1_t = cpool.tile([P, C], F32)
            w2_t = cpool.tile([C, 2], F16)
            b2_t = cpool.tile([P, 2], F32)
            nc.sync.dma_start(out=w1_t[:], in_=W1[:, :])
            nc.sync.dma_start(out=b1_t[:], in_=b1r[:, :])
            nc.sync.dma_start(out=w2_t[:], in_=W2[:, :])
            nc.sync.dma_start(out=b2_t[:], in_=b2r[:, :])
            a_rep, b_rep = _emit_norm_prelude(
                nc, cpool, pconst, stats, ones8, onesr, gamma, beta, ms, biasr)

            for t in range(NCHUNK):
                xt = pool.tile([P, HC], F16, tag="xt")
                nc.sync.dma_start(out=xt[:], in_=x[:, t * HC:(t + 1) * HC])
                hn = pool.tile([P, HC], F32, tag="hn")
                nc.vector.tensor_mul(out=hn[:], in0=xt[:], in1=a_rep[:])
                nc.vector.tensor_add(out=hn[:], in0=hn[:], in1=b_rep[:])
                hn16 = pool.tile([P, HC], F16, tag="hn16")
                nc.scalar.activation(hn16[:], hn[:], AF.Relu)
                xT_ps = psum.tile([P, P], F16, tag="xT")
                nc.tensor.transpose(xT_ps[:], hn16[:], ident[:])
                xT = pool.tile([P, P], F16, tag="xTs")
                nc.vector.tensor_copy(out=xT[:], in_=xT_ps[:])
                z_ps = pone.tile([P, C], F32, tag="z")
                nc.tensor.matmul(z_ps[:], xT[:], w1_t[:], start=True, stop=True)
                z = pool.tile([P, C], F32, tag="zs")
                nc.vector.tensor_add(out=z[:], in0=z_ps[:], in1=b1_t[:])
                z16 = pool.tile([P, C], F16, tag="z16")
                nc.scalar.activation(z16[:], z[:], AF.Relu)
                zT_ps = pone.tile([C, P], F16, tag="zT")
                nc.tensor.transpose(zT_ps[:], z16[:], ident[:])
                zT = pool.tile([C, P], F16, tag="zTs")
                nc.vector.tensor_copy(out=zT[:], in_=zT_ps[:])
                y_ps = pone.tile([P, 2], F32, tag="y")
                nc.tensor.matmul(y_ps[:], zT[:], w2_t[:], start=True, stop=True)
                yt = pool.tile([P, 2], F32, tag="yt")
                nc.vector.tensor_add(out=yt[:], in0=y_ps[:], in1=b2_t[:])
                nc.sync.dma_start(out=y[t * P:(t + 1) * P, :], in_=yt[:])
    nc.finalize()
    return nc


# ----------------------------------------------------------------------------
# host orchestration
# ----------------------------------------------------------------------------

TRACE = False
LAST_EXEC_NS = []


def _run(nc, in_maps, trace=None):
    trace = TRACE if trace is None else trace
    last_err = None
    for attempt in range(3):
        try:
            res = bass_utils.run_bass_kernel_spmd(
                nc, in_maps, core_ids=list(range(NCORES)), trace=trace)
            LAST_EXEC_NS.append(res.exec_time_ns)
            return res
        except Exception as e:
            last_err = e
            import time as _t
            _t.sleep(2.0 * (attempt + 1))
    raise last_err


def _rep(v):
    v = np.asarray(v, np.float32).reshape(1, -1)
    return np.tile(v, (P, 1))


def _head_perm(att):
    """Channel order: c-major head-interleaved (col = c*H + h), pos-att-first
    within each head.  Keeps the innermost stride of per-(edge,head)-scalar
    broadcasts at 1 so DVE 2x applies.  Returns (perm, (p0, p1))."""
    att = np.asarray(att, np.float32).reshape(H, C)
    heads = []
    counts = []
    for h in range(H):
        pos = np.nonzero(att[h] > 0)[0]
        neg = np.nonzero(att[h] <= 0)[0]
        heads.append(np.concatenate([pos, neg]) + h * C)
        counts.append(len(pos))
    perm = np.empty(HC, np.int64)
    for c in range(C):
        for h in range(H):
            perm[c * H + h] = heads[h][c]
    return perm, tuple(counts)


def _assemble_table(xl_shards, row_of_slot):
    tbl = np.empty((ROWS_TOT, HC), np.float16)
    tbl[0] = PAD_VAL
    tbl[ROWS_TOT - 1] = PAD_VAL
    allrows = row_of_slot.reshape(-1)
    tbl[allrows] = np.concatenate(xl_shards, axis=0)
    return tbl


def kernel(**inputs):
    LAST_EXEC_NS.clear()
    x = np.asarray(inputs["x"], np.float32)
    edge_index = np.asarray(inputs["edge_index"])
    key = hashlib.sha1(np.ascontiguousarray(edge_index).tobytes()).hexdigest()

    att0 = np.asarray(inputs["att0"], np.float32).reshape(-1)
    att1 = np.asarray(inputs["att1"], np.float32).reshape(-1)
    pi0, r0 = _head_perm(att0)
    pi1, r1 = _head_perm(att1)

    if _cache.get("edge_key") != key:
        plan = build_plan(edge_index)
        _cache.clear()
        _cache["edge_key"] = key
        _cache["plan"] = plan
        _cache["ncA"] = build_transform()
        _cache["ncC"] = build_norm_transform()
        _cache["ncE"] = build_norm_mlp()
    plan = _cache["plan"]
    if _cache.get("r0") != r0:
        _cache["ncB0"] = build_conv(plan["windows"], plan["TOTI"], r0)
        _cache["r0"] = r0
    if _cache.get("r1") != r1:
        if r1 == r0:
            _cache["ncB1"] = _cache["ncB0"]
        else:
            _cache["ncB1"] = build_conv(plan["windows"], plan["TOTI"], r1)
        _cache["r1"] = r1
    ncA, ncB0, ncC, ncB1, ncE = (_cache["ncA"], _cache["ncB0"], _cache["ncC"],
                                 _cache["ncB1"], _cache["ncE"])

    node_of_slot = plan["node_of_slot"]
    row_of_slot = plan["row_of_slot"]

    # ---- host weight prep (channel perms + att folding) ----
    a0p = att0[pi0]
    a1p = att1[pi1]
    inv0 = _rep(1.0 / a0p)
    inv1 = _rep(1.0 / a1p)

    Wl0 = (np.asarray(inputs["Wl0"], np.float32)[:, pi0] * a0p).astype(np.float16)
    Wr0 = (np.asarray(inputs["Wr0"], np.float32)[:, pi0] * a0p).astype(np.float16)
    bl0 = np.asarray(inputs["bl0"], np.float32)[pi0] * a0p
    br0 = np.asarray(inputs["br0"], np.float32)[pi0] * a0p
    # layer-1 weights: rows in pi0 space (h lives there), cols pi1+att1-scaled
    Wl1 = (np.asarray(inputs["Wl1"], np.float32)[pi0][:, pi1] * a1p).astype(np.float16)
    Wr1 = (np.asarray(inputs["Wr1"], np.float32)[pi0][:, pi1] * a1p).astype(np.float16)
    bl1 = np.asarray(inputs["bl1"], np.float32)[pi1] * a1p
    br1 = np.asarray(inputs["br1"], np.float32)[pi1] * a1p
    W1 = np.asarray(inputs["W1"], np.float32)[pi1].astype(np.float16)
    b1 = np.asarray(inputs["b1"], np.float32)
    W2 = np.asarray(inputs["W2"], np.float32).astype(np.float16)
    b2 = np.asarray(inputs["b2"], np.float32)

    g0 = np.asarray(inputs["g0"], np.float32)[pi0].reshape(1, HC)
    be0 = np.asarray(inputs["be0"], np.float32)[pi0].reshape(1, HC)
    ms0 = np.asarray(inputs["ms0"], np.float32)[pi0].reshape(1, HC)
    bias0 = np.asarray(inputs["bias0"], np.float32)[pi0].reshape(1, HC)
    g1 = np.asarray(inputs["g1"], np.float32)[pi1].reshape(1, HC)
    be1 = np.asarray(inputs["be1"], np.float32)[pi1].reshape(1, HC)
    ms1 = np.asarray(inputs["ms1"], np.float32)[pi1].reshape(1, HC)
    bias1 = np.asarray(inputs["bias1"], np.float32)[pi1].reshape(1, HC)

    ones8 = np.ones((NCORES, 1), np.float32)
    onesr = np.ones((1, P), np.float32)
    onescol = np.ones((P, 1), np.float32)

    # ---- launch A: layer-0 transforms ----
    x_slots = [x[np.clip(node_of_slot[ci], 0, N - 1)] for ci in range(NCORES)]
    in_maps = [{"x": x_slots[ci], "Wl": Wl0, "Wr": Wr0,
                "blr": _rep(bl0), "brr": _rep(br0)} for ci in range(NCORES)]
    resA = _run(ncA, in_maps)
    xl_sh = [resA.results[ci]["xl"] for ci in range(NCORES)]
    xr_sh = [resA.results[ci]["xr"] for ci in range(NCORES)]

    def conv(ncB, xl_shards, xr_shards, inv):
        tbl = _assemble_table(xl_shards, row_of_slot)
        tlo = np.ascontiguousarray(tbl[:ROWS_LO])
        thi = np.ascontiguousarray(tbl[ROWS_LO:])
        in_maps = []
        for ci in range(NCORES):
            xr_pm = np.ascontiguousarray(
                xr_shards[ci].reshape(NCHUNK, P, HC).transpose(1, 0, 2)
                .reshape(P, NCHUNK * HC))
            in_maps.append({
                "tlo": tlo, "thi": thi, "xr": xr_pm,
                "idx16": plan["idx16"][ci], "invatt": inv,
                "onescol": onescol,
            })
        res = _run(ncB, in_maps)
        outs = [res.results[ci]["out"] for ci in range(NCORES)]
        stats = np.concatenate([res.results[ci]["stats"] for ci in range(NCORES)],
                               axis=0)
        return outs, stats

    out0, stats0 = conv(ncB0, xl_sh, xr_sh, inv0)

    # ---- launch C: norm0 + relu + layer-1 transforms ----
    in_maps = [{"x": out0[ci], "stats": stats0, "ones8": ones8, "onesr": onesr,
                "gamma": g0, "beta": be0, "ms": ms0, "biasr": bias0,
                "Wl": Wl1, "Wr": Wr1, "blr": _rep(bl1), "brr": _rep(br1)}
               for ci in range(NCORES)]
    resC = _run(ncC, in_maps)
    xl1_sh = [resC.results[ci]["xl"] for ci in range(NCORES)]
    xr1_sh = [resC.results[ci]["xr"] for ci in range(NCORES)]

    out1, stats1 = conv(ncB1, xl1_sh, xr1_sh, inv1)

    # ---- launch E: norm1 + relu + MLP ----
    in_maps = [{"x": out1[ci], "stats": stats1, "ones8": ones8, "onesr": onesr,
                "gamma": g1, "beta": be1, "ms": ms1, "biasr": bias1,
                "W1": W1, "b1r": _rep(b1), "W2": W2, "b2r": _rep(b2)}
               for ci in range(NCORES)]
    resE = _run(ncE, in_maps)

    y = np.empty((N, 2), np.float32)
    for ci in range(NCORES):
        valid = node_of_slot[ci] >= 0
        y[node_of_slot[ci][valid]] = resE.results[ci]["y"][valid]
    return y


# revision 6
# speedup vs baseline: 1.6135x; 1.0457x over previous
"""GATv2 (2-layer, GraphNorm, MLP head) on 8 Trainium2 NeuronCores — v2.

Design (vs the v1 edge-tile/one-hot-matmul kernel):
- dst-per-partition layout: each dst node owns one SBUF partition slot; its
  incoming edges lie along the free dimension.  Softmax and the weighted sum
  become free-dim tensor_reduce ops — no one-hot matmuls, no xr edge gather.
- Destinations are sorted by (lo_degree, hi_degree) and packed into chunks of
  1024 (128 partitions x 8 cores) so the rectangular edge padding stays small.
  Consecutive chunks merge into "windows" that share one dma_gather pair,
  amortizing the ~1us SWDGE fixed cost per gather.
- att is folded into the node tables (xl'' = att*xl): since lrelu is
  positively homogeneous and  min(x, .2x) = Prelu_{alpha=5}(0.2x),  the
  per-channel score term att_c*lrelu(v_c) becomes a plain Prelu over
  channels permuted pos-first per head.  This kills one full-size DVE pass.
  The aggregation output is un-scaled by 1/att at the end.
- Tables are fp16 (DVE runs 2x on 16-bit); scores skip the segment-max
  (exp never overflows here), pad edges point at a -1e4 table row so their
  exp underflows to exactly 0.
- conv bias + GraphNorm fold into the following launch's affine.

5 launches: A (layer-0 transforms), B0 (conv0), C (norm0+relu+layer-1
transforms), B1 (conv1), E (norm1+relu+MLP head).  Host work between
launches is index prep + memory movement only.
"""

import hashlib
import numpy as np

import concourse.bass as bass  # noqa: F401
import concourse.bacc as bacc
import concourse.tile as tile
from concourse import mybir
from concourse import bass_utils
from concourse.masks import make_identity

F32 = mybir.dt.float32
F16 = mybir.dt.float16
I16 = mybir.dt.int16
AF = mybir.ActivationFunctionType
ALU = mybir.AluOpType
AX = mybir.AxisListType

N, IN, H, C, E = 50000, 128, 2, 64, 800000
HC = H * C  # 128
NEG_SLOPE = 0.2
EPS_GN = 1e-5
NCORES = 8
P = 128

NCHUNK = 49                  # chunks of 1024 dsts (128 per core x 8)
SHARD_SLOTS = NCHUNK * P     # 6272 dst slots per core
NLO = 31360                  # nodes [0, NLO) gathered from the lo table
ROWS_LO = NLO + 1            # row 0 = pad(-1e4), node n -> row n+1
ROWS_HI = 50000 - NLO + 177  # 18817: nodes NLO.. at row n+1-ROWS_LO, spares, pad
ROWS_TOT = ROWS_LO + ROWS_HI  # 50178
PADHI_IDX = ROWS_HI - 1      # hi-local index of the hi pad row
PAD_VAL = -1e4
SBUF_CAP = 96                # max G*(Dlo+Dhi) per window
G_MAX = 8

_cache = {}


# ----------------------------------------------------------------------------
# host-side planning
# ----------------------------------------------------------------------------

def _wrap_idx_multi(buf):
    """[8, n] int16 -> [8, 128, n//16]: idx i -> [i%16, i//16], tiled x8."""
    nc_, n = buf.shape
    w = buf.reshape(nc_, n // 16, 16).transpose(0, 2, 1)  # [8, 16, n/16]
    return np.tile(w, (1, 8, 1))                          # [8, 128, n/16]


def build_plan(edge_index):
    ei = np.asarray(edge_index).astype(np.int64)
    loop = np.arange(N, dtype=np.int64)
    src = np.concatenate([ei[0], loop])
    dst = np.concatenate([ei[1], loop])
    is_lo = src < NLO

    lo_deg = np.bincount(dst[is_lo], minlength=N)
    hi_deg = np.bincount(dst[~is_lo], minlength=N)

    # Chunk packing (lo_deg and hi_deg are independent Poissons, so no 1D
    # sort bins both): lo-sorted bands of 7 chunks, hi-sorted within a band.
    # All chunks of a band share Dlo, so window-merging within a band only
    # maxes the (sorted, adjacent) Dhi values.
    order = np.lexsort((-lo_deg, -(hi_deg // 3)))
    rank = np.empty(N, np.int64)
    rank[order] = np.arange(N)
    chunk = rank // 1024
    within = rank % 1024
    core_of = within // P
    part_of = within % P

    ld = np.zeros(NCHUNK * 1024, np.int64)
    hd = np.zeros(NCHUNK * 1024, np.int64)
    ld[: N] = lo_deg[order]
    hd[: N] = hi_deg[order]
    Dlo_c = np.maximum(ld.reshape(NCHUNK, 1024).max(1), 1)
    Dhi_c = np.maximum(hd.reshape(NCHUNK, 1024).max(1), 1)

    # windows: merge consecutive chunks (sorted desc, so maxes come first)
    windows = []  # (g0, G, Dlo, Dhi)
    g = 0
    while g < NCHUNK:
        Dl, Dh = int(Dlo_c[g]), int(Dhi_c[g])
        G = 1
        waste = 0
        while G < G_MAX and g + G < NCHUNK:
            nl = max(Dl, int(Dlo_c[g + G]))
            nh = max(Dh, int(Dhi_c[g + G]))
            if (G + 1) * (nl + nh) > SBUF_CAP:
                break
            nw = (G + 1) * (nl + nh) - sum(
                int(Dlo_c[g + k] + Dhi_c[g + k]) for k in range(G + 1))
            if nw > 4:
                break
            Dl, Dh = nl, nh
            waste = nw
            G += 1
        windows.append((g, G, Dl, Dh))
        g += G

    # per-core flat idx buffer layout: [w0-lo | w0-hi | w1-lo | ...]
    base_lo = np.zeros(NCHUNK, np.int64)   # indexed by chunk
    base_hi = np.zeros(NCHUNK, np.int64)
    w_of_chunk = np.zeros(NCHUNK, np.int64)
    glocal = np.zeros(NCHUNK, np.int64)
    Dlo_w = np.zeros(NCHUNK, np.int64)     # per chunk: its window's Dlo
    Dhi_w = np.zeros(NCHUNK, np.int64)
    tot = 0
    for wi, (g0, G, Dl, Dh) in enumerate(windows):
        for k in range(G):
            ch = g0 + k
            w_of_chunk[ch] = wi
            glocal[ch] = k
            Dlo_w[ch] = Dl
            Dhi_w[ch] = Dh
            base_lo[ch] = tot
            base_hi[ch] = tot + G * Dl * P
        tot += G * (Dl + Dh) * P
    TOTI = tot

    # pad template (per window/region), then scatter real edges
    tmpl = np.empty(TOTI, np.int16)
    off = 0
    for (g0, G, Dl, Dh) in windows:
        tmpl[off: off + G * Dl * P] = 0          # lo pad row
        off += G * Dl * P
        tmpl[off: off + G * Dh * P] = PADHI_IDX  # hi pad row
        off += G * Dh * P
    buf = np.tile(tmpl, (NCORES, 1))

    for side in (0, 1):  # 0 = lo, 1 = hi
        mask = is_lo if side == 0 else ~is_lo
        es = np.nonzero(mask)[0]
        d_e = dst[es]
        o2 = np.argsort(d_e, kind="stable")
        es = es[o2]
        d_e = d_e[o2]
        first = np.searchsorted(d_e, np.arange(N))
        j = np.arange(len(es)) - first[d_e]
        ch = chunk[d_e]
        Dr = (Dlo_w if side == 0 else Dhi_w)[ch]
        base = (base_lo if side == 0 else base_hi)[ch]
        t = glocal[ch] * Dr + j
        pos = base + t * P + part_of[d_e]
        val = src[es] + 1 if side == 0 else src[es] + 1 - ROWS_LO
        buf[core_of[d_e], pos] = val.astype(np.int16)

    idx16 = _wrap_idx_multi(buf)  # [8, 128, TOTI//16]

    # slot maps
    node_of_slot = np.full((NCORES, SHARD_SLOTS), -1, np.int64)
    slot = chunk * P + part_of
    node_of_slot[core_of, slot] = np.arange(N)
    row_of_slot = np.empty((NCORES, SHARD_SLOTS), np.int64)
    pad_mask = node_of_slot < 0
    row_of_slot[~pad_mask] = node_of_slot[~pad_mask] + 1
    row_of_slot[pad_mask] = 50001 + np.arange(pad_mask.sum())  # spare rows

    real = float(len(src))
    return {
        "windows": windows, "TOTI": TOTI, "idx16": idx16,
        "node_of_slot": node_of_slot, "row_of_slot": row_of_slot,
        "pad_factor": TOTI / real,
    }


# ----------------------------------------------------------------------------
# kernel builders
# ----------------------------------------------------------------------------

def _new_nc(nq=1):
    return bacc.Bacc("TRN2", target_bir_lowering=False, num_swdge_queues=nq)


def build_transform():
    """Launch A: xl'' = x @ Wl'' + bl'', xr'' = x @ Wr'' + br'' (fp16 out)."""
    nc = _new_nc()
    x = nc.dram_tensor("x", [SHARD_SLOTS, IN], F32, kind="ExternalInput")
    Wl = nc.dram_tensor("Wl", [IN, HC], F16, kind="ExternalInput")
    Wr = nc.dram_tensor("Wr", [IN, HC], F16, kind="ExternalInput")
    blr = nc.dram_tensor("blr", [P, HC], F32, kind="ExternalInput")
    brr = nc.dram_tensor("brr", [P, HC], F32, kind="ExternalInput")
    xl = nc.dram_tensor("xl", [SHARD_SLOTS, HC], F16, kind="ExternalOutput")
    xr = nc.dram_tensor("xr", [SHARD_SLOTS, HC], F16, kind="ExternalOutput")

    with tile.TileContext(nc) as tc:
        with (
            tc.tile_pool(name="const", bufs=1) as cpool,
            tc.tile_pool(name="sbuf", bufs=3) as pool,
            tc.tile_pool(name="psum", bufs=2, space="PSUM") as psum,
        ):
            ident = cpool.tile([P, P], F16)
            make_identity(nc, ident[:])
            wl_t = cpool.tile([IN, HC], F16)
            wr_t = cpool.tile([IN, HC], F16)
            bl_t = cpool.tile([P, HC], F32)
            br_t = cpool.tile([P, HC], F32)
            nc.sync.dma_start(out=wl_t[:], in_=Wl[:, :])
            nc.sync.dma_start(out=wr_t[:], in_=Wr[:, :])
            nc.sync.dma_start(out=bl_t[:], in_=blr[:, :])
            nc.sync.dma_start(out=br_t[:], in_=brr[:, :])

            for t in range(NCHUNK):
                xt = pool.tile([P, IN], F32, tag="xt")
                nc.sync.dma_start(out=xt[:], in_=x[t * P:(t + 1) * P, :])
                xt16 = pool.tile([P, IN], F16, tag="xt16")
                nc.vector.tensor_copy(out=xt16[:], in_=xt[:])
                xT_ps = psum.tile([P, P], F16, tag="xT")
                nc.tensor.transpose(xT_ps[:], xt16[:], ident[:])
                xT = pool.tile([P, P], F16, tag="xTs")
                nc.vector.tensor_copy(out=xT[:], in_=xT_ps[:])
                for (w_t, b_t, out_d, tag) in ((wl_t, bl_t, xl, "l"),
                                               (wr_t, br_t, xr, "r")):
                    ps = psum.tile([P, HC], F32, tag="mm" + tag)
                    nc.tensor.matmul(ps[:], xT[:], w_t[:], start=True, stop=True)
                    ot = pool.tile([P, HC], F16, tag="ot" + tag)
                    nc.vector.tensor_add(out=ot[:], in0=ps[:], in1=b_t[:])
                    nc.sync.dma_start(out=out_d[t * P:(t + 1) * P, :], in_=ot[:])
    nc.finalize()
    return nc


def build_conv(windows, TOTI, ranges):
    """Launch B: GATv2 conv, dst-per-partition layout.

    ranges = (p0, p1): count of positive-att channels per head (channels are
    host-permuted pos-first within each head).
    """
    p0, p1 = ranges
    nc = _new_nc(nq=4)
    tlo = nc.dram_tensor("tlo", [ROWS_LO, HC], F16, kind="ExternalInput")
    thi = nc.dram_tensor("thi", [ROWS_HI, HC], F16, kind="ExternalInput")
    xr_d = nc.dram_tensor("xr", [P, NCHUNK * HC], F16, kind="ExternalInput")
    idx_d = nc.dram_tensor("idx16", [P, TOTI // 16], I16, kind="ExternalInput")
    invatt = nc.dram_tensor("invatt", [P, HC], F32, kind="ExternalInput")
    onescol = nc.dram_tensor("onescol", [P, 1], F32, kind="ExternalInput")
    out_d = nc.dram_tensor("out", [P, NCHUNK * HC], F16, kind="ExternalOutput")
    stats = nc.dram_tensor("stats", [1, 2 * HC], F32, kind="ExternalOutput")

    # activation ranges in c-major space: head h occupies (c, h) columns;
    # pos channels are c < p_h.  (c0, clen, h, alpha)
    act_ranges = []
    for h, pp in ((0, p0), (1, p1)):
        if pp > 0:
            act_ranges.append((0, pp, h, NEG_SLOPE))
        if pp < C:
            act_ranges.append((pp, C - pp, h, 5.0))

    NW = len(windows)
    with tile.TileContext(nc) as tc:
        with (
            tc.tile_pool(name="const", bufs=1) as cpool,
            tc.tile_pool(name="gath", bufs=4) as gpool,
            tc.tile_pool(name="work", bufs=2) as pool,
            tc.tile_pool(name="oh", bufs=3) as ohpool,
            tc.tile_pool(name="pstat", bufs=1, space="PSUM") as pstat,
        ):
            inv_t = cpool.tile([P, HC], F32)
            ones_t = cpool.tile([P, 1], F32)
            acc = cpool.tile([P, 2 * HC], F32)
            eps_t = cpool.tile([P, 1], F32)
            nc.sync.dma_start(out=inv_t[:], in_=invatt[:, :])
            nc.sync.dma_start(out=ones_t[:], in_=onescol[:, :])
            nc.vector.memset(acc[:], 0.0)
            nc.vector.memset(eps_t[:], 1e-16)

            state = {}  # per-window live tiles

            def emit_load(i):
                g0, G, Dl, Dh = windows[i]
                nlo, nhi = G * Dl * P, G * Dh * P
                ioff = sum(w[1] * (w[2] + w[3]) * P for w in windows[:i]) // 16
                ilo = gpool.tile([P, nlo // 16], I16, tag="ilo")
                ihi = gpool.tile([P, nhi // 16], I16, tag="ihi")
                nc.sync.dma_start(out=ilo[:], in_=idx_d[:, ioff: ioff + nlo // 16])
                nc.sync.dma_start(
                    out=ihi[:], in_=idx_d[:, ioff + nlo // 16: ioff + (nlo + nhi) // 16])
                glo = gpool.tile([P, G * Dl, HC], F16, tag="glo")
                ghi = gpool.tile([P, G * Dh, HC], F16, tag="ghi")
                nc.gpsimd.dma_gather(glo[:], tlo[:, :], ilo[:], nlo, nlo, HC,
                                     single_packet=False,
                                     queue_num=(2 * i) % 4)
                nc.gpsimd.dma_gather(ghi[:], thi[:, :], ihi[:], nhi, nhi, HC,
                                     single_packet=False,
                                     queue_num=(2 * i + 1) % 4)
                xrw = gpool.tile([P, G, HC], F16, tag="xrw")
                nc.sync.dma_start(out=xrw[:], in_=xr_d[:, g0 * HC:(g0 + G) * HC])
                state[i] = {"glo": glo, "ghi": ghi, "xrw": xrw}

            def emit_add_prelu(i):
                g0, G, Dl, Dh = windows[i]
                st = state[i]
                for (reg, Dr) in (("lo", Dl), ("hi", Dh)):
                    xlg = st["g" + reg]
                    v = pool.tile([P, G * Dr, HC], F16, tag="v" + reg)
                    xr_b = st["xrw"][:].unsqueeze(2).broadcast_to([P, G, Dr, HC])
                    nc.vector.tensor_add(
                        out=v[:].rearrange("p (g d) c -> p g d c", g=G),
                        in0=xlg[:].rearrange("p (g d) c -> p g d c", g=G),
                        in1=xr_b)
                    vv = v[:].rearrange("p (g d) (c h) -> p g d c h",
                                        g=G, h=H)
                    for (c0, ln, h, alpha) in act_ranges:
                        sl = vv[:, :, :, c0:c0 + ln, h:h + 1]
                        scale = 1.0 if alpha == NEG_SLOPE else NEG_SLOPE
                        nc.scalar.activation(sl, sl, AF.Prelu, scale=scale,
                                             alpha=alpha)
                    st["v" + reg] = v

            def emit_scores(i):
                g0, G, Dl, Dh = windows[i]
                st = state[i]
                for (reg, Dr) in (("lo", Dl), ("hi", Dh)):
                    v = st["v" + reg]
                    # c-major makes c-halves contiguous: 3D slab adds
                    vv = v[:]  # [P, G*Dr, HC]
                    cur = C
                    while cur > 1:
                        half = cur // 2
                        nc.vector.tensor_tensor(
                            out=vv[:, :, 0:half * H],
                            in0=vv[:, :, 0:half * H],
                            in1=vv[:, :, half * H:cur * H], op=ALU.add)
                        cur = half
                    pex = pool.tile([P, G, Dr, H], F16, tag="pex" + reg)
                    nc.scalar.activation(
                        pex[:], vv[:, :, 0:H].rearrange(
                            "p (g d) h -> p g d h", g=G), AF.Exp)
                    st["pex" + reg] = pex

            def emit_main(i):
                g0, G, Dl, Dh = windows[i]
                st = state[i]
                den = pool.tile([P, G, H], F32, tag="den")
                nc.vector.tensor_reduce(
                    out=den[:],
                    in_=st["pexlo"][:].rearrange("p g d h -> p g h d"),
                    axis=AX.X, op=ALU.add)
                den2 = pool.tile([P, G, H], F32, tag="den2")
                nc.vector.tensor_reduce(
                    out=den2[:],
                    in_=st["pexhi"][:].rearrange("p g d h -> p g h d"),
                    axis=AX.X, op=ALU.add)
                nc.vector.tensor_add(out=den[:], in0=den[:], in1=den2[:])
                rec = pool.tile([P, G, H], F32, tag="rec")
                nc.scalar.activation(den[:], den[:], AF.Identity,
                                     bias=eps_t[:])
                nc.vector.reciprocal(out=rec[:], in_=den[:])
                for (reg, Dr) in (("lo", Dl), ("hi", Dh)):
                    xlg = st["g" + reg]
                    t_r = st["v" + reg]  # overwrite (dead after scores)
                    pex = st["pex" + reg]
                    p_b = pex[:].rearrange("p g d h -> p (g d) h") \
                        .unsqueeze(2).broadcast_to([P, G * Dr, C, H])
                    nc.vector.tensor_mul(
                        out=t_r[:].rearrange("p g (c h) -> p g c h", h=H),
                        in0=xlg[:].rearrange("p g (c h) -> p g c h", h=H),
                        in1=p_b)
                    # pairwise tree over D (odd tail folded into the front)
                    tv = t_r[:].rearrange("p (g d) c -> p g (d c)", g=G)
                    cur = Dr
                    while cur > 1:
                        half = cur // 2
                        rem = cur - 2 * half
                        if rem:
                            nc.vector.tensor_tensor(
                                out=tv[:, :, 0:rem * HC],
                                in0=tv[:, :, 0:rem * HC],
                                in1=tv[:, :, 2 * half * HC:cur * HC],
                                op=ALU.add)
                        nc.vector.tensor_tensor(
                            out=tv[:, :, 0:half * HC],
                            in0=tv[:, :, 0:half * HC],
                            in1=tv[:, :, half * HC:(cur - rem) * HC],
                            op=ALU.add)
                        cur = half
                osum = pool.tile([P, G, HC], F32, tag="osum")
                nc.vector.tensor_add(
                    out=osum[:],
                    in0=st["vlo"][:].rearrange("p (g d) c -> p g d c", g=G)
                    [:, :, 0, :],
                    in1=st["vhi"][:].rearrange("p (g d) c -> p g d c", g=G)
                    [:, :, 0, :])
                rec_b = rec[:].rearrange("p g h -> p g h").unsqueeze(2) \
                    .broadcast_to([P, G, C, H])
                nc.vector.tensor_mul(
                    out=osum[:].rearrange("p g (c h) -> p g c h", c=C),
                    in0=osum[:].rearrange("p g (c h) -> p g c h", c=C),
                    in1=rec_b)
                oh = ohpool.tile([P, G, HC], F16, tag="oh")
                inv_b = inv_t[:].unsqueeze(1).broadcast_to([P, G, HC])
                nc.vector.tensor_mul(out=oh[:], in0=osum[:], in1=inv_b)
                nc.sync.dma_start(out=out_d[:, g0 * HC:(g0 + G) * HC],
                                  in_=oh[:].rearrange("p g c -> p (g c)"))
                st["oh"] = oh
                for k in ("glo", "ghi", "vlo", "vhi", "pexlo", "pexhi"):
                    st.pop(k, None)

            def emit_stats(i):
                g0, G, Dl, Dh = windows[i]
                st = state.pop(i)
                oh = st["oh"]
                sq = pool.tile([P, G, HC], F32, tag="sq")
                nc.scalar.activation(sq[:], oh[:], AF.Square)
                s1 = pool.tile([P, HC], F32, tag="s1")
                nc.vector.tensor_reduce(
                    out=s1[:], in_=oh[:].rearrange("p g c -> p c g"),
                    axis=AX.X, op=ALU.add)
                nc.vector.tensor_add(out=acc[:, 0:HC], in0=acc[:, 0:HC],
                                     in1=s1[:])
                s2 = pool.tile([P, HC], F32, tag="s2")
                nc.vector.tensor_reduce(
                    out=s2[:], in_=sq[:].rearrange("p g c -> p c g"),
                    axis=AX.X, op=ALU.add)
                nc.vector.tensor_add(out=acc[:, HC:2 * HC], in0=acc[:, HC:2 * HC],
                                     in1=s2[:])

            # software-pipelined emission.  Per-iteration engine-queue order is
            # chosen so ACT's exp(i-1) precedes the 8 prelus(i) (else the DVE
            # wmults of window i-1 would stall ~5us behind them), and gathers
            # run one window ahead of their adds.
            emit_load(0)
            if NW > 1:
                emit_load(1)
            for i in range(NW + 2):
                if i + 2 < NW:
                    emit_load(i + 2)
                if 1 <= i <= NW:
                    emit_scores(i - 1)
                if i < NW:
                    emit_add_prelu(i)
                if 1 <= i <= NW:
                    emit_main(i - 1)
                if 2 <= i <= NW + 1:
                    emit_stats(i - 2)

            st_ps = pstat.tile([1, 2 * HC], F32, tag="st")
            nc.tensor.matmul(st_ps[:], ones_t[:], acc[:], start=True, stop=True)
            stt = pool.tile([1, 2 * HC], F32, tag="stt")
            nc.vector.tensor_copy(out=stt[:], in_=st_ps[:])
            nc.sync.dma_start(out=stats[:, :], in_=stt[:])
    nc.finalize()
    return nc


def _emit_norm_prelude(nc, cpool, pconst, stats, ones8, onesr, gamma, beta, ms,
                       biasr):
    """Common GraphNorm-affine computation with conv-bias folding.

    Returns (a_rep, b_rep): normalized = a_rep * o' + b_rep where o' is the
    bias-less conv output."""
    st8 = cpool.tile([NCORES, 2 * HC], F32)
    o8 = cpool.tile([NCORES, 1], F32)
    orow = cpool.tile([1, P], F32)
    g_t = cpool.tile([1, HC], F32)
    be_t = cpool.tile([1, HC], F32)
    ms_t = cpool.tile([1, HC], F32)
    bi_t = cpool.tile([1, HC], F32)
    nc.sync.dma_start(out=st8[:], in_=stats[:, :])
    nc.sync.dma_start(out=o8[:], in_=ones8[:, :])
    nc.sync.dma_start(out=orow[:], in_=onesr[:, :])
    nc.sync.dma_start(out=g_t[:], in_=gamma[:, :])
    nc.sync.dma_start(out=be_t[:], in_=beta[:, :])
    nc.sync.dma_start(out=ms_t[:], in_=ms[:, :])
    nc.sync.dma_start(out=bi_t[:], in_=biasr[:, :])

    sg_ps = pconst.tile([1, 2 * HC], F32, tag="sg")
    nc.tensor.matmul(sg_ps[:], o8[:], st8[:], start=True, stop=True)
    # mean_o = S1/N ; mean_y = mean_o + bias
    mean = cpool.tile([1, HC], F32)
    nc.scalar.mul(mean[:], sg_ps[:, 0:HC], 1.0 / N)
    mean_y = cpool.tile([1, HC], F32)
    nc.vector.tensor_add(out=mean_y[:], in0=mean[:], in1=bi_t[:])
    # E[y^2] = S2/N + bias*(2*mean_o + bias)
    ey2 = cpool.tile([1, HC], F32)
    nc.scalar.mul(ey2[:], sg_ps[:, HC:2 * HC], 1.0 / N)
    t1 = cpool.tile([1, HC], F32)
    nc.scalar.mul(t1[:], mean[:], 2.0)
    nc.vector.tensor_add(out=t1[:], in0=t1[:], in1=bi_t[:])
    nc.vector.tensor_mul(out=t1[:], in0=t1[:], in1=bi_t[:])
    nc.vector.tensor_add(out=ey2[:], in0=ey2[:], in1=t1[:])
    # var = E[y^2] - ms*(2-ms)*mean_y^2
    two_b = cpool.tile([1, 1], F32)
    nc.vector.memset(two_b[:], 2.0)
    eps_b = cpool.tile([1, 1], F32)
    nc.vector.memset(eps_b[:], EPS_GN)
    two_minus = cpool.tile([1, HC], F32)
    nc.scalar.activation(two_minus[:], ms_t[:], AF.Identity, bias=two_b[:],
                         scale=-1.0)
    msm = cpool.tile([1, HC], F32)
    nc.vector.tensor_mul(out=msm[:], in0=two_minus[:], in1=ms_t[:])
    m2 = cpool.tile([1, HC], F32)
    nc.vector.tensor_mul(out=m2[:], in0=mean_y[:], in1=mean_y[:])
    var = cpool.tile([1, HC], F32)
    nc.vector.tensor_mul(out=var[:], in0=m2[:], in1=msm[:])
    nc.vector.tensor_tensor(out=var[:], in0=ey2[:], in1=var[:],
                            op=ALU.subtract)
    nc.scalar.activation(var[:], var[:], AF.Identity, bias=eps_b[:])
    sd = cpool.tile([1, HC], F32)
    nc.scalar.activation(sd[:], var[:], AF.Sqrt)
    rsd = cpool.tile([1, HC], F32)
    nc.vector.reciprocal(out=rsd[:], in_=sd[:])
    arow = cpool.tile([1, HC], F32)      # A = gamma * rsd
    nc.vector.tensor_mul(out=arow[:], in0=g_t[:], in1=rsd[:])
    brow = cpool.tile([1, HC], F32)      # B = beta - A*ms*mean_y
    nc.vector.tensor_mul(out=brow[:], in0=arow[:], in1=ms_t[:])
    nc.vector.tensor_mul(out=brow[:], in0=brow[:], in1=mean_y[:])
    nc.vector.tensor_tensor(out=brow[:], in0=be_t[:], in1=brow[:],
                            op=ALU.subtract)
    # fold: normalized = A*(o'+bias) + B = A*o' + (A*bias + B)
    b2row = cpool.tile([1, HC], F32)
    nc.vector.tensor_mul(out=b2row[:], in0=arow[:], in1=bi_t[:])
    nc.vector.tensor_add(out=b2row[:], in0=b2row[:], in1=brow[:])
    # broadcast to [P, HC]
    a_ps = pconst.tile([P, HC], F32, tag="arep")
    b_ps = pconst.tile([P, HC], F32, tag="brep")
    nc.tensor.matmul(a_ps[:], orow[:], arow[:], start=True, stop=True)
    nc.tensor.matmul(b_ps[:], orow[:], b2row[:], start=True, stop=True)
    a_rep = cpool.tile([P, HC], F32)
    b_rep = cpool.tile([P, HC], F32)
    nc.vector.tensor_copy(out=a_rep[:], in_=a_ps[:])
    nc.vector.tensor_copy(out=b_rep[:], in_=b_ps[:])
    return a_rep, b_rep


def build_norm_transform():
    """Launch C: h = relu(norm(out0+bias)); xl1'' = h@Wl1''+bl1''; xr1''."""
    nc = _new_nc()
    x = nc.dram_tensor("x", [P, NCHUNK * HC], F16, kind="ExternalInput")
    stats = nc.dram_tensor("stats", [NCORES, 2 * HC], F32, kind="ExternalInput")
    ones8 = nc.dram_tensor("ones8", [NCORES, 1], F32, kind="ExternalInput")
    onesr = nc.dram_tensor("onesr", [1, P], F32, kind="ExternalInput")
    gamma = nc.dram_tensor("gamma", [1, HC], F32, kind="ExternalInput")
    beta = nc.dram_tensor("beta", [1, HC], F32, kind="ExternalInput")
    ms = nc.dram_tensor("ms", [1, HC], F32, kind="ExternalInput")
    biasr = nc.dram_tensor("biasr", [1, HC], F32, kind="ExternalInput")
    Wl = nc.dram_tensor("Wl", [HC, HC], F16, kind="ExternalInput")
    Wr = nc.dram_tensor("Wr", [HC, HC], F16, kind="ExternalInput")
    blr = nc.dram_tensor("blr", [P, HC], F32, kind="ExternalInput")
    brr = nc.dram_tensor("brr", [P, HC], F32, kind="ExternalInput")
    xl = nc.dram_tensor("xl", [SHARD_SLOTS, HC], F16, kind="ExternalOutput")
    xr = nc.dram_tensor("xr", [SHARD_SLOTS, HC], F16, kind="ExternalOutput")

    with tile.TileContext(nc) as tc:
        with (
            tc.tile_pool(name="const", bufs=1) as cpool,
            tc.tile_pool(name="sbuf", bufs=3) as pool,
            tc.tile_pool(name="psum", bufs=2, space="PSUM") as psum,
            tc.tile_pool(name="pconst", bufs=1, space="PSUM") as pconst,
        ):
            ident = cpool.tile([P, P], F16)
            make_identity(nc, ident[:])
            wl_t = cpool.tile([HC, HC], F16)
            wr_t = cpool.tile([HC, HC], F16)
            bl_t = cpool.tile([P, HC], F32)
            br_t = cpool.tile([P, HC], F32)
            nc.sync.dma_start(out=wl_t[:], in_=Wl[:, :])
            nc.sync.dma_start(out=wr_t[:], in_=Wr[:, :])
            nc.sync.dma_start(out=bl_t[:], in_=blr[:, :])
            nc.sync.dma_start(out=br_t[:], in_=brr[:, :])
            a_rep, b_rep = _emit_norm_prelude(
                nc, cpool, pconst, stats, ones8, onesr, gamma, beta, ms, biasr)

            for t in range(NCHUNK):
                xt = pool.tile([P, HC], F16, tag="xt")
                nc.sync.dma_start(out=xt[:], in_=x[:, t * HC:(t + 1) * HC])
                hn = pool.tile([P, HC], F32, tag="hn")
                nc.vector.tensor_mul(out=hn[:], in0=xt[:], in1=a_rep[:])
                nc.vector.tensor_add(out=hn[:], in0=hn[:], in1=b_rep[:])
                hn16 = pool.tile([P, HC], F16, tag="hn16")
                nc.scalar.activation(hn16[:], hn[:], AF.Relu)
                xT_ps = psum.tile([P, P], F16, tag="xT")
                nc.tensor.transpose(xT_ps[:], hn16[:], ident[:])
                xT = pool.tile([P, P], F16, tag="xTs")
                nc.vector.tensor_copy(out=xT[:], in_=xT_ps[:])
                ps = psum.tile([P, 2 * HC], F32, tag="mm")
                nc.tensor.matmul(ps[:, 0:HC], xT[:], wl_t[:], start=True,
                                 stop=True)
                nc.tensor.matmul(ps[:, HC:2 * HC], xT[:], wr_t[:], start=True,
                                 stop=True)
                for (b_t, out_dd, sl, tag) in ((bl_t, xl, slice(0, HC), "l"),
                                               (br_t, xr, slice(HC, 2 * HC), "r")):
                    ot = pool.tile([P, HC], F16, tag="ot" + tag)
                    nc.vector.tensor_add(out=ot[:], in0=ps[:, sl], in1=b_t[:])
                    nc.sync.dma_start(out=out_dd[t * P:(t + 1) * P, :], in_=ot[:])
    nc.finalize()
    return nc


def build_norm_mlp():
    """Launch E: h = relu(norm(out1+bias)); y = relu(h@W1+b1)@W2+b2."""
    nc = _new_nc()
    x = nc.dram_tensor("x", [P, NCHUNK * HC], F16, kind="ExternalInput")
    stats = nc.dram_tensor("stats", [NCORES, 2 * HC], F32, kind="ExternalInput")
    ones8 = nc.dram_tensor("ones8", [NCORES, 1], F32, kind="ExternalInput")
    onesr = nc.dram_tensor("onesr", [1, P], F32, kind="ExternalInput")
    gamma = nc.dram_tensor("gamma", [1, HC], F32, kind="ExternalInput")
    beta = nc.dram_tensor("beta", [1, HC], F32, kind="ExternalInput")
    ms = nc.dram_tensor("ms", [1, HC], F32, kind="ExternalInput")
    biasr = nc.dram_tensor("biasr", [1, HC], F32, kind="ExternalInput")
    W1 = nc.dram_tensor("W1", [HC, C], F16, kind="ExternalInput")
    b1r = nc.dram_tensor("b1r", [P, C], F32, kind="ExternalInput")
    W2 = nc.dram_tensor("W2", [C, 2], F16, kind="ExternalInput")
    b2r = nc.dram_tensor("b2r", [P, 2], F32, kind="ExternalInput")
    y = nc.dram_tensor("y", [SHARD_SLOTS, 2], F32, kind="ExternalOutput")

    with tile.TileContext(nc) as tc:
        with (
            tc.tile_pool(name="const", bufs=1) as cpool,
            tc.tile_pool(name="sbuf", bufs=3) as pool,
            tc.tile_pool(name="psum", bufs=2, space="PSUM") as psum,
            tc.tile_pool(name="pone", bufs=1, space="PSUM") as pone,
            tc.tile_pool(name="pconst", bufs=1, space="PSUM") as pconst,
        ):
            ident = cpool.tile([P, P], F16)
            make_identity(nc, ident[:])
            w1_t = cpool.tile([HC, C], F16)
            b1_t = cpool.tile([P, C], F32)
            w2_t = cpool.tile([C, 2], F16)
            b2_t = cpool.tile([P, 2], F32)
            nc.sync.dma_start(out=w1_t[:], in_=W1[:, :])
            nc.sync.dma_start(out=b1_t[:], in_=b1r[:, :])
            nc.sync.dma_start(out=w2_t[:], in_=W2[:, :])
            nc.sync.dma_start(out=b2_t[:], in_=b2r[:, :])
            a_rep, b_rep = _emit_norm_prelude(
                nc, cpool, pconst, stats, ones8, onesr, gamma, beta, ms, biasr)

            for t in range(NCHUNK):
                xt = pool.tile([P, HC], F16, tag="xt")
                nc.sync.dma_start(out=xt[:], in_=x[:, t * HC:(t + 1) * HC])
                hn = pool.tile([P, HC], F32, tag="hn")
                nc.vector.tensor_mul(out=hn[:], in0=xt[:], in1=a_rep[:])
                nc.vector.tensor_add(out=hn[:], in0=hn[:], in1=b_rep[:])
                hn16 = pool.tile([P, HC], F16, tag="hn16")
                nc.scalar.activation(hn16[:], hn[:], AF.Relu)
                xT_ps = psum.tile([P, P], F16, tag="xT")
                nc.tensor.transpose(xT_ps[:], hn16[:], ident[:])
                xT = pool.tile([P, P], F16, tag="xTs")
                nc.vector.tensor_copy(out=xT[:], in_=xT_ps[:])
                z_ps = pone.tile([P, C], F32, tag="z")
                nc.tensor.matmul(z_ps[:], xT[:], w1_t[:], start=True, stop=True)
                z = pool.tile([P, C], F32, tag="zs")
                nc.vector.tensor_add(out=z[:], in0=z_ps[:], in1=b1_t[:])
                z16 = pool.tile([P, C], F16, tag="z16")
                nc.scalar.activation(z16[:], z[:], AF.Relu)
                zT_ps = pone.tile([C, P], F16, tag="zT")
                nc.tensor.transpose(zT_ps[:], z16[:], ident[:])
                zT = pool.tile([C, P], F16, tag="zTs")
                nc.vector.tensor_copy(out=zT[:], in_=zT_ps[:])
                y_ps = pone.tile([P, 2], F32, tag="y")
                nc.tensor.matmul(y_ps[:], zT[:], w2_t[:], start=True, stop=True)
                yt = pool.tile([P, 2], F32, tag="yt")
                nc.vector.tensor_add(out=yt[:], in0=y_ps[:], in1=b2_t[:])
                nc.sync.dma_start(out=y[t * P:(t + 1) * P, :], in_=yt[:])
    nc.finalize()
    return nc


# ----------------------------------------------------------------------------
# host orchestration
# ----------------------------------------------------------------------------

TRACE = False
LAST_EXEC_NS = []


def _run(nc, in_maps, trace=None):
    trace = TRACE if trace is None else trace
    last_err = None
    for attempt in range(3):
        try:
            res = bass_utils.run_bass_kernel_spmd(
                nc, in_maps, core_ids=list(range(NCORES)), trace=trace)
            LAST_EXEC_NS.append(res.exec_time_ns)
            return res
        except Exception as e:
            last_err = e
            import time as _t
            _t.sleep(2.0 * (attempt + 1))
    raise last_err


def _rep(v):
    v = np.asarray(v, np.float32).reshape(1, -1)
    return np.tile(v, (P, 1))


def _head_perm(att):
    """Channel order: c-major head-interleaved (col = c*H + h), pos-att-first
    within each head.  Keeps the innermost stride of per-(edge,head)-scalar
    broadcasts at 1 so DVE 2x applies.  Returns (perm, (p0, p1))."""
    att = np.asarray(att, np.float32).reshape(H, C)
    heads = []
    counts = []
    for h in range(H):
        pos = np.nonzero(att[h] > 0)[0]
        neg = np.nonzero(att[h] <= 0)[0]
        heads.append(np.concatenate([pos, neg]) + h * C)
        counts.append(len(pos))
    perm = np.empty(HC, np.int64)
    for c in range(C):
        for h in range(H):
            perm[c * H + h] = heads[h][c]
    return perm, tuple(counts)


def _assemble_table(xl_shards, row_of_slot):
    tbl = np.empty((ROWS_TOT, HC), np.float16)
    tbl[0] = PAD_VAL
    tbl[ROWS_TOT - 1] = PAD_VAL
    allrows = row_of_slot.reshape(-1)
    tbl[allrows] = np.concatenate(xl_shards, axis=0)
    return tbl


def kernel(**inputs):
    LAST_EXEC_NS.clear()
    x = np.asarray(inputs["x"], np.float32)
    edge_index = np.asarray(inputs["edge_index"])
    key = hashlib.sha1(np.ascontiguousarray(edge_index).tobytes()).hexdigest()

    att0 = np.asarray(inputs["att0"], np.float32).reshape(-1)
    att1 = np.asarray(inputs["att1"], np.float32).reshape(-1)
    pi0, r0 = _head_perm(att0)
    pi1, r1 = _head_perm(att1)

    if _cache.get("edge_key") != key:
        plan = build_plan(edge_index)
        _cache.clear()
        _cache["edge_key"] = key
        _cache["plan"] = plan
        _cache["ncA"] = build_transform()
        _cache["ncC"] = build_norm_transform()
        _cache["ncE"] = build_norm_mlp()
    plan = _cache["plan"]
    if _cache.get("r0") != r0:
        _cache["ncB0"] = build_conv(plan["windows"], plan["TOTI"], r0)
        _cache["r0"] = r0
    if _cache.get("r1") != r1:
        if r1 == r0:
            _cache["ncB1"] = _cache["ncB0"]
        else:
            _cache["ncB1"] = build_conv(plan["windows"], plan["TOTI"], r1)
        _cache["r1"] = r1
    ncA, ncB0, ncC, ncB1, ncE = (_cache["ncA"], _cache["ncB0"], _cache["ncC"],
                                 _cache["ncB1"], _cache["ncE"])

    node_of_slot = plan["node_of_slot"]
    row_of_slot = plan["row_of_slot"]

    # ---- host weight prep (channel perms + att folding) ----
    a0p = att0[pi0]
    a1p = att1[pi1]
    inv0 = _rep(1.0 / a0p)
    inv1 = _rep(1.0 / a1p)

    Wl0 = (np.asarray(inputs["Wl0"], np.float32)[:, pi0] * a0p).astype(np.float16)
    Wr0 = (np.asarray(inputs["Wr0"], np.float32)[:, pi0] * a0p).astype(np.float16)
    bl0 = np.asarray(inputs["bl0"], np.float32)[pi0] * a0p
    br0 = np.asarray(inputs["br0"], np.float32)[pi0] * a0p
    # layer-1 weights: rows in pi0 space (h lives there), cols pi1+att1-scaled
    Wl1 = (np.asarray(inputs["Wl1"], np.float32)[pi0][:, pi1] * a1p).astype(np.float16)
    Wr1 = (np.asarray(inputs["Wr1"], np.float32)[pi0][:, pi1] * a1p).astype(np.float16)
    bl1 = np.asarray(inputs["bl1"], np.float32)[pi1] * a1p
    br1 = np.asarray(inputs["br1"], np.float32)[pi1] * a1p
    W1 = np.asarray(inputs["W1"], np.float32)[pi1].astype(np.float16)
    b1 = np.asarray(inputs["b1"], np.float32)
    W2 = np.asarray(inputs["W2"], np.float32).astype(np.float16)
    b2 = np.asarray(inputs["b2"], np.float32)

    g0 = np.asarray(inputs["g0"], np.float32)[pi0].reshape(1, HC)
    be0 = np.asarray(inputs["be0"], np.float32)[pi0].reshape(1, HC)
    ms0 = np.asarray(inputs["ms0"], np.float32)[pi0].reshape(1, HC)
    bias0 = np.asarray(inputs["bias0"], np.float32)[pi0].reshape(1, HC)
    g1 = np.asarray(inputs["g1"], np.float32)[pi1].reshape(1, HC)
    be1 = np.asarray(inputs["be1"], np.float32)[pi1].reshape(1, HC)
    ms1 = np.asarray(inputs["ms1"], np.float32)[pi1].reshape(1, HC)
    bias1 = np.asarray(inputs["bias1"], np.float32)[pi1].reshape(1, HC)

    ones8 = np.ones((NCORES, 1), np.float32)
    onesr = np.ones((1, P), np.float32)
    onescol = np.ones((P, 1), np.float32)

    # ---- launch A: layer-0 transforms ----
    x_slots = [x[np.clip(node_of_slot[ci], 0, N - 1)] for ci in range(NCORES)]
    in_maps = [{"x": x_slots[ci], "Wl": Wl0, "Wr": Wr0,
                "blr": _rep(bl0), "brr": _rep(br0)} for ci in range(NCORES)]
    resA = _run(ncA, in_maps)
    xl_sh = [resA.results[ci]["xl"] for ci in range(NCORES)]
    xr_sh = [resA.results[ci]["xr"] for ci in range(NCORES)]

    def conv(ncB, xl_shards, xr_shards, inv):
        tbl = _assemble_table(xl_shards, row_of_slot)
        tlo = np.ascontiguousarray(tbl[:ROWS_LO])
        thi = np.ascontiguousarray(tbl[ROWS_LO:])
        in_maps = []
        for ci in range(NCORES):
            xr_pm = np.ascontiguousarray(
                xr_shards[ci].reshape(NCHUNK, P, HC).transpose(1, 0, 2)
                .reshape(P, NCHUNK * HC))
            in_maps.append({
                "tlo": tlo, "thi": thi, "xr": xr_pm,
                "idx16": plan["idx16"][ci], "invatt": inv,
                "onescol": onescol,
            })
        res = _run(ncB, in_maps)
        outs = [res.results[ci]["out"] for ci in range(NCORES)]
        stats = np.concatenate([res.results[ci]["stats"] for ci in range(NCORES)],
                               axis=0)
        return outs, stats

    out0, stats0 = conv(ncB0, xl_sh, xr_sh, inv0)

    # ---- launch C: norm0 + relu + layer-1 transforms ----
    in_maps = [{"x": out0[ci], "stats": stats0, "ones8": ones8, "onesr": onesr,
                "gamma": g0, "beta": be0, "ms": ms0, "biasr": bias0,
                "Wl": Wl1, "Wr": Wr1, "blr": _rep(bl1), "brr": _rep(br1)}
               for ci in range(NCORES)]
    resC = _run(ncC, in_maps)
    xl1_sh = [resC.results[ci]["xl"] for ci in range(NCORES)]
    xr1_sh = [resC.results[ci]["xr"] for ci in range(NCORES)]

    out1, stats1 = conv(ncB1, xl1_sh, xr1_sh, inv1)

    # ---- launch E: norm1 + relu + MLP ----
    in_maps = [{"x": out1[ci], "stats": stats1, "ones8": ones8, "onesr": onesr,
                "gamma": g1, "beta": be1, "ms": ms1, "biasr": bias1,
                "W1": W1, "b1r": _rep(b1), "W2": W2, "b2r": _rep(b2)}
               for ci in range(NCORES)]
    resE = _run(ncE, in_maps)

    y = np.empty((N, 2), np.float32)
    for ci in range(NCORES):
        valid = node_of_slot[ci] >= 0
        y[node_of_slot[ci][valid]] = resE.results[ci]["y"][valid]
    return y
